# revision 41
# baseline (speedup 1.0000x reference)
"""Causal self-attention (B=4, S=2048, D=768, H=12) on 8 trn2 NeuronCores.

Sharding: core c -> (batch b = c//2, head-half hh = c%2). Each core handles
one batch and 6 of the 12 heads: it computes qkv for its 384 q/k/v columns,
full causal attention for its 6 heads, and a partial output projection over
its 384 rows of w_proj. Host sums the two half partials per batch + b_proj.

Device pipeline (bf16 matmul operands / f32 PSUM accumulation):
  x arrives PRE-TRANSPOSED from the host (xT [768, 2048] bf16) so no PE
  transposes are needed.  QT/KT pack 2 heads per 128 partitions (q
  pre-scaled by 1/8); VV v-tiles carry a ones column per head at column 0
  so A@V also yields the softmax rowsum on PSUM partition 0, with the 64
  v-dims at partitions 64-127 (legal partition bases for the custom-DVE
  reciprocal and the extract multiply).

  The Tile scheduler builds static in-order per-engine streams from
  emission order (dependency replay has no timing model), so the emission
  WEAVES the work: attention chunk c's score-strip pairs (PE) + exp (ACT)
  + causal mask (gpsimd) + AV accumulation are interleaved with "filler"
  closures carrying qkv chunk c+1 (half-groups of accumulation matmuls)
  and the projection of chunk c-1 (two 1-bank PSUM pieces per s-tile).
  This keeps the PE stream stocked with ready work at chunk boundaries so
  the exp stream never stalls and the PE clock-ramp never resets.

  Per (c,t) normalization: one DVE reciprocal straight off the PSUM rowsum
  rows, one fused gpsimd partition_broadcast, and two DVE multiplies that
  extract+normalize U^T from PSUM in one pass.  Output: per s-tile
  projection into PSUM, DVE drain to bf16, DMA out (host sums in f32).
"""

import numpy as np

B, S, D, H, HD = 4, 2048, 768, 12, 64
HPC = 6  # heads per core
N_CORES = 8

_built_nc = None


def _build():
    import concourse.bass as bass
    import concourse.mybir as mybir
    from concourse import bacc
    import concourse.tile as tile
    from concourse.masks import make_upper_triangular
    from contextlib import ExitStack

    f32 = mybir.dt.float32
    bf16 = mybir.dt.bfloat16
    fp8 = mybir.dt.float8e4
    DR = mybir.MatmulPerfMode.DoubleRow
    FT = mybir.ActivationFunctionType
    MUL = mybir.AluOpType.mult

    nc = bacc.Bacc("TRN2", target_bir_lowering=False, debug=False)
    # x arrives pre-transposed + pre-cast to bf16 from the host
    xT_d = nc.dram_tensor("xT_in", [D, S], bf16, kind="ExternalInput").ap()
    w_d = nc.dram_tensor("w_in", [D, 1152], bf16, kind="ExternalInput").ap()
    bqkv_d = nc.dram_tensor("bqkv_in", [1152], f32, kind="ExternalInput").ap()
    wp_d = nc.dram_tensor("wp_in", [384, D], bf16, kind="ExternalInput").ap()
    out_d = nc.dram_tensor("out", [S, D], bf16, kind="ExternalOutput").ap()

    with tile.TileContext(nc) as tc, ExitStack() as ctx:
        # ---------------- constants + persistent tiles ----------------
        pconst = ctx.enter_context(tc.tile_pool(name="const", bufs=1))
        utri = pconst.tile([128, 128], bf16)  # 1.0 where p <= c else 0.0
        make_upper_triangular(nc, utri[:], val=1.0, diag=True)
        bq = pconst.tile([128, 6], f32)  # per-chunk bias vecs: cols 0-2 q, 3-5 k
        ones64 = pconst.tile([1, 64], f32)
        nc.vector.memset(ones64[:], 1.0)
        bv_row = pconst.tile([1, 384], f32)
        bvb = pconst.tile([128, 384], f32)  # bias_v broadcast to 128 partitions

        pqkv = ctx.enter_context(tc.tile_pool(name="qkvout", bufs=1))
        # q/k in fp8 for DoubleRow score matmuls (0.5 cyc/row, effective
        # K=64 via the 2 pair slots).  Host orders the w_qkv columns so
        # slice ncI=0 holds heads 0-3 dims 0-31 (pair slot 0), ncI=1 holds
        # their dims 32-63 (slot 1), and ncI=2 holds heads 4/5 both halves.
        # Per s-chunk: f4 [128,2,512] = heads 0-3, f2 [64,2,512] = heads 4/5.
        # operand partition bases must be 0/32/64, so: f4 serves heads
        # 0/1/2 at bases 0/32/64 (head 3's data is parked at base 96 and
        # copied out), f2 serves heads 4@0, 5@32, 3@64
        # bf16 q/k for chunk 0 (permuted dim order; slot 3 = parked copies
        # of the base-96 rows): the j=0 strips of chunk 0 run in bf16 so the
        # short-softmax rows 0-127 (which set the output's max magnitude)
        # don't carry fp8 score noise
        QTb = pqkv.tile([128, 5, 512], bf16, name="qtb")
        KTb = pqkv.tile([128, 5, 512], bf16, name="ktb")
        Qf4 = [pqkv.tile([128, 2, 512], fp8, name=f"qf4_{sc}") for sc in range(4)]
        Qf2 = [pqkv.tile([96, 2, 512], fp8, name=f"qf2_{sc}") for sc in range(4)]
        Kf4 = [pqkv.tile([128, 2, 512], fp8, name=f"kf4_{sc}") for sc in range(4)]
        Kf2 = [pqkv.tile([96, 2, 512], fp8, name=f"kf2_{sc}") for sc in range(4)]
        # v tiles: per head 128 columns [ones | 63 unused | 64 v-dims], see
        # module docstring
        VV = [pqkv.tile([128, HPC * 128], bf16, name=f"vv{i}") for i in range(16)]
        UT = [pqkv.tile([128, S], bf16, name=f"ut{t}") for t in range(3)]
        wpt = pqkv.tile([128, 3, D], bf16)
        pes = ctx.enter_context(tc.tile_pool(name="espool", bufs=6))
        pnrm = ctx.enter_context(tc.tile_pool(name="nrm", bufs=3))
        prr = ctx.enter_context(tc.tile_pool(name="rrp", bufs=4))
        pout = ctx.enter_context(tc.tile_pool(name="outp", bufs=4))

        # attention PSUM: strips (4 banks) + AV (2 banks) + qkv/proj mm ring
        pst2 = ctx.enter_context(tc.tile_pool(name="stps", space="PSUM", bufs=2))
        pav = ctx.enter_context(tc.tile_pool(name="avps", space="PSUM", bufs=1))
        pmm = ctx.enter_context(tc.tile_pool(name="mmps", space="PSUM", bufs=2))

        p1 = ctx.enter_context(tc.tile_pool(name="ph1", bufs=1))
        wt = p1.tile([128, 6, 1152], bf16)
        xt = [
            [p1.tile([128, 3, 512], bf16, name=f"xt{sc}_{h}") for h in range(2)]
            for sc in range(4)
        ]

        def xts(sc, c):  # c-th 128-row input-dim slice of chunk sc
            return xt[sc][c // 3][:, c % 3, :]
        # The cost model serializes all transfers through one DMA lane, so
        # issue order ~= arrival order.  Gate-first: xt0 (SP queue) || wq, wk
        # (ACT queue), then everything else in need order.
        for h in range(2):
            nc.sync.dma_start(
                xt[0][h][:],
                xT_d[384 * h : 384 * (h + 1), 0:512].rearrange(
                    "(c p) s -> p c s", p=128
                ),
            )
        # q/k weights: bq first (it gates the QT/KT drains), then the
        # ncI=0 slices (gate the first strips), then the wide remainder
        nc.scalar.dma_start(bq[:], bqkv_d[0:768].rearrange("(c p) -> p c", p=128))
        for lo, hi in ((0, 128), (384, 512), (128, 384), (512, 768)):
            nc.scalar.dma_start(
                wt[:, :, lo:hi],
                w_d[:, lo:hi].rearrange("(c p) n -> p c n", p=128),
            )
        nc.scalar.dma_start(
            bv_row[:], bqkv_d[768:1152].rearrange("(o n) -> o n", o=1)
        )
        nc.gpsimd.partition_broadcast(bvb[:], bv_row[:])
        nc.scalar.dma_start(  # wv
            wt[:, :, 768:1152], w_d[:, 768:1152].rearrange("(c p) n -> p c n", p=128)
        )
        for sc in range(1, 4):
            for h in range(2):
                nc.scalar.dma_start(
                    xt[sc][h][:],
                    xT_d[384 * h : 384 * (h + 1), sc * 512 : (sc + 1) * 512]
                    .rearrange("(c p) s -> p c s", p=128),
                )
        nc.scalar.dma_start(wpt[:], wp_d.rearrange("(c p) n -> p c n", p=128))

        # ---------------- emission building blocks ----------------

        def qkv_fillers(sc):
            """qkv chunk sc as a list of ~0.5-0.7us PE closures (half
            accumulation groups). QT/KT slices first (they gate the next
            chunk's exp stream), V tiles after."""
            out = []
            state = {}

            def qk_half(ncI, which, dst, second):
                def run():
                    base = which * 384
                    if not second:
                        state[(ncI, which)] = pmm.tile([128, 512], f32, tag="mm", name=f"qk{sc}_{ncI}_{which}")
                    ps = state[(ncI, which)]
                    for c in range(3, 6) if second else range(3):
                        nc.tensor.matmul(
                            ps[:],
                            lhsT=wt[:, c, base + ncI * 128 : base + (ncI + 1) * 128],
                            rhs=xts(sc, c),
                            start=(c == 0),
                            stop=(c == 5),
                        )
                    if second:
                        cidx = which * 3 + ncI
                        f4, f2, fb = dst
                        if sc == 0:
                            nc.vector.tensor_scalar_add(
                                fb[:, ncI, :], ps[:], bq[:, cidx : cidx + 1]
                            )
                            # park base-96 rows so every head's two
                            # 32-partition pieces share a base (groups
                            # must keep a constant tile row position):
                            # h3 -> (3,64)+(4,64); h4 -> (2,0)+(3,0);
                            # h5 -> (2,32)+(3,32)
                            if ncI == 0:
                                nc.vector.tensor_copy(
                                    fb[64:96, 3, :], fb[96:128, 0, :]
                                )
                            elif ncI == 1:
                                nc.vector.tensor_copy(
                                    fb[64:96, 4, :], fb[96:128, 1, :]
                                )
                            elif ncI == 2:
                                nc.vector.tensor_copy(
                                    fb[0:64, 3, :], fb[64:128, 2, :]
                                )
                        if ncI < 2:
                            # drain on DVE: keeps the ACT stream pure-exp so
                            # drains never throttle the mm ring behind exps
                            nc.vector.tensor_scalar_add(
                                f4[sc][:, ncI, :],
                                ps[:],
                                bq[:, cidx : cidx + 1],
                            )
                            if ncI == 1:
                                # head 3 parked at f4 base 96 -> f2 base 64
                                nc.vector.tensor_copy(
                                    f2[sc][64:96, :, :], f4[sc][96:128, :, :]
                                )
                        else:
                            # heads 4/5 both halves: two partition-shifted
                            # half drains (DVE handles base shifts)
                            for off, slot in ((0, 0), (64, 1)):
                                nc.vector.tensor_scalar_add(
                                    f2[sc][0:64, slot, :],
                                    ps[off : off + 64, :],
                                    bq[off : off + 64, cidx : cidx + 1],
                                )
                return run

            def v_half(i, second):
                def run():
                    if not second:
                        state[("v", i)] = pmm.tile([128, 384], f32, tag="mm", name=f"psv{i}")
                    psv = state[("v", i)]
                    for c in range(3, 6) if second else range(3):
                        nc.tensor.matmul(
                            psv[:],
                            lhsT=xts(sc, c)[:, (i % 4) * 128 : (i % 4 + 1) * 128],
                            rhs=wt[:, c, 768:1152],
                            start=(c == 0),
                            stop=(c == 5),
                        )
                    if second:
                        vt = VV[i][:].rearrange("p (h m) -> p h m", m=128)
                        nc.vector.tensor_tensor(
                            vt[:, :, 64:128],
                            psv[:].rearrange("p (h m) -> p h m", m=64),
                            bvb[:].rearrange("p (h m) -> p h m", m=64),
                            mybir.AluOpType.add,
                        )
                        nc.vector.memset(vt[:, :, 0:1], 1.0)
                return run

            for ncI in range(3):
                for which, dst in ((0, (Qf4, Qf2, QTb)), (1, (Kf4, Kf2, KTb))):
                    out.append(qk_half(ncI, which, dst, False))
                    out.append(qk_half(ncI, which, dst, True))
            for i in range(sc * 4, sc * 4 + 4):
                out.append(v_half(i, False))
                out.append(v_half(i, True))
            return out

        def proj_fillers(c):
            """Projection of chunk c as 1-bank mm-ring pieces (2 per
            s-tile).  Accumulation leads with t=2 (the last-normalized
            pack) so a piece can't start and then block the PE stream."""
            out = []
            state = {}

            def piece(i, half):
                def run():
                    n0, n1 = (0, 512) if half == 0 else (512, 768)
                    po = pmm.tile([128, n1 - n0], f32, tag="mm", name=f"po{i}_{half}")
                    for t in (2, 0, 1):
                        nc.tensor.matmul(
                            po[:],
                            lhsT=UT[t][:, i * 128 : (i + 1) * 128],
                            rhs=wpt[:, t, n0:n1],
                            start=(t == 2),
                            stop=(t == 1),
                        )
                    if half == 0:
                        state[i] = pout.tile([128, D], bf16, tag="ob", name=f"ob{i}")
                    ob = state[i]
                    nc.scalar.activation(ob[:, n0:n1], po[:], FT.Identity)
                    if half == 1:
                        nc.sync.dma_start(
                            out_d[i * 128 : (i + 1) * 128, :], ob[:]
                        )
                return run

            for i in range(4 * c, 4 * c + 4):
                out.append(piece(i, 0))
                out.append(piece(i, 1))
            return out

        def attention_chunk(c, fillers, target_pairs=None):
            """Emit chunk c's attention, weaving filler closures between
            strip-pairs (never right before a pack boundary)."""
            g0 = c * 512
            npairs = 3 * (4 * c + 4)
            target = target_pairs if target_pairs else npairs - 2
            emitted = [0]

            def weave(allow=True):
                k = emitted[0] = emitted[0] + 1
                if not allow:
                    return
                total = len(fillers)
                want = min(total, (k * total) // target)
                while weave.done < want:
                    fillers[weave.done]()
                    weave.done += 1
            weave.done = 0

            for t in range(3):
                av = pav.tile([128, 2, 512], f32, tag="av")
                pend_av = []
                for j in range(4 * c + 4):
                    n0 = max(0, j * 128 - g0)
                    W = 512 - n0
                    jc, jr = j // 4, (j % 4) * 128
                    st = pst2.tile([128, 1024], f32, tag="st")
                    if c == 0 and j == 0:
                        # bf16 strips for the shortest-softmax rows; each
                        # head's 64 dims live as two 32-partition pieces
                        BFP = (
                            ((0, 0), (1, 0)),    # h0
                            ((0, 32), (1, 32)),  # h1
                            ((0, 64), (1, 64)),  # h2
                            ((3, 64), (4, 64)),  # h3
                            ((2, 0), (3, 0)),    # h4
                            ((2, 32), (3, 32)),  # h5
                        )
                        for hh in (0, 1):
                            for pi, (sl, rb) in enumerate(BFP[2 * t + hh]):
                                nc.tensor.matmul(
                                    st[:, 512 * hh : 512 * hh + 512],
                                    lhsT=KTb[rb : rb + 32, sl, 0:128],
                                    rhs=QTb[rb : rb + 32, sl, :],
                                    start=(pi == 0),
                                    stop=(pi == 1),
                                )
                    else:
                        # head -> (tile, base): t0: f4@0,f4@32; t1: f4@64,
                        # f2@64; t2: f2@0,f2@32
                        hmap = (
                            ((Qf4, Kf4, 0), (Qf4, Kf4, 32)),
                            ((Qf4, Kf4, 64), (Qf2, Kf2, 64)),
                            ((Qf2, Kf2, 0), (Qf2, Kf2, 32)),
                        )[t]
                        for hh, (qtl, ktl, hb) in enumerate(hmap):
                            nc.tensor.matmul(
                                st[:, 512 * hh : 512 * hh + W],
                                lhsT=ktl[jc][hb : hb + 32, :, jr : jr + 128],
                                rhs=qtl[c][hb : hb + 32, :, n0:512],
                                perf_mode=DR,
                                start=True,
                                stop=True,
                            )
                    es = pes.tile([128, 1024], bf16, tag="es")
                    # 1/sqrt(HD) folded into the exp's free affine scale
                    nc.scalar.activation(
                        es[:].rearrange("p (h w) -> p h w", h=2)[:, :, 0:W],
                        st[:].rearrange("p (h w) -> p h w", h=2)[:, :, 0:W],
                        FT.Exp,
                        scale=0.125,
                    )
                    if j * 128 >= g0:  # diagonal block at start of valid region
                        nc.gpsimd.tensor_tensor(
                            es[:, 0:128], es[:, 0:128], utri[:], MUL
                        )
                        nc.gpsimd.tensor_tensor(
                            es[:, 512:640], es[:, 512:640], utri[:], MUL
                        )
                    def av_mm(j, n0, W, es):
                        def run():
                            last = j == 4 * c + 3
                            nc.tensor.matmul(
                                av[:, 0, n0:512],
                                lhsT=VV[j][:, (2 * t) * 128 : (2 * t + 1) * 128],
                                rhs=es[:, 0:W],
                                start=(j == 0),
                                stop=last,
                            )
                            nc.tensor.matmul(
                                av[:, 1, n0:512],
                                lhsT=VV[j][:, (2 * t + 1) * 128 : (2 * t + 2) * 128],
                                rhs=es[:, 512 : 512 + W],
                                start=(j == 0),
                                stop=last,
                            )
                        return run

                    # delay AV by one j so the in-order PE stream never
                    # commits to an av-slot wait before the next strips
                    pend_av.append(av_mm(j, n0, W, es))
                    if len(pend_av) > 1:
                        pend_av.pop(0)()
                    # c=0: fillers carry this chunk's own V tiles, which the
                    # next AV emission needs -- never defer them
                    weave(allow=(c == 0 or j < 4 * c + 2))
                while pend_av:
                    pend_av.pop(0)()
                # normalize + extract U^T: per-head reciprocal straight off
                # PSUM partition 0, replicate across partitions, then one
                # multiply per head from PSUM.  Split per head to halve the
                # chain latency (av-slot release gates the next pack's AV).
                # The last pack replicates via an f32r PE matmul instead of
                # the gpsimd broadcast -- the PE is idle in the tail and the
                # matmul is 7x faster than the Pool broadcast.
                rsr = prr.tile([1, 2, 512], f32, tag="rr")
                for hh in (0, 1):
                    nc.vector.reciprocal_approx_fast(
                        rsr[0:1, hh, :], av[0:1, hh, :]
                    )
                    rec = pnrm.tile([64, 512], f32, tag="rec", name=f"rc{hh}")
                    nc.gpsimd.partition_broadcast(rec[:], rsr[0:1, hh, :])
                    nc.vector.tensor_tensor(
                        UT[t][64 * hh : 64 * hh + 64, g0 : g0 + 512],
                        av[64:128, hh, :],
                        rec[:],
                        MUL,
                    )
            # anything not woven (short chunks): emit now
            while weave.done < len(fillers):
                fillers[weave.done]()
                weave.done += 1

        def proj_tail():
            # final chunk's projection, 4-wide (the two strip slots + the mm
            # ring are idle; the AV slot is NOT used -- allocating it would
            # insert a ring-wait on the last norm into the PE stream).  All
            # t=0/t=1 accumulation matmuls run first: they only need the
            # already-normalized UT[0]/UT[1] and keep the PE busy (and the
            # clock-ramp warm) while the last pack's norm chain drains; the
            # 8 t=2 matmuls + drains follow.
            pos = {}
            for i in (12, 13):
                po = pst2.tile([128, 1024], f32, tag="st", name=f"pot{i}")
                pos[i] = [po[:, 0:512], po[:, 512:768]]
                for t in (0, 1):
                    for half, (n0, n1) in enumerate(((0, 512), (512, 768))):
                        nc.tensor.matmul(
                            pos[i][half][:],
                            lhsT=UT[t][:, i * 128 : (i + 1) * 128],
                            rhs=wpt[:, t, n0:n1],
                            start=(t == 0),
                            stop=False,
                        )
            for i in (14, 15):
                pos[i] = [
                    pmm.tile([128, 512], f32, tag="mm", name=f"pot{i}a"),
                    pmm.tile([128, 256], f32, tag="mm", name=f"pot{i}b"),
                ]
                for t in (0, 1):
                    for half in (0, 1):
                        nc.tensor.matmul(
                            pos[i][half][:],
                            lhsT=UT[t][:, i * 128 : (i + 1) * 128],
                            rhs=wpt[:, t, [0, 512][half] : [512, 768][half]],
                            start=(t == 0),
                            stop=False,
                        )
            for i in range(12, 16):
                ob = pout.tile([128, D], bf16, tag="ob", name=f"obt{i}")
                for half, (n0, n1) in enumerate(((0, 512), (512, 768))):
                    nc.tensor.matmul(
                        pos[i][half][:],
                        lhsT=UT[2][:, i * 128 : (i + 1) * 128],
                        rhs=wpt[:, 2, n0:n1],
                        start=False,
                        stop=True,
                    )
                    nc.scalar.activation(ob[:, n0:n1], pos[i][half][:], FT.Identity)
                nc.sync.dma_start(out_d[i * 128 : (i + 1) * 128, :], ob[:])

        # ---------------- the program ----------------
        f0 = qkv_fillers(0)
        # f0 order: Q0a,Q0b,K0a,K0b, Q1a,Q1b,K1a,K1b, Q2a,Q2b,K2a,K2b, V0..V3
        # ncI 0 AND 1 must complete before the first strip (the strips read
        # both pair slots of the f4 tiles)
        for f in f0[0:8]:
            f()
        # rest of qkv(0): V tiles early (the first AVs need them) woven with
        # the ncI=2 drains (pack t=1/t=2 strips read the f2 tiles)
        rest0 = [
            f0[12], f0[13], f0[8],   # V0 | Q2a
            f0[14], f0[15], f0[9],   # V1 | Q2b
            f0[16], f0[17], f0[10],  # V2 | K2a
            f0[18], f0[19], f0[11],  # V3 | K2b
        ]
        attention_chunk(0, rest0 + qkv_fillers(1), target_pairs=9)
        for c in range(1, 4):
            fillers = qkv_fillers(c + 1) if c < 3 else []
            fillers += proj_fillers(c - 1)
            attention_chunk(c, fillers)
        proj_tail()

    nc.compile()
    return nc


def _get_nc():
    global _built_nc
    if _built_nc is None:
        _built_nc = _build()
    return _built_nc


def _make_in_maps(x, w_qkv, b_qkv, w_proj):
    import ml_dtypes

    bf16 = ml_dtypes.bfloat16
    in_maps = []
    xTb = [np.ascontiguousarray(x[b].T.astype(bf16)) for b in range(B)]
    for core in range(N_CORES):
        b, hh = core // 2, core % 2
        cs = slice(hh * 384, (hh + 1) * 384)
        # head-interleaved column order for the fp8 DoubleRow layout
        # (1/sqrt(64) is applied in the exp's scale, not here)
        perm = np.concatenate(
            [np.arange(h * 64, h * 64 + 32) for h in range(4)]
            + [np.arange(h * 64 + 32, h * 64 + 64) for h in range(4)]
            + [np.arange(h * 64, h * 64 + 32) for h in (4, 5)]
            + [np.arange(h * 64 + 32, h * 64 + 64) for h in (4, 5)]
        )
        wq = w_qkv[:, 0:768][:, cs][:, perm]
        wk = w_qkv[:, 768:1536][:, cs][:, perm]
        wv = w_qkv[:, 1536:2304][:, cs]
        w_in = np.ascontiguousarray(
            np.concatenate([wq, wk, wv], axis=1).astype(bf16)
        )
        bqv = np.concatenate(
            [
                b_qkv[0:768][cs][perm],
                b_qkv[768:1536][cs][perm],
                b_qkv[1536:2304][cs],
            ]
        ).astype(np.float32)
        wp = np.ascontiguousarray(w_proj[cs, :].astype(bf16))
        in_maps.append(
            {
                "xT_in": xTb[b],
                "w_in": w_in,
                "bqkv_in": bqv,
                "wp_in": wp,
            }
        )
    return in_maps


def _run(x, w_qkv, b_qkv, w_proj, b_proj, trace=False):
    from concourse.bass_utils import run_bass_kernel_spmd

    nc = _get_nc()
    in_maps = _make_in_maps(x, w_qkv, b_qkv, w_proj)
    res = run_bass_kernel_spmd(
        nc, in_maps, core_ids=list(range(N_CORES)), trace=trace
    )
    out = np.zeros((B, S, D), np.float32)
    for core in range(N_CORES):
        out[core // 2] += np.asarray(res.results[core]["out"], np.float32)
    out += np.asarray(b_proj, np.float32)[None, None, :]
    return out, res


def kernel(**inputs):
    x = np.asarray(inputs["x"], np.float32)
    w_qkv = np.asarray(inputs["w_qkv"], np.float32)
    b_qkv = np.asarray(inputs["b_qkv"], np.float32)
    w_proj = np.asarray(inputs["w_proj"], np.float32)
    b_proj = np.asarray(inputs["b_proj"], np.float32)
    out, _ = _run(x, w_qkv, b_qkv, w_proj, b_proj, trace=False)
    return out


# revision 42
# speedup vs baseline: 1.0300x; 1.0300x over previous
"""Causal self-attention (B=4, S=2048, D=768, H=12) on 8 trn2 NeuronCores.

Sharding: core c -> (batch b = c//2, head-half hh = c%2). Each core handles
one batch and 6 of the 12 heads: it computes qkv for its 384 q/k/v columns,
full causal attention for its 6 heads, and a partial output projection over
its 384 rows of w_proj. Host sums the two half partials per batch + b_proj.

Device pipeline (bf16 matmul operands / f32 PSUM accumulation):
  x arrives PRE-TRANSPOSED from the host (xT [768, 2048] bf16) so no PE
  transposes are needed.  QT/KT pack 2 heads per 128 partitions (q
  pre-scaled by 1/8); VV v-tiles carry a ones column per head at column 0
  so A@V also yields the softmax rowsum on PSUM partition 0, with the 64
  v-dims at partitions 64-127 (legal partition bases for the custom-DVE
  reciprocal and the extract multiply).

  The Tile scheduler builds static in-order per-engine streams from
  emission order (dependency replay has no timing model), so the emission
  WEAVES the work: attention chunk c's score-strip pairs (PE) + exp (ACT)
  + causal mask (gpsimd) + AV accumulation are interleaved with "filler"
  closures carrying qkv chunk c+1 (half-groups of accumulation matmuls)
  and the projection of chunk c-1 (two 1-bank PSUM pieces per s-tile).
  This keeps the PE stream stocked with ready work at chunk boundaries so
  the exp stream never stalls and the PE clock-ramp never resets.

  Per (c,t) normalization: one DVE reciprocal straight off the PSUM rowsum
  rows, one fused gpsimd partition_broadcast, and two DVE multiplies that
  extract+normalize U^T from PSUM in one pass.  Output: per s-tile
  projection into PSUM, DVE drain to bf16, DMA out (host sums in f32).
"""

import numpy as np

B, S, D, H, HD = 4, 2048, 768, 12, 64
HPC = 6  # heads per core
N_CORES = 8

_built_nc = None


def _build():
    import concourse.bass as bass
    import concourse.mybir as mybir
    from concourse import bacc
    import concourse.tile as tile
    from concourse.masks import make_upper_triangular
    from contextlib import ExitStack

    f32 = mybir.dt.float32
    bf16 = mybir.dt.bfloat16
    fp8 = mybir.dt.float8e4
    DR = mybir.MatmulPerfMode.DoubleRow
    FT = mybir.ActivationFunctionType
    MUL = mybir.AluOpType.mult

    nc = bacc.Bacc("TRN2", target_bir_lowering=False, debug=False)
    # x arrives pre-transposed + pre-cast to bf16 from the host
    xT_d = nc.dram_tensor("xT_in", [D, S], bf16, kind="ExternalInput").ap()
    w_d = nc.dram_tensor("w_in", [D, 1152], bf16, kind="ExternalInput").ap()
    bqkv_d = nc.dram_tensor("bqkv_in", [1152], f32, kind="ExternalInput").ap()
    wp_d = nc.dram_tensor("wp_in", [384, D], bf16, kind="ExternalInput").ap()
    out_d = nc.dram_tensor("out", [S, D], bf16, kind="ExternalOutput").ap()

    with tile.TileContext(nc) as tc, ExitStack() as ctx:
        # ---------------- constants + persistent tiles ----------------
        pconst = ctx.enter_context(tc.tile_pool(name="const", bufs=1))
        utri = pconst.tile([128, 128], bf16)  # 1.0 where p <= c else 0.0
        make_upper_triangular(nc, utri[:], val=1.0, diag=True)
        bq = pconst.tile([128, 6], f32)  # per-chunk bias vecs: cols 0-2 q, 3-5 k
        ones64 = pconst.tile([1, 64], f32)
        nc.vector.memset(ones64[:], 1.0)
        bv_row = pconst.tile([1, 384], f32)
        bvb = pconst.tile([128, 384], f32)  # bias_v broadcast to 128 partitions

        pqkv = ctx.enter_context(tc.tile_pool(name="qkvout", bufs=1))
        # q/k in fp8 for DoubleRow score matmuls (0.5 cyc/row, effective
        # K=64 via the 2 pair slots).  Host orders the w_qkv columns so
        # slice ncI=0 holds heads 0-3 dims 0-31 (pair slot 0), ncI=1 holds
        # their dims 32-63 (slot 1), and ncI=2 holds heads 4/5 both halves.
        # Per s-chunk: f4 [128,2,512] = heads 0-3, f2 [64,2,512] = heads 4/5.
        # operand partition bases must be 0/32/64, so: f4 serves heads
        # 0/1/2 at bases 0/32/64 (head 3's data is parked at base 96 and
        # copied out), f2 serves heads 4@0, 5@32, 3@64
        # bf16 q/k for chunk 0 (permuted dim order; slot 3 = parked copies
        # of the base-96 rows): the j=0 strips of chunk 0 run in bf16 so the
        # short-softmax rows 0-127 (which set the output's max magnitude)
        # don't carry fp8 score noise
        QTb = pqkv.tile([128, 5, 512], bf16, name="qtb")
        KTb = pqkv.tile([128, 5, 512], bf16, name="ktb")
        Qf4 = [pqkv.tile([128, 2, 512], fp8, name=f"qf4_{sc}") for sc in range(4)]
        Qf2 = [pqkv.tile([96, 2, 512], fp8, name=f"qf2_{sc}") for sc in range(4)]
        Kf4 = [pqkv.tile([128, 2, 512], fp8, name=f"kf4_{sc}") for sc in range(4)]
        Kf2 = [pqkv.tile([96, 2, 512], fp8, name=f"kf2_{sc}") for sc in range(4)]
        # v tiles: per head 128 columns [ones | 63 unused | 64 v-dims], see
        # module docstring
        VV = [pqkv.tile([128, HPC * 128], bf16, name=f"vv{i}") for i in range(16)]
        UT = [pqkv.tile([128, S], bf16, name=f"ut{t}") for t in range(3)]
        wpt = pqkv.tile([128, 3, D], bf16)
        pes = ctx.enter_context(tc.tile_pool(name="espool", bufs=6))
        pnrm = ctx.enter_context(tc.tile_pool(name="nrm", bufs=3))
        prr = ctx.enter_context(tc.tile_pool(name="rrp", bufs=4))
        pout = ctx.enter_context(tc.tile_pool(name="outp", bufs=4))

        # attention PSUM: strips (4 banks) + AV (2 banks) + qkv/proj mm ring
        pst2 = ctx.enter_context(tc.tile_pool(name="stps", space="PSUM", bufs=2))
        pav = ctx.enter_context(tc.tile_pool(name="avps", space="PSUM", bufs=1))
        pmm = ctx.enter_context(tc.tile_pool(name="mmps", space="PSUM", bufs=2))

        p1 = ctx.enter_context(tc.tile_pool(name="ph1", bufs=1))
        wt = p1.tile([128, 6, 1152], bf16)
        xt = [
            [p1.tile([128, 3, 512], bf16, name=f"xt{sc}_{h}") for h in range(2)]
            for sc in range(4)
        ]

        def xts(sc, c):  # c-th 128-row input-dim slice of chunk sc
            return xt[sc][c // 3][:, c % 3, :]
        # The cost model serializes all transfers through one DMA lane, so
        # issue order ~= arrival order.  Gate-first: xt0 (SP queue) || wq, wk
        # (ACT queue), then everything else in need order.
        for h in range(2):
            nc.sync.dma_start(
                xt[0][h][:],
                xT_d[384 * h : 384 * (h + 1), 0:512].rearrange(
                    "(c p) s -> p c s", p=128
                ),
            )
        # q/k weights: bq first (it gates the QT/KT drains), then the
        # ncI=0 slices (gate the first strips), then the wide remainder
        nc.scalar.dma_start(bq[:], bqkv_d[0:768].rearrange("(c p) -> p c", p=128))
        for lo, hi in ((0, 128), (384, 512), (128, 384), (512, 768)):
            nc.scalar.dma_start(
                wt[:, :, lo:hi],
                w_d[:, lo:hi].rearrange("(c p) n -> p c n", p=128),
            )
        nc.scalar.dma_start(
            bv_row[:], bqkv_d[768:1152].rearrange("(o n) -> o n", o=1)
        )
        nc.gpsimd.partition_broadcast(bvb[:], bv_row[:])
        nc.scalar.dma_start(  # wv
            wt[:, :, 768:1152], w_d[:, 768:1152].rearrange("(c p) n -> p c n", p=128)
        )
        for sc in range(1, 4):
            for h in range(2):
                nc.scalar.dma_start(
                    xt[sc][h][:],
                    xT_d[384 * h : 384 * (h + 1), sc * 512 : (sc + 1) * 512]
                    .rearrange("(c p) s -> p c s", p=128),
                )
        nc.scalar.dma_start(wpt[:], wp_d.rearrange("(c p) n -> p c n", p=128))

        # ---------------- emission building blocks ----------------

        def qkv_fillers(sc):
            """qkv chunk sc as a list of ~0.5-0.7us PE closures (half
            accumulation groups). QT/KT slices first (they gate the next
            chunk's exp stream), V tiles after."""
            out = []
            state = {}

            def qk_half(ncI, which, dst, second):
                def run():
                    base = which * 384
                    if not second:
                        state[(ncI, which)] = pmm.tile([128, 512], f32, tag="mm", name=f"qk{sc}_{ncI}_{which}")
                    ps = state[(ncI, which)]
                    for c in range(3, 6) if second else range(3):
                        nc.tensor.matmul(
                            ps[:],
                            lhsT=wt[:, c, base + ncI * 128 : base + (ncI + 1) * 128],
                            rhs=xts(sc, c),
                            start=(c == 0),
                            stop=(c == 5),
                        )
                    if second:
                        cidx = which * 3 + ncI
                        f4, f2, fb = dst
                        if sc == 0:
                            nc.vector.tensor_scalar_add(
                                fb[:, ncI, :], ps[:], bq[:, cidx : cidx + 1]
                            )
                            # park base-96 rows so every head's two
                            # 32-partition pieces share a base (groups
                            # must keep a constant tile row position):
                            # h3 -> (3,64)+(4,64); h4 -> (2,0)+(3,0);
                            # h5 -> (2,32)+(3,32)
                            if ncI == 0:
                                nc.vector.tensor_copy(
                                    fb[64:96, 3, :], fb[96:128, 0, :]
                                )
                            elif ncI == 1:
                                nc.vector.tensor_copy(
                                    fb[64:96, 4, :], fb[96:128, 1, :]
                                )
                            elif ncI == 2:
                                nc.vector.tensor_copy(
                                    fb[0:64, 3, :], fb[64:128, 2, :]
                                )
                        if ncI < 2:
                            # drain on DVE: keeps the ACT stream pure-exp so
                            # drains never throttle the mm ring behind exps
                            nc.vector.tensor_scalar_add(
                                f4[sc][:, ncI, :],
                                ps[:],
                                bq[:, cidx : cidx + 1],
                            )
                            if ncI == 1:
                                # head 3 parked at f4 base 96 -> f2 base 64
                                nc.vector.tensor_copy(
                                    f2[sc][64:96, :, :], f4[sc][96:128, :, :]
                                )
                        else:
                            # heads 4/5 both halves: two partition-shifted
                            # half drains (DVE handles base shifts)
                            for off, slot in ((0, 0), (64, 1)):
                                nc.vector.tensor_scalar_add(
                                    f2[sc][0:64, slot, :],
                                    ps[off : off + 64, :],
                                    bq[off : off + 64, cidx : cidx + 1],
                                )
                return run

            def v_half(i, second):
                def run():
                    if not second:
                        state[("v", i)] = pmm.tile([128, 384], f32, tag="mm", name=f"psv{i}")
                    psv = state[("v", i)]
                    for c in range(3, 6) if second else range(3):
                        nc.tensor.matmul(
                            psv[:],
                            lhsT=xts(sc, c)[:, (i % 4) * 128 : (i % 4 + 1) * 128],
                            rhs=wt[:, c, 768:1152],
                            start=(c == 0),
                            stop=(c == 5),
                        )
                    if second:
                        vt = VV[i][:].rearrange("p (h m) -> p h m", m=128)
                        nc.vector.tensor_tensor(
                            vt[:, :, 64:128],
                            psv[:].rearrange("p (h m) -> p h m", m=64),
                            bvb[:].rearrange("p (h m) -> p h m", m=64),
                            mybir.AluOpType.add,
                        )
                        nc.vector.memset(vt[:, :, 0:1], 1.0)
                return run

            for ncI in range(3):
                for which, dst in ((0, (Qf4, Qf2, QTb)), (1, (Kf4, Kf2, KTb))):
                    out.append(qk_half(ncI, which, dst, False))
                    out.append(qk_half(ncI, which, dst, True))
            for i in range(sc * 4, sc * 4 + 4):
                out.append(v_half(i, False))
                out.append(v_half(i, True))
            return out

        def proj_fillers(c):
            """Projection of chunk c as 1-bank mm-ring pieces (2 per
            s-tile).  Accumulation leads with t=2 (the last-normalized
            pack) so a piece can't start and then block the PE stream."""
            out = []
            state = {}

            def piece(i, half):
                def run():
                    n0, n1 = (0, 512) if half == 0 else (512, 768)
                    po = pmm.tile([128, n1 - n0], f32, tag="mm", name=f"po{i}_{half}")
                    for t in (2, 0, 1):
                        nc.tensor.matmul(
                            po[:],
                            lhsT=UT[t][:, i * 128 : (i + 1) * 128],
                            rhs=wpt[:, t, n0:n1],
                            start=(t == 2),
                            stop=(t == 1),
                        )
                    if half == 0:
                        state[i] = pout.tile([128, D], bf16, tag="ob", name=f"ob{i}")
                    ob = state[i]
                    nc.vector.tensor_copy(ob[:, n0:n1], po[:])
                    if half == 1:
                        nc.sync.dma_start(
                            out_d[i * 128 : (i + 1) * 128, :], ob[:]
                        )
                return run

            for i in range(4 * c, 4 * c + 4):
                out.append(piece(i, 0))
                out.append(piece(i, 1))
            return out

        def attention_chunk(c, fillers, target_pairs=None):
            """Emit chunk c's attention, weaving filler closures between
            strip-pairs (never right before a pack boundary)."""
            g0 = c * 512
            npairs = 3 * (4 * c + 4)
            target = target_pairs if target_pairs else npairs - 2
            emitted = [0]

            def weave(allow=True):
                k = emitted[0] = emitted[0] + 1
                if not allow:
                    return
                total = len(fillers)
                want = min(total, (k * total) // target)
                while weave.done < want:
                    fillers[weave.done]()
                    weave.done += 1
            weave.done = 0

            for t in range(3):
                av = pav.tile([128, 2, 512], f32, tag="av")
                pend_av = []
                for j in range(4 * c + 4):
                    n0 = max(0, j * 128 - g0)
                    W = 512 - n0
                    jc, jr = j // 4, (j % 4) * 128
                    st = pst2.tile([128, 1024], f32, tag="st")
                    if c == 0 and j == 0:
                        # bf16 strips for the shortest-softmax rows; each
                        # head's 64 dims live as two 32-partition pieces
                        BFP = (
                            ((0, 0), (1, 0)),    # h0
                            ((0, 32), (1, 32)),  # h1
                            ((0, 64), (1, 64)),  # h2
                            ((3, 64), (4, 64)),  # h3
                            ((2, 0), (3, 0)),    # h4
                            ((2, 32), (3, 32)),  # h5
                        )
                        for hh in (0, 1):
                            for pi, (sl, rb) in enumerate(BFP[2 * t + hh]):
                                nc.tensor.matmul(
                                    st[:, 512 * hh : 512 * hh + 512],
                                    lhsT=KTb[rb : rb + 32, sl, 0:128],
                                    rhs=QTb[rb : rb + 32, sl, :],
                                    start=(pi == 0),
                                    stop=(pi == 1),
                                )
                    else:
                        # head -> (tile, base): t0: f4@0,f4@32; t1: f4@64,
                        # f2@64; t2: f2@0,f2@32
                        hmap = (
                            ((Qf4, Kf4, 0), (Qf4, Kf4, 32)),
                            ((Qf4, Kf4, 64), (Qf2, Kf2, 64)),
                            ((Qf2, Kf2, 0), (Qf2, Kf2, 32)),
                        )[t]
                        for hh, (qtl, ktl, hb) in enumerate(hmap):
                            nc.tensor.matmul(
                                st[:, 512 * hh : 512 * hh + W],
                                lhsT=ktl[jc][hb : hb + 32, :, jr : jr + 128],
                                rhs=qtl[c][hb : hb + 32, :, n0:512],
                                perf_mode=DR,
                                start=True,
                                stop=True,
                            )
                    es = pes.tile([128, 1024], bf16, tag="es")
                    # 1/sqrt(HD) folded into the exp's free affine scale
                    nc.scalar.activation(
                        es[:].rearrange("p (h w) -> p h w", h=2)[:, :, 0:W],
                        st[:].rearrange("p (h w) -> p h w", h=2)[:, :, 0:W],
                        FT.Exp,
                        scale=0.125,
                    )
                    if j * 128 >= g0:  # diagonal block at start of valid region
                        nc.gpsimd.tensor_tensor(
                            es[:, 0:128], es[:, 0:128], utri[:], MUL
                        )
                        nc.gpsimd.tensor_tensor(
                            es[:, 512:640], es[:, 512:640], utri[:], MUL
                        )
                    def av_mm(j, n0, W, es):
                        def run():
                            last = j == 4 * c + 3
                            nc.tensor.matmul(
                                av[:, 0, n0:512],
                                lhsT=VV[j][:, (2 * t) * 128 : (2 * t + 1) * 128],
                                rhs=es[:, 0:W],
                                start=(j == 0),
                                stop=last,
                            )
                            nc.tensor.matmul(
                                av[:, 1, n0:512],
                                lhsT=VV[j][:, (2 * t + 1) * 128 : (2 * t + 2) * 128],
                                rhs=es[:, 512 : 512 + W],
                                start=(j == 0),
                                stop=last,
                            )
                        return run

                    # delay AV by one j so the in-order PE stream never
                    # commits to an av-slot wait before the next strips
                    pend_av.append(av_mm(j, n0, W, es))
                    if len(pend_av) > 1:
                        pend_av.pop(0)()
                    # c=0: fillers carry this chunk's own V tiles, which the
                    # next AV emission needs -- never defer them
                    weave(allow=(c == 0 or j < 4 * c + 2))
                while pend_av:
                    pend_av.pop(0)()
                # normalize + extract U^T: per-head reciprocal straight off
                # PSUM partition 0, replicate across partitions, then one
                # multiply per head from PSUM.  Split per head to halve the
                # chain latency (av-slot release gates the next pack's AV).
                # The last pack replicates via an f32r PE matmul instead of
                # the gpsimd broadcast -- the PE is idle in the tail and the
                # matmul is 7x faster than the Pool broadcast.
                rsr = prr.tile([1, 2, 512], f32, tag="rr")
                for hh in (0, 1):
                    nc.vector.reciprocal_approx_fast(
                        rsr[0:1, hh, :], av[0:1, hh, :]
                    )
                    rec = pnrm.tile([64, 512], f32, tag="rec", name=f"rc{hh}")
                    nc.gpsimd.partition_broadcast(rec[:], rsr[0:1, hh, :])
                    nc.vector.tensor_tensor(
                        UT[t][64 * hh : 64 * hh + 64, g0 : g0 + 512],
                        av[64:128, hh, :],
                        rec[:],
                        MUL,
                    )
            # anything not woven (short chunks): emit now
            while weave.done < len(fillers):
                fillers[weave.done]()
                weave.done += 1

        def proj_tail():
            # final chunk's projection, 4-wide (the two strip slots + the mm
            # ring are idle; the AV slot is NOT used -- allocating it would
            # insert a ring-wait on the last norm into the PE stream).  All
            # t=0/t=1 accumulation matmuls run first: they only need the
            # already-normalized UT[0]/UT[1] and keep the PE busy (and the
            # clock-ramp warm) while the last pack's norm chain drains; the
            # 8 t=2 matmuls + drains follow.
            pos = {}
            for i in (12, 13):
                po = pst2.tile([128, 1024], f32, tag="st", name=f"pot{i}")
                pos[i] = [po[:, 0:512], po[:, 512:768]]
                for t in (0, 1):
                    for half, (n0, n1) in enumerate(((0, 512), (512, 768))):
                        nc.tensor.matmul(
                            pos[i][half][:],
                            lhsT=UT[t][:, i * 128 : (i + 1) * 128],
                            rhs=wpt[:, t, n0:n1],
                            start=(t == 0),
                            stop=False,
                        )
            for i in (14, 15):
                pos[i] = [
                    pmm.tile([128, 512], f32, tag="mm", name=f"pot{i}a"),
                    pmm.tile([128, 256], f32, tag="mm", name=f"pot{i}b"),
                ]
                for t in (0, 1):
                    for half in (0, 1):
                        nc.tensor.matmul(
                            pos[i][half][:],
                            lhsT=UT[t][:, i * 128 : (i + 1) * 128],
                            rhs=wpt[:, t, [0, 512][half] : [512, 768][half]],
                            start=(t == 0),
                            stop=False,
                        )
            for i in range(12, 16):
                ob = pout.tile([128, D], bf16, tag="ob", name=f"obt{i}")
                for half, (n0, n1) in enumerate(((0, 512), (512, 768))):
                    nc.tensor.matmul(
                        pos[i][half][:],
                        lhsT=UT[2][:, i * 128 : (i + 1) * 128],
                        rhs=wpt[:, 2, n0:n1],
                        start=False,
                        stop=True,
                    )
                    nc.vector.tensor_copy(ob[:, n0:n1], pos[i][half][:])
                nc.sync.dma_start(out_d[i * 128 : (i + 1) * 128, :], ob[:])

        # ---------------- the program ----------------
        f0 = qkv_fillers(0)
        # f0 order: Q0a,Q0b,K0a,K0b, Q1a,Q1b,K1a,K1b, Q2a,Q2b,K2a,K2b, V0..V3
        # ncI 0 AND 1 must complete before the first strip (the strips read
        # both pair slots of the f4 tiles)
        for f in f0[0:8]:
            f()
        # rest of qkv(0): V tiles early (the first AVs need them) woven with
        # the ncI=2 drains (pack t=1/t=2 strips read the f2 tiles)
        rest0 = [
            f0[12], f0[13], f0[8],   # V0 | Q2a
            f0[14], f0[15], f0[9],   # V1 | Q2b
            f0[16], f0[17], f0[10],  # V2 | K2a
            f0[18], f0[19], f0[11],  # V3 | K2b
        ]
        attention_chunk(0, rest0 + qkv_fillers(1), target_pairs=9)
        for c in range(1, 4):
            fillers = qkv_fillers(c + 1) if c < 3 else []
            fillers += proj_fillers(c - 1)
            attention_chunk(c, fillers)
        proj_tail()

    nc.compile()
    return nc


def _get_nc():
    global _built_nc
    if _built_nc is None:
        _built_nc = _build()
    return _built_nc


def _make_in_maps(x, w_qkv, b_qkv, w_proj):
    import ml_dtypes

    bf16 = ml_dtypes.bfloat16
    in_maps = []
    xTb = [np.ascontiguousarray(x[b].T.astype(bf16)) for b in range(B)]
    for core in range(N_CORES):
        b, hh = core // 2, core % 2
        cs = slice(hh * 384, (hh + 1) * 384)
        # head-interleaved column order for the fp8 DoubleRow layout
        # (1/sqrt(64) is applied in the exp's scale, not here)
        perm = np.concatenate(
            [np.arange(h * 64, h * 64 + 32) for h in range(4)]
            + [np.arange(h * 64 + 32, h * 64 + 64) for h in range(4)]
            + [np.arange(h * 64, h * 64 + 32) for h in (4, 5)]
            + [np.arange(h * 64 + 32, h * 64 + 64) for h in (4, 5)]
        )
        wq = w_qkv[:, 0:768][:, cs][:, perm]
        wk = w_qkv[:, 768:1536][:, cs][:, perm]
        wv = w_qkv[:, 1536:2304][:, cs]
        w_in = np.ascontiguousarray(
            np.concatenate([wq, wk, wv], axis=1).astype(bf16)
        )
        bqv = np.concatenate(
            [
                b_qkv[0:768][cs][perm],
                b_qkv[768:1536][cs][perm],
                b_qkv[1536:2304][cs],
            ]
        ).astype(np.float32)
        wp = np.ascontiguousarray(w_proj[cs, :].astype(bf16))
        in_maps.append(
            {
                "xT_in": xTb[b],
                "w_in": w_in,
                "bqkv_in": bqv,
                "wp_in": wp,
            }
        )
    return in_maps


def _run(x, w_qkv, b_qkv, w_proj, b_proj, trace=False):
    from concourse.bass_utils import run_bass_kernel_spmd

    nc = _get_nc()
    in_maps = _make_in_maps(x, w_qkv, b_qkv, w_proj)
    res = run_bass_kernel_spmd(
        nc, in_maps, core_ids=list(range(N_CORES)), trace=trace
    )
    out = np.zeros((B, S, D), np.float32)
    for core in range(N_CORES):
        out[core // 2] += np.asarray(res.results[core]["out"], np.float32)
    out += np.asarray(b_proj, np.float32)[None, None, :]
    return out, res


def kernel(**inputs):
    x = np.asarray(inputs["x"], np.float32)
    w_qkv = np.asarray(inputs["w_qkv"], np.float32)
    b_qkv = np.asarray(inputs["b_qkv"], np.float32)
    w_proj = np.asarray(inputs["w_proj"], np.float32)
    b_proj = np.asarray(inputs["b_proj"], np.float32)
    out, _ = _run(x, w_qkv, b_qkv, w_proj, b_proj, trace=False)
    return out


# revision 43
# speedup vs baseline: 1.0316x; 1.0015x over previous
"""Causal self-attention (B=4, S=2048, D=768, H=12) on 8 trn2 NeuronCores.

Sharding: core c -> (batch b = c//2, head-half hh = c%2). Each core handles
one batch and 6 of the 12 heads: it computes qkv for its 384 q/k/v columns,
full causal attention for its 6 heads, and a partial output projection over
its 384 rows of w_proj. Host sums the two half partials per batch + b_proj.

Device pipeline (bf16 matmul operands / f32 PSUM accumulation):
  x arrives PRE-TRANSPOSED from the host (xT [768, 2048] bf16) so no PE
  transposes are needed.  QT/KT pack 2 heads per 128 partitions (q
  pre-scaled by 1/8); VV v-tiles carry a ones column per head at column 0
  so A@V also yields the softmax rowsum on PSUM partition 0, with the 64
  v-dims at partitions 64-127 (legal partition bases for the custom-DVE
  reciprocal and the extract multiply).

  The Tile scheduler builds static in-order per-engine streams from
  emission order (dependency replay has no timing model), so the emission
  WEAVES the work: attention chunk c's score-strip pairs (PE) + exp (ACT)
  + causal mask (gpsimd) + AV accumulation are interleaved with "filler"
  closures carrying qkv chunk c+1 (half-groups of accumulation matmuls)
  and the projection of chunk c-1 (two 1-bank PSUM pieces per s-tile).
  This keeps the PE stream stocked with ready work at chunk boundaries so
  the exp stream never stalls and the PE clock-ramp never resets.

  Per (c,t) normalization: one DVE reciprocal straight off the PSUM rowsum
  rows, one fused gpsimd partition_broadcast, and two DVE multiplies that
  extract+normalize U^T from PSUM in one pass.  Output: per s-tile
  projection into PSUM, DVE drain to bf16, DMA out (host sums in f32).
"""

import numpy as np

B, S, D, H, HD = 4, 2048, 768, 12, 64
HPC = 6  # heads per core
N_CORES = 8

_built_nc = None


def _build():
    import concourse.bass as bass
    import concourse.mybir as mybir
    from concourse import bacc
    import concourse.tile as tile
    from concourse.masks import make_upper_triangular
    from contextlib import ExitStack

    f32 = mybir.dt.float32
    bf16 = mybir.dt.bfloat16
    fp8 = mybir.dt.float8e4
    DR = mybir.MatmulPerfMode.DoubleRow
    FT = mybir.ActivationFunctionType
    MUL = mybir.AluOpType.mult

    nc = bacc.Bacc("TRN2", target_bir_lowering=False, debug=False)
    # x arrives pre-transposed + pre-cast to bf16 from the host
    xT_d = nc.dram_tensor("xT_in", [D, S], bf16, kind="ExternalInput").ap()
    w_d = nc.dram_tensor("w_in", [D, 1152], bf16, kind="ExternalInput").ap()
    bqkv_d = nc.dram_tensor("bqkv_in", [1152], f32, kind="ExternalInput").ap()
    wp_d = nc.dram_tensor("wp_in", [384, D], bf16, kind="ExternalInput").ap()
    out_d = nc.dram_tensor("out", [S, D], bf16, kind="ExternalOutput").ap()

    with tile.TileContext(nc) as tc, ExitStack() as ctx:
        # ---------------- constants + persistent tiles ----------------
        pconst = ctx.enter_context(tc.tile_pool(name="const", bufs=1))
        utri = pconst.tile([128, 128], bf16)  # 1.0 where p <= c else 0.0
        make_upper_triangular(nc, utri[:], val=1.0, diag=True)
        bq = pconst.tile([128, 6], f32)  # per-chunk bias vecs: cols 0-2 q, 3-5 k
        ones64 = pconst.tile([1, 64], f32)
        nc.vector.memset(ones64[:], 1.0)
        bv_row = pconst.tile([1, 384], f32)
        bvb = pconst.tile([128, 384], f32)  # bias_v broadcast to 128 partitions

        pqkv = ctx.enter_context(tc.tile_pool(name="qkvout", bufs=1))
        # q/k in fp8 for DoubleRow score matmuls (0.5 cyc/row, effective
        # K=64 via the 2 pair slots).  Host orders the w_qkv columns so
        # slice ncI=0 holds heads 0-3 dims 0-31 (pair slot 0), ncI=1 holds
        # their dims 32-63 (slot 1), and ncI=2 holds heads 4/5 both halves.
        # Per s-chunk: f4 [128,2,512] = heads 0-3, f2 [64,2,512] = heads 4/5.
        # operand partition bases must be 0/32/64, so: f4 serves heads
        # 0/1/2 at bases 0/32/64 (head 3's data is parked at base 96 and
        # copied out), f2 serves heads 4@0, 5@32, 3@64
        # bf16 q/k for chunk 0 (permuted dim order; slot 3 = parked copies
        # of the base-96 rows): the j=0 strips of chunk 0 run in bf16 so the
        # short-softmax rows 0-127 (which set the output's max magnitude)
        # don't carry fp8 score noise
        QTb = pqkv.tile([128, 5, 512], bf16, name="qtb")
        KTb = pqkv.tile([128, 5, 512], bf16, name="ktb")
        Qf4 = [pqkv.tile([128, 2, 512], fp8, name=f"qf4_{sc}") for sc in range(4)]
        Qf2 = [pqkv.tile([96, 2, 512], fp8, name=f"qf2_{sc}") for sc in range(4)]
        Kf4 = [pqkv.tile([128, 2, 512], fp8, name=f"kf4_{sc}") for sc in range(4)]
        Kf2 = [pqkv.tile([96, 2, 512], fp8, name=f"kf2_{sc}") for sc in range(4)]
        # v tiles: per head 128 columns [ones | 63 unused | 64 v-dims], see
        # module docstring
        VV = [pqkv.tile([128, HPC * 128], bf16, name=f"vv{i}") for i in range(16)]
        UT = [pqkv.tile([128, S], bf16, name=f"ut{t}") for t in range(3)]
        wpt = pqkv.tile([128, 3, D], bf16)
        pes = ctx.enter_context(tc.tile_pool(name="espool", bufs=8))
        pnrm = ctx.enter_context(tc.tile_pool(name="nrm", bufs=6))
        prr = ctx.enter_context(tc.tile_pool(name="rrp", bufs=4))
        pout = ctx.enter_context(tc.tile_pool(name="outp", bufs=6))

        # attention PSUM: strips (4 banks) + AV (2 banks) + qkv/proj mm ring
        pst2 = ctx.enter_context(tc.tile_pool(name="stps", space="PSUM", bufs=2))
        pav = ctx.enter_context(tc.tile_pool(name="avps", space="PSUM", bufs=1))
        pmm = ctx.enter_context(tc.tile_pool(name="mmps", space="PSUM", bufs=2))

        p1 = ctx.enter_context(tc.tile_pool(name="ph1", bufs=1))
        wt = p1.tile([128, 6, 1152], bf16)
        xt = [
            [p1.tile([128, 3, 512], bf16, name=f"xt{sc}_{h}") for h in range(2)]
            for sc in range(4)
        ]

        def xts(sc, c):  # c-th 128-row input-dim slice of chunk sc
            return xt[sc][c // 3][:, c % 3, :]
        # The cost model serializes all transfers through one DMA lane, so
        # issue order ~= arrival order.  Gate-first: xt0 (SP queue) || wq, wk
        # (ACT queue), then everything else in need order.
        for h in range(2):
            nc.sync.dma_start(
                xt[0][h][:],
                xT_d[384 * h : 384 * (h + 1), 0:512].rearrange(
                    "(c p) s -> p c s", p=128
                ),
            )
        # q/k weights: bq first (it gates the QT/KT drains), then the
        # ncI=0 slices (gate the first strips), then the wide remainder
        nc.scalar.dma_start(bq[:], bqkv_d[0:768].rearrange("(c p) -> p c", p=128))
        for lo, hi in ((0, 128), (384, 512), (128, 384), (512, 768)):
            nc.scalar.dma_start(
                wt[:, :, lo:hi],
                w_d[:, lo:hi].rearrange("(c p) n -> p c n", p=128),
            )
        nc.scalar.dma_start(
            bv_row[:], bqkv_d[768:1152].rearrange("(o n) -> o n", o=1)
        )
        nc.gpsimd.partition_broadcast(bvb[:], bv_row[:])
        nc.scalar.dma_start(  # wv
            wt[:, :, 768:1152], w_d[:, 768:1152].rearrange("(c p) n -> p c n", p=128)
        )
        for sc in range(1, 4):
            for h in range(2):
                nc.scalar.dma_start(
                    xt[sc][h][:],
                    xT_d[384 * h : 384 * (h + 1), sc * 512 : (sc + 1) * 512]
                    .rearrange("(c p) s -> p c s", p=128),
                )
        nc.scalar.dma_start(wpt[:], wp_d.rearrange("(c p) n -> p c n", p=128))

        # ---------------- emission building blocks ----------------

        def qkv_fillers(sc):
            """qkv chunk sc as a list of ~0.5-0.7us PE closures (half
            accumulation groups). QT/KT slices first (they gate the next
            chunk's exp stream), V tiles after."""
            out = []
            state = {}

            def qk_half(ncI, which, dst, second):
                def run():
                    base = which * 384
                    if not second:
                        state[(ncI, which)] = pmm.tile([128, 512], f32, tag="mm", name=f"qk{sc}_{ncI}_{which}")
                    ps = state[(ncI, which)]
                    for c in range(3, 6) if second else range(3):
                        nc.tensor.matmul(
                            ps[:],
                            lhsT=wt[:, c, base + ncI * 128 : base + (ncI + 1) * 128],
                            rhs=xts(sc, c),
                            start=(c == 0),
                            stop=(c == 5),
                        )
                    if second:
                        cidx = which * 3 + ncI
                        f4, f2, fb = dst
                        if sc == 0:
                            nc.vector.tensor_scalar_add(
                                fb[:, ncI, :], ps[:], bq[:, cidx : cidx + 1]
                            )
                            # park base-96 rows so every head's two
                            # 32-partition pieces share a base (groups
                            # must keep a constant tile row position):
                            # h3 -> (3,64)+(4,64); h4 -> (2,0)+(3,0);
                            # h5 -> (2,32)+(3,32)
                            if ncI == 0:
                                nc.vector.tensor_copy(
                                    fb[64:96, 3, :], fb[96:128, 0, :]
                                )
                            elif ncI == 1:
                                nc.vector.tensor_copy(
                                    fb[64:96, 4, :], fb[96:128, 1, :]
                                )
                            elif ncI == 2:
                                nc.vector.tensor_copy(
                                    fb[0:64, 3, :], fb[64:128, 2, :]
                                )
                        if ncI < 2:
                            # drain on DVE: keeps the ACT stream pure-exp so
                            # drains never throttle the mm ring behind exps
                            nc.vector.tensor_scalar_add(
                                f4[sc][:, ncI, :],
                                ps[:],
                                bq[:, cidx : cidx + 1],
                            )
                            if ncI == 1:
                                # head 3 parked at f4 base 96 -> f2 base 64
                                nc.vector.tensor_copy(
                                    f2[sc][64:96, :, :], f4[sc][96:128, :, :]
                                )
                        else:
                            # heads 4/5 both halves: two partition-shifted
                            # half drains (DVE handles base shifts)
                            for off, slot in ((0, 0), (64, 1)):
                                nc.vector.tensor_scalar_add(
                                    f2[sc][0:64, slot, :],
                                    ps[off : off + 64, :],
                                    bq[off : off + 64, cidx : cidx + 1],
                                )
                return run

            def v_half(i, second):
                def run():
                    if not second:
                        state[("v", i)] = pmm.tile([128, 384], f32, tag="mm", name=f"psv{i}")
                    psv = state[("v", i)]
                    for c in range(3, 6) if second else range(3):
                        nc.tensor.matmul(
                            psv[:],
                            lhsT=xts(sc, c)[:, (i % 4) * 128 : (i % 4 + 1) * 128],
                            rhs=wt[:, c, 768:1152],
                            start=(c == 0),
                            stop=(c == 5),
                        )
                    if second:
                        vt = VV[i][:].rearrange("p (h m) -> p h m", m=128)
                        nc.vector.tensor_tensor(
                            vt[:, :, 64:128],
                            psv[:].rearrange("p (h m) -> p h m", m=64),
                            bvb[:].rearrange("p (h m) -> p h m", m=64),
                            mybir.AluOpType.add,
                        )
                        nc.vector.memset(vt[:, :, 0:1], 1.0)
                return run

            for ncI in range(3):
                for which, dst in ((0, (Qf4, Qf2, QTb)), (1, (Kf4, Kf2, KTb))):
                    out.append(qk_half(ncI, which, dst, False))
                    out.append(qk_half(ncI, which, dst, True))
            for i in range(sc * 4, sc * 4 + 4):
                out.append(v_half(i, False))
                out.append(v_half(i, True))
            return out

        def proj_fillers(c):
            """Projection of chunk c as 1-bank mm-ring pieces (2 per
            s-tile).  Accumulation leads with t=2 (the last-normalized
            pack) so a piece can't start and then block the PE stream."""
            out = []
            state = {}

            def piece(i, half):
                def run():
                    n0, n1 = (0, 512) if half == 0 else (512, 768)
                    po = pmm.tile([128, n1 - n0], f32, tag="mm", name=f"po{i}_{half}")
                    for t in (2, 0, 1):
                        nc.tensor.matmul(
                            po[:],
                            lhsT=UT[t][:, i * 128 : (i + 1) * 128],
                            rhs=wpt[:, t, n0:n1],
                            start=(t == 2),
                            stop=(t == 1),
                        )
                    if half == 0:
                        state[i] = pout.tile([128, D], bf16, tag="ob", name=f"ob{i}")
                    ob = state[i]
                    nc.vector.tensor_copy(ob[:, n0:n1], po[:])
                    if half == 1:
                        nc.sync.dma_start(
                            out_d[i * 128 : (i + 1) * 128, :], ob[:]
                        )
                return run

            for i in range(4 * c, 4 * c + 4):
                out.append(piece(i, 0))
                out.append(piece(i, 1))
            return out

        def attention_chunk(c, fillers, target_pairs=None):
            """Emit chunk c's attention, weaving filler closures between
            strip-pairs (never right before a pack boundary)."""
            g0 = c * 512
            npairs = 3 * (4 * c + 4)
            target = target_pairs if target_pairs else npairs - 2
            emitted = [0]

            def weave(allow=True):
                k = emitted[0] = emitted[0] + 1
                if not allow:
                    return
                total = len(fillers)
                want = min(total, (k * total) // target)
                while weave.done < want:
                    fillers[weave.done]()
                    weave.done += 1
            weave.done = 0

            for t in range(3):
                av = pav.tile([128, 2, 512], f32, tag="av")
                pend_av = []
                for j in range(4 * c + 4):
                    n0 = max(0, j * 128 - g0)
                    W = 512 - n0
                    jc, jr = j // 4, (j % 4) * 128
                    st = pst2.tile([128, 1024], f32, tag="st")
                    if c == 0 and j == 0:
                        # bf16 strips for the shortest-softmax rows; each
                        # head's 64 dims live as two 32-partition pieces
                        BFP = (
                            ((0, 0), (1, 0)),    # h0
                            ((0, 32), (1, 32)),  # h1
                            ((0, 64), (1, 64)),  # h2
                            ((3, 64), (4, 64)),  # h3
                            ((2, 0), (3, 0)),    # h4
                            ((2, 32), (3, 32)),  # h5
                        )
                        for hh in (0, 1):
                            for pi, (sl, rb) in enumerate(BFP[2 * t + hh]):
                                nc.tensor.matmul(
                                    st[:, 512 * hh : 512 * hh + 512],
                                    lhsT=KTb[rb : rb + 32, sl, 0:128],
                                    rhs=QTb[rb : rb + 32, sl, :],
                                    start=(pi == 0),
                                    stop=(pi == 1),
                                )
                    else:
                        # head -> (tile, base): t0: f4@0,f4@32; t1: f4@64,
                        # f2@64; t2: f2@0,f2@32
                        hmap = (
                            ((Qf4, Kf4, 0), (Qf4, Kf4, 32)),
                            ((Qf4, Kf4, 64), (Qf2, Kf2, 64)),
                            ((Qf2, Kf2, 0), (Qf2, Kf2, 32)),
                        )[t]
                        for hh, (qtl, ktl, hb) in enumerate(hmap):
                            nc.tensor.matmul(
                                st[:, 512 * hh : 512 * hh + W],
                                lhsT=ktl[jc][hb : hb + 32, :, jr : jr + 128],
                                rhs=qtl[c][hb : hb + 32, :, n0:512],
                                perf_mode=DR,
                                start=True,
                                stop=True,
                            )
                    es = pes.tile([128, 1024], bf16, tag="es")
                    # 1/sqrt(HD) folded into the exp's free affine scale
                    nc.scalar.activation(
                        es[:].rearrange("p (h w) -> p h w", h=2)[:, :, 0:W],
                        st[:].rearrange("p (h w) -> p h w", h=2)[:, :, 0:W],
                        FT.Exp,
                        scale=0.125,
                    )
                    if j * 128 >= g0:  # diagonal block at start of valid region
                        nc.gpsimd.tensor_tensor(
                            es[:, 0:128], es[:, 0:128], utri[:], MUL
                        )
                        nc.gpsimd.tensor_tensor(
                            es[:, 512:640], es[:, 512:640], utri[:], MUL
                        )
                    def av_mm(j, n0, W, es):
                        def run():
                            last = j == 4 * c + 3
                            nc.tensor.matmul(
                                av[:, 0, n0:512],
                                lhsT=VV[j][:, (2 * t) * 128 : (2 * t + 1) * 128],
                                rhs=es[:, 0:W],
                                start=(j == 0),
                                stop=last,
                            )
                            nc.tensor.matmul(
                                av[:, 1, n0:512],
                                lhsT=VV[j][:, (2 * t + 1) * 128 : (2 * t + 2) * 128],
                                rhs=es[:, 512 : 512 + W],
                                start=(j == 0),
                                stop=last,
                            )
                        return run

                    # delay AV by one j so the in-order PE stream never
                    # commits to an av-slot wait before the next strips
                    pend_av.append(av_mm(j, n0, W, es))
                    if len(pend_av) > 1:
                        pend_av.pop(0)()
                    # c=0: fillers carry this chunk's own V tiles, which the
                    # next AV emission needs -- never defer them
                    weave(allow=(c == 0 or j < 4 * c + 2))
                while pend_av:
                    pend_av.pop(0)()
                # normalize + extract U^T: per-head reciprocal straight off
                # PSUM partition 0, replicate across partitions, then one
                # multiply per head from PSUM.  Split per head to halve the
                # chain latency (av-slot release gates the next pack's AV).
                # The last pack replicates via an f32r PE matmul instead of
                # the gpsimd broadcast -- the PE is idle in the tail and the
                # matmul is 7x faster than the Pool broadcast.
                rsr = prr.tile([1, 2, 512], f32, tag="rr")
                for hh in (0, 1):
                    nc.vector.reciprocal_approx_fast(
                        rsr[0:1, hh, :], av[0:1, hh, :]
                    )
                    rec = pnrm.tile([64, 512], f32, tag="rec", name=f"rc{hh}")
                    nc.gpsimd.partition_broadcast(rec[:], rsr[0:1, hh, :])
                    nc.vector.tensor_tensor(
                        UT[t][64 * hh : 64 * hh + 64, g0 : g0 + 512],
                        av[64:128, hh, :],
                        rec[:],
                        MUL,
                    )
            # anything not woven (short chunks): emit now
            while weave.done < len(fillers):
                fillers[weave.done]()
                weave.done += 1

        def proj_tail():
            # final chunk's projection, 4-wide (the two strip slots + the mm
            # ring are idle; the AV slot is NOT used -- allocating it would
            # insert a ring-wait on the last norm into the PE stream).  All
            # t=0/t=1 accumulation matmuls run first: they only need the
            # already-normalized UT[0]/UT[1] and keep the PE busy (and the
            # clock-ramp warm) while the last pack's norm chain drains; the
            # 8 t=2 matmuls + drains follow.
            pos = {}
            for i in (12, 13):
                po = pst2.tile([128, 1024], f32, tag="st", name=f"pot{i}")
                pos[i] = [po[:, 0:512], po[:, 512:768]]
                for t in (0, 1):
                    for half, (n0, n1) in enumerate(((0, 512), (512, 768))):
                        nc.tensor.matmul(
                            pos[i][half][:],
                            lhsT=UT[t][:, i * 128 : (i + 1) * 128],
                            rhs=wpt[:, t, n0:n1],
                            start=(t == 0),
                            stop=False,
                        )
            for i in (14, 15):
                pos[i] = [
                    pmm.tile([128, 512], f32, tag="mm", name=f"pot{i}a"),
                    pmm.tile([128, 256], f32, tag="mm", name=f"pot{i}b"),
                ]
                for t in (0, 1):
                    for half in (0, 1):
                        nc.tensor.matmul(
                            pos[i][half][:],
                            lhsT=UT[t][:, i * 128 : (i + 1) * 128],
                            rhs=wpt[:, t, [0, 512][half] : [512, 768][half]],
                            start=(t == 0),
                            stop=False,
                        )
            for i in range(12, 16):
                ob = pout.tile([128, D], bf16, tag="ob", name=f"obt{i}")
                for half, (n0, n1) in enumerate(((0, 512), (512, 768))):
                    nc.tensor.matmul(
                        pos[i][half][:],
                        lhsT=UT[2][:, i * 128 : (i + 1) * 128],
                        rhs=wpt[:, 2, n0:n1],
                        start=False,
                        stop=True,
                    )
                    nc.vector.tensor_copy(ob[:, n0:n1], pos[i][half][:])
                nc.sync.dma_start(out_d[i * 128 : (i + 1) * 128, :], ob[:])

        # ---------------- the program ----------------
        f0 = qkv_fillers(0)
        # f0 order: Q0a,Q0b,K0a,K0b, Q1a,Q1b,K1a,K1b, Q2a,Q2b,K2a,K2b, V0..V3
        # ncI 0 AND 1 must complete before the first strip (the strips read
        # both pair slots of the f4 tiles)
        for f in f0[0:8]:
            f()
        # rest of qkv(0): V tiles early (the first AVs need them) woven with
        # the ncI=2 drains (pack t=1/t=2 strips read the f2 tiles)
        rest0 = [
            f0[12], f0[13], f0[8],   # V0 | Q2a
            f0[14], f0[15], f0[9],   # V1 | Q2b
            f0[16], f0[17], f0[10],  # V2 | K2a
            f0[18], f0[19], f0[11],  # V3 | K2b
        ]
        attention_chunk(0, rest0 + qkv_fillers(1), target_pairs=9)
        for c in range(1, 4):
            fillers = qkv_fillers(c + 1) if c < 3 else []
            fillers += proj_fillers(c - 1)
            attention_chunk(c, fillers)
        proj_tail()

    nc.compile()
    return nc


def _get_nc():
    global _built_nc
    if _built_nc is None:
        _built_nc = _build()
    return _built_nc


def _make_in_maps(x, w_qkv, b_qkv, w_proj):
    import ml_dtypes

    bf16 = ml_dtypes.bfloat16
    in_maps = []
    xTb = [np.ascontiguousarray(x[b].T.astype(bf16)) for b in range(B)]
    for core in range(N_CORES):
        b, hh = core // 2, core % 2
        cs = slice(hh * 384, (hh + 1) * 384)
        # head-interleaved column order for the fp8 DoubleRow layout
        # (1/sqrt(64) is applied in the exp's scale, not here)
        perm = np.concatenate(
            [np.arange(h * 64, h * 64 + 32) for h in range(4)]
            + [np.arange(h * 64 + 32, h * 64 + 64) for h in range(4)]
            + [np.arange(h * 64, h * 64 + 32) for h in (4, 5)]
            + [np.arange(h * 64 + 32, h * 64 + 64) for h in (4, 5)]
        )
        wq = w_qkv[:, 0:768][:, cs][:, perm]
        wk = w_qkv[:, 768:1536][:, cs][:, perm]
        wv = w_qkv[:, 1536:2304][:, cs]
        w_in = np.ascontiguousarray(
            np.concatenate([wq, wk, wv], axis=1).astype(bf16)
        )
        bqv = np.concatenate(
            [
                b_qkv[0:768][cs][perm],
                b_qkv[768:1536][cs][perm],
                b_qkv[1536:2304][cs],
            ]
        ).astype(np.float32)
        wp = np.ascontiguousarray(w_proj[cs, :].astype(bf16))
        in_maps.append(
            {
                "xT_in": xTb[b],
                "w_in": w_in,
                "bqkv_in": bqv,
                "wp_in": wp,
            }
        )
    return in_maps


def _run(x, w_qkv, b_qkv, w_proj, b_proj, trace=False):
    from concourse.bass_utils import run_bass_kernel_spmd

    nc = _get_nc()
    in_maps = _make_in_maps(x, w_qkv, b_qkv, w_proj)
    res = run_bass_kernel_spmd(
        nc, in_maps, core_ids=list(range(N_CORES)), trace=trace
    )
    out = np.zeros((B, S, D), np.float32)
    for core in range(N_CORES):
        out[core // 2] += np.asarray(res.results[core]["out"], np.float32)
    out += np.asarray(b_proj, np.float32)[None, None, :]
    return out, res


def kernel(**inputs):
    x = np.asarray(inputs["x"], np.float32)
    w_qkv = np.asarray(inputs["w_qkv"], np.float32)
    b_qkv = np.asarray(inputs["b_qkv"], np.float32)
    w_proj = np.asarray(inputs["w_proj"], np.float32)
    b_proj = np.asarray(inputs["b_proj"], np.float32)
    out, _ = _run(x, w_qkv, b_qkv, w_proj, b_proj, trace=False)
    return out


# revision 44
# speedup vs baseline: 1.0453x; 1.0133x over previous
"""Causal self-attention (B=4, S=2048, D=768, H=12) on 8 trn2 NeuronCores.

Sharding: core c -> (batch b = c//2, head-half hh = c%2). Each core handles
one batch and 6 of the 12 heads: it computes qkv for its 384 q/k/v columns,
full causal attention for its 6 heads, and a partial output projection over
its 384 rows of w_proj. Host sums the two half partials per batch + b_proj.

Device pipeline (bf16 matmul operands / f32 PSUM accumulation):
  x arrives PRE-TRANSPOSED from the host (xT [768, 2048] bf16) so no PE
  transposes are needed.  QT/KT pack 2 heads per 128 partitions (q
  pre-scaled by 1/8); VV v-tiles carry a ones column per head at column 0
  so A@V also yields the softmax rowsum on PSUM partition 0, with the 64
  v-dims at partitions 64-127 (legal partition bases for the custom-DVE
  reciprocal and the extract multiply).

  The Tile scheduler builds static in-order per-engine streams from
  emission order (dependency replay has no timing model), so the emission
  WEAVES the work: attention chunk c's score-strip pairs (PE) + exp (ACT)
  + causal mask (gpsimd) + AV accumulation are interleaved with "filler"
  closures carrying qkv chunk c+1 (half-groups of accumulation matmuls)
  and the projection of chunk c-1 (two 1-bank PSUM pieces per s-tile).
  This keeps the PE stream stocked with ready work at chunk boundaries so
  the exp stream never stalls and the PE clock-ramp never resets.

  Per (c,t) normalization: one DVE reciprocal straight off the PSUM rowsum
  rows, one fused gpsimd partition_broadcast, and two DVE multiplies that
  extract+normalize U^T from PSUM in one pass.  Output: per s-tile
  projection into PSUM, DVE drain to bf16, DMA out (host sums in f32).
"""

import numpy as np

B, S, D, H, HD = 4, 2048, 768, 12, 64
HPC = 6  # heads per core
N_CORES = 8

_built_nc = None


def _build():
    import concourse.bass as bass
    import concourse.mybir as mybir
    from concourse import bacc
    import concourse.tile as tile
    from concourse.masks import make_upper_triangular
    from contextlib import ExitStack

    f32 = mybir.dt.float32
    bf16 = mybir.dt.bfloat16
    fp8 = mybir.dt.float8e4
    DR = mybir.MatmulPerfMode.DoubleRow
    FT = mybir.ActivationFunctionType
    MUL = mybir.AluOpType.mult

    nc = bacc.Bacc("TRN2", target_bir_lowering=False, debug=False)
    # x arrives pre-transposed + pre-cast to bf16 from the host
    xT_d = nc.dram_tensor("xT_in", [D, S], bf16, kind="ExternalInput").ap()
    w_d = nc.dram_tensor("w_in", [D, 1152], bf16, kind="ExternalInput").ap()
    bqkv_d = nc.dram_tensor("bqkv_in", [1152], f32, kind="ExternalInput").ap()
    wp_d = nc.dram_tensor("wp_in", [384, D], bf16, kind="ExternalInput").ap()
    out_d = nc.dram_tensor("out", [S, D], bf16, kind="ExternalOutput").ap()

    with tile.TileContext(nc) as tc, ExitStack() as ctx:
        # ---------------- constants + persistent tiles ----------------
        pconst = ctx.enter_context(tc.tile_pool(name="const", bufs=1))
        utri = pconst.tile([128, 128], bf16)  # 1.0 where p <= c else 0.0
        make_upper_triangular(nc, utri[:], val=1.0, diag=True)
        bq = pconst.tile([128, 6], f32)  # per-chunk bias vecs: cols 0-2 q, 3-5 k
        ones64 = pconst.tile([1, 64], f32)
        nc.vector.memset(ones64[:], 1.0)
        bv_row = pconst.tile([1, 384], f32)
        bvb = pconst.tile([128, 384], f32)  # bias_v broadcast to 128 partitions

        pqkv = ctx.enter_context(tc.tile_pool(name="qkvout", bufs=1))
        # q/k in fp8 for DoubleRow score matmuls (0.5 cyc/row, effective
        # K=64 via the 2 pair slots).  Host orders the w_qkv columns so
        # slice ncI=0 holds heads 0-3 dims 0-31 (pair slot 0), ncI=1 holds
        # their dims 32-63 (slot 1), and ncI=2 holds heads 4/5 both halves.
        # Per s-chunk: f4 [128,2,512] = heads 0-3, f2 [64,2,512] = heads 4/5.
        # operand partition bases must be 0/32/64, so: f4 serves heads
        # 0/1/2 at bases 0/32/64 (head 3's data is parked at base 96 and
        # copied out), f2 serves heads 4@0, 5@32, 3@64
        # bf16 q/k for chunk 0 (permuted dim order; slot 3 = parked copies
        # of the base-96 rows): the j=0 strips of chunk 0 run in bf16 so the
        # short-softmax rows 0-127 (which set the output's max magnitude)
        # don't carry fp8 score noise
        QTb = pqkv.tile([128, 5, 512], bf16, name="qtb")
        KTb = pqkv.tile([128, 5, 512], bf16, name="ktb")
        Qf4 = [pqkv.tile([128, 2, 512], fp8, name=f"qf4_{sc}") for sc in range(4)]
        Qf2 = [pqkv.tile([96, 2, 512], fp8, name=f"qf2_{sc}") for sc in range(4)]
        Kf4 = [pqkv.tile([128, 2, 512], fp8, name=f"kf4_{sc}") for sc in range(4)]
        Kf2 = [pqkv.tile([96, 2, 512], fp8, name=f"kf2_{sc}") for sc in range(4)]
        # v tiles: per head 128 columns [ones | 63 unused | 64 v-dims], see
        # module docstring
        VV = [pqkv.tile([128, HPC * 128], bf16, name=f"vv{i}") for i in range(16)]
        UT = [pqkv.tile([128, S], bf16, name=f"ut{t}") for t in range(3)]
        wpt = pqkv.tile([128, 3, D], bf16)
        pes = ctx.enter_context(tc.tile_pool(name="espool", bufs=8))
        pnrm = ctx.enter_context(tc.tile_pool(name="nrm", bufs=6))
        prr = ctx.enter_context(tc.tile_pool(name="rrp", bufs=4))
        pout = ctx.enter_context(tc.tile_pool(name="outp", bufs=6))

        # attention PSUM: strips (4 banks) + AV (2 banks) + qkv/proj mm ring
        pst2 = ctx.enter_context(tc.tile_pool(name="stps", space="PSUM", bufs=2))
        pav = ctx.enter_context(tc.tile_pool(name="avps", space="PSUM", bufs=1))
        pmm = ctx.enter_context(tc.tile_pool(name="mmps", space="PSUM", bufs=2))

        p1 = ctx.enter_context(tc.tile_pool(name="ph1", bufs=1))
        wt = p1.tile([128, 6, 1152], bf16)
        xt = [
            [p1.tile([128, 3, 512], bf16, name=f"xt{sc}_{h}") for h in range(2)]
            for sc in range(4)
        ]

        def xts(sc, c):  # c-th 128-row input-dim slice of chunk sc
            return xt[sc][c // 3][:, c % 3, :]
        # The cost model serializes all transfers through one DMA lane, so
        # issue order ~= arrival order.  Gate-first: xt0 (SP queue) || wq, wk
        # (ACT queue), then everything else in need order.
        for h in range(2):
            nc.sync.dma_start(
                xt[0][h][:],
                xT_d[384 * h : 384 * (h + 1), 0:512].rearrange(
                    "(c p) s -> p c s", p=128
                ),
            )
        # q/k weights: bq first (it gates the QT/KT drains), then the
        # ncI=0 slices (gate the first strips), then the wide remainder
        nc.scalar.dma_start(bq[:], bqkv_d[0:768].rearrange("(c p) -> p c", p=128))
        for lo, hi in ((0, 128), (384, 512), (128, 384), (512, 768)):
            nc.scalar.dma_start(
                wt[:, :, lo:hi],
                w_d[:, lo:hi].rearrange("(c p) n -> p c n", p=128),
            )
        nc.scalar.dma_start(
            bv_row[:], bqkv_d[768:1152].rearrange("(o n) -> o n", o=1)
        )
        nc.gpsimd.partition_broadcast(bvb[:], bv_row[:])
        nc.scalar.dma_start(  # wv
            wt[:, :, 768:1152], w_d[:, 768:1152].rearrange("(c p) n -> p c n", p=128)
        )
        for sc in range(1, 4):
            for h in range(2):
                nc.scalar.dma_start(
                    xt[sc][h][:],
                    xT_d[384 * h : 384 * (h + 1), sc * 512 : (sc + 1) * 512]
                    .rearrange("(c p) s -> p c s", p=128),
                )
        nc.scalar.dma_start(wpt[:], wp_d.rearrange("(c p) n -> p c n", p=128))

        # ---------------- emission building blocks ----------------

        def qkv_fillers(sc):
            """qkv chunk sc as a list of ~0.5-0.7us PE closures (half
            accumulation groups). QT/KT slices first (they gate the next
            chunk's exp stream), V tiles after."""
            out = []
            state = {}

            def qk_half(ncI, which, dst, second):
                def run():
                    base = which * 384
                    if not second:
                        state[(ncI, which)] = pmm.tile([128, 512], f32, tag="mm", name=f"qk{sc}_{ncI}_{which}")
                    ps = state[(ncI, which)]
                    for c in range(3, 6) if second else range(3):
                        nc.tensor.matmul(
                            ps[:],
                            lhsT=wt[:, c, base + ncI * 128 : base + (ncI + 1) * 128],
                            rhs=xts(sc, c),
                            start=(c == 0),
                            stop=(c == 5),
                        )
                    if second:
                        cidx = which * 3 + ncI
                        f4, f2, fb = dst
                        if sc == 0:
                            nc.vector.tensor_scalar_add(
                                fb[:, ncI, :], ps[:], bq[:, cidx : cidx + 1]
                            )
                            # park base-96 rows so every head's two
                            # 32-partition pieces share a base (groups
                            # must keep a constant tile row position):
                            # h3 -> (3,64)+(4,64); h4 -> (2,0)+(3,0);
                            # h5 -> (2,32)+(3,32)
                            if ncI == 0:
                                nc.vector.tensor_copy(
                                    fb[64:96, 3, :], fb[96:128, 0, :]
                                )
                            elif ncI == 1:
                                nc.vector.tensor_copy(
                                    fb[64:96, 4, :], fb[96:128, 1, :]
                                )
                            elif ncI == 2:
                                nc.vector.tensor_copy(
                                    fb[0:64, 3, :], fb[64:128, 2, :]
                                )
                        if ncI < 2:
                            # drain on DVE: keeps the ACT stream pure-exp so
                            # drains never throttle the mm ring behind exps
                            nc.vector.tensor_scalar_add(
                                f4[sc][:, ncI, :],
                                ps[:],
                                bq[:, cidx : cidx + 1],
                            )
                            if ncI == 1:
                                # head 3 parked at f4 base 96 -> f2 base 64
                                nc.vector.tensor_copy(
                                    f2[sc][64:96, :, :], f4[sc][96:128, :, :]
                                )
                        else:
                            # heads 4/5 both halves: two partition-shifted
                            # half drains (DVE handles base shifts)
                            for off, slot in ((0, 0), (64, 1)):
                                nc.vector.tensor_scalar_add(
                                    f2[sc][0:64, slot, :],
                                    ps[off : off + 64, :],
                                    bq[off : off + 64, cidx : cidx + 1],
                                )
                return run

            def v_half(i, second):
                def run():
                    if not second:
                        state[("v", i)] = pmm.tile([128, 384], f32, tag="mm", name=f"psv{i}")
                    psv = state[("v", i)]
                    for c in range(3, 6) if second else range(3):
                        nc.tensor.matmul(
                            psv[:],
                            lhsT=xts(sc, c)[:, (i % 4) * 128 : (i % 4 + 1) * 128],
                            rhs=wt[:, c, 768:1152],
                            start=(c == 0),
                            stop=(c == 5),
                        )
                    if second:
                        vt = VV[i][:].rearrange("p (h m) -> p h m", m=128)
                        nc.vector.tensor_tensor(
                            vt[:, :, 64:128],
                            psv[:].rearrange("p (h m) -> p h m", m=64),
                            bvb[:].rearrange("p (h m) -> p h m", m=64),
                            mybir.AluOpType.add,
                        )
                        nc.vector.memset(vt[:, :, 0:1], 1.0)
                return run

            for ncI in range(3):
                for which, dst in ((0, (Qf4, Qf2, QTb)), (1, (Kf4, Kf2, KTb))):
                    out.append(qk_half(ncI, which, dst, False))
                    out.append(qk_half(ncI, which, dst, True))
            for i in range(sc * 4, sc * 4 + 4):
                out.append(v_half(i, False))
                out.append(v_half(i, True))
            return out

        def proj_fillers(c):
            """Projection of chunk c as 1-bank mm-ring pieces (2 per
            s-tile).  Accumulation leads with t=2 (the last-normalized
            pack) so a piece can't start and then block the PE stream."""
            out = []
            state = {}

            def piece(i, half):
                def run():
                    n0, n1 = (0, 512) if half == 0 else (512, 768)
                    po = pmm.tile([128, n1 - n0], f32, tag="mm", name=f"po{i}_{half}")
                    for t in (2, 0, 1):
                        nc.tensor.matmul(
                            po[:],
                            lhsT=UT[t][:, i * 128 : (i + 1) * 128],
                            rhs=wpt[:, t, n0:n1],
                            start=(t == 2),
                            stop=(t == 1),
                        )
                    if half == 0:
                        state[i] = pout.tile([128, D], bf16, tag="ob", name=f"ob{i}")
                    ob = state[i]
                    nc.vector.tensor_copy(ob[:, n0:n1], po[:])
                    if half == 1:
                        nc.sync.dma_start(
                            out_d[i * 128 : (i + 1) * 128, :], ob[:]
                        )
                return run

            for i in range(4 * c, 4 * c + 4):
                out.append(piece(i, 0))
                out.append(piece(i, 1))
            return out

        def attention_chunk(c, fillers, target_pairs=None):
            """Emit chunk c's attention, weaving filler closures between
            strip-pairs (never right before a pack boundary)."""
            g0 = c * 512
            npairs = 3 * (4 * c + 4)
            target = target_pairs if target_pairs else npairs - 2
            emitted = [0]

            def weave(allow=True):
                k = emitted[0] = emitted[0] + 1
                if not allow:
                    return
                total = len(fillers)
                want = min(total, (k * total) // target)
                while weave.done < want:
                    fillers[weave.done]()
                    weave.done += 1
            weave.done = 0

            for t in range(3):
                av = pav.tile([128, 2, 512], f32, tag="av")
                pend_av = []
                for j in range(4 * c + 4):
                    n0 = max(0, j * 128 - g0)
                    W = 512 - n0
                    jc, jr = j // 4, (j % 4) * 128
                    st = pst2.tile([128, 1024], f32, tag="st")
                    if c == 0 and j == 0:
                        # bf16 strips for the shortest-softmax rows; each
                        # head's 64 dims live as two 32-partition pieces
                        BFP = (
                            ((0, 0), (1, 0)),    # h0
                            ((0, 32), (1, 32)),  # h1
                            ((0, 64), (1, 64)),  # h2
                            ((3, 64), (4, 64)),  # h3
                            ((2, 0), (3, 0)),    # h4
                            ((2, 32), (3, 32)),  # h5
                        )
                        for hh in (0, 1):
                            for pi, (sl, rb) in enumerate(BFP[2 * t + hh]):
                                nc.tensor.matmul(
                                    st[:, 512 * hh : 512 * hh + 512],
                                    lhsT=KTb[rb : rb + 32, sl, 0:128],
                                    rhs=QTb[rb : rb + 32, sl, :],
                                    start=(pi == 0),
                                    stop=(pi == 1),
                                )
                    else:
                        # head -> (tile, base): t0: f4@0,f4@32; t1: f4@64,
                        # f2@64; t2: f2@0,f2@32
                        hmap = (
                            ((Qf4, Kf4, 0), (Qf4, Kf4, 32)),
                            ((Qf4, Kf4, 64), (Qf2, Kf2, 64)),
                            ((Qf2, Kf2, 0), (Qf2, Kf2, 32)),
                        )[t]
                        for hh, (qtl, ktl, hb) in enumerate(hmap):
                            nc.tensor.matmul(
                                st[:, 512 * hh : 512 * hh + W],
                                lhsT=ktl[jc][hb : hb + 32, :, jr : jr + 128],
                                rhs=qtl[c][hb : hb + 32, :, n0:512],
                                perf_mode=DR,
                                start=True,
                                stop=True,
                            )
                    es = pes.tile([128, 1024], bf16, tag="es")
                    # 1/sqrt(HD) folded into the exp's free affine scale
                    nc.scalar.activation(
                        es[:].rearrange("p (h w) -> p h w", h=2)[:, :, 0:W],
                        st[:].rearrange("p (h w) -> p h w", h=2)[:, :, 0:W],
                        FT.Exp,
                        scale=0.125,
                    )
                    if j * 128 >= g0:  # diagonal block at start of valid region
                        nc.gpsimd.tensor_tensor(
                            es[:, 0:128], es[:, 0:128], utri[:], MUL
                        )
                        nc.gpsimd.tensor_tensor(
                            es[:, 512:640], es[:, 512:640], utri[:], MUL
                        )
                    def av_mm(j, n0, W, es):
                        def run():
                            last = j == 4 * c + 3
                            nc.tensor.matmul(
                                av[:, 0, n0:512],
                                lhsT=VV[j][:, (2 * t) * 128 : (2 * t + 1) * 128],
                                rhs=es[:, 0:W],
                                start=(j == 0),
                                stop=last,
                            )
                            nc.tensor.matmul(
                                av[:, 1, n0:512],
                                lhsT=VV[j][:, (2 * t + 1) * 128 : (2 * t + 2) * 128],
                                rhs=es[:, 512 : 512 + W],
                                start=(j == 0),
                                stop=last,
                            )
                        return run

                    # delay AV by one j so the in-order PE stream never
                    # commits to an av-slot wait before the next strips
                    pend_av.append(av_mm(j, n0, W, es))
                    if len(pend_av) > 2:
                        pend_av.pop(0)()
                    # c=0: fillers carry this chunk's own V tiles, which the
                    # next AV emission needs -- never defer them
                    weave(allow=(c == 0 or j < 4 * c + 2))
                while pend_av:
                    pend_av.pop(0)()
                # normalize + extract U^T: per-head reciprocal straight off
                # PSUM partition 0, replicate across partitions, then one
                # multiply per head from PSUM.  Split per head to halve the
                # chain latency (av-slot release gates the next pack's AV).
                # The last pack replicates via an f32r PE matmul instead of
                # the gpsimd broadcast -- the PE is idle in the tail and the
                # matmul is 7x faster than the Pool broadcast.
                rsr = prr.tile([1, 2, 512], f32, tag="rr")
                for hh in (0, 1):
                    nc.vector.reciprocal_approx_fast(
                        rsr[0:1, hh, :], av[0:1, hh, :]
                    )
                    rec = pnrm.tile([64, 512], f32, tag="rec", name=f"rc{hh}")
                    nc.gpsimd.partition_broadcast(rec[:], rsr[0:1, hh, :])
                    nc.vector.tensor_tensor(
                        UT[t][64 * hh : 64 * hh + 64, g0 : g0 + 512],
                        av[64:128, hh, :],
                        rec[:],
                        MUL,
                    )
            # anything not woven (short chunks): emit now
            while weave.done < len(fillers):
                fillers[weave.done]()
                weave.done += 1

        def proj_tail():
            # final chunk's projection, 4-wide (the two strip slots + the mm
            # ring are idle; the AV slot is NOT used -- allocating it would
            # insert a ring-wait on the last norm into the PE stream).  All
            # t=0/t=1 accumulation matmuls run first: they only need the
            # already-normalized UT[0]/UT[1] and keep the PE busy (and the
            # clock-ramp warm) while the last pack's norm chain drains; the
            # 8 t=2 matmuls + drains follow.
            pos = {}
            for i in (12, 13):
                po = pst2.tile([128, 1024], f32, tag="st", name=f"pot{i}")
                pos[i] = [po[:, 0:512], po[:, 512:768]]
                for t in (0, 1):
                    for half, (n0, n1) in enumerate(((0, 512), (512, 768))):
                        nc.tensor.matmul(
                            pos[i][half][:],
                            lhsT=UT[t][:, i * 128 : (i + 1) * 128],
                            rhs=wpt[:, t, n0:n1],
                            start=(t == 0),
                            stop=False,
                        )
            for i in (14, 15):
                pos[i] = [
                    pmm.tile([128, 512], f32, tag="mm", name=f"pot{i}a"),
                    pmm.tile([128, 256], f32, tag="mm", name=f"pot{i}b"),
                ]
                for t in (0, 1):
                    for half in (0, 1):
                        nc.tensor.matmul(
                            pos[i][half][:],
                            lhsT=UT[t][:, i * 128 : (i + 1) * 128],
                            rhs=wpt[:, t, [0, 512][half] : [512, 768][half]],
                            start=(t == 0),
                            stop=False,
                        )
            for i in range(12, 16):
                ob = pout.tile([128, D], bf16, tag="ob", name=f"obt{i}")
                for half, (n0, n1) in enumerate(((0, 512), (512, 768))):
                    nc.tensor.matmul(
                        pos[i][half][:],
                        lhsT=UT[2][:, i * 128 : (i + 1) * 128],
                        rhs=wpt[:, 2, n0:n1],
                        start=False,
                        stop=True,
                    )
                    nc.vector.tensor_copy(ob[:, n0:n1], pos[i][half][:])
                nc.sync.dma_start(out_d[i * 128 : (i + 1) * 128, :], ob[:])

        # ---------------- the program ----------------
        f0 = qkv_fillers(0)
        # f0 order: Q0a,Q0b,K0a,K0b, Q1a,Q1b,K1a,K1b, Q2a,Q2b,K2a,K2b, V0..V3
        # ncI 0 AND 1 must complete before the first strip (the strips read
        # both pair slots of the f4 tiles)
        for f in f0[0:8]:
            f()
        # rest of qkv(0): V tiles early (the first AVs need them) woven with
        # the ncI=2 drains (pack t=1/t=2 strips read the f2 tiles)
        rest0 = [
            f0[12], f0[13], f0[8],   # V0 | Q2a
            f0[14], f0[15], f0[9],   # V1 | Q2b
            f0[16], f0[17], f0[10],  # V2 | K2a
            f0[18], f0[19], f0[11],  # V3 | K2b
        ]
        attention_chunk(0, rest0 + qkv_fillers(1), target_pairs=9)
        for c in range(1, 4):
            fillers = qkv_fillers(c + 1) if c < 3 else []
            fillers += proj_fillers(c - 1)
            attention_chunk(c, fillers)
        proj_tail()

    nc.compile()
    return nc


def _get_nc():
    global _built_nc
    if _built_nc is None:
        _built_nc = _build()
    return _built_nc


def _make_in_maps(x, w_qkv, b_qkv, w_proj):
    import ml_dtypes

    bf16 = ml_dtypes.bfloat16
    in_maps = []
    xTb = [np.ascontiguousarray(x[b].T.astype(bf16)) for b in range(B)]
    for core in range(N_CORES):
        b, hh = core // 2, core % 2
        cs = slice(hh * 384, (hh + 1) * 384)
        # head-interleaved column order for the fp8 DoubleRow layout
        # (1/sqrt(64) is applied in the exp's scale, not here)
        perm = np.concatenate(
            [np.arange(h * 64, h * 64 + 32) for h in range(4)]
            + [np.arange(h * 64 + 32, h * 64 + 64) for h in range(4)]
            + [np.arange(h * 64, h * 64 + 32) for h in (4, 5)]
            + [np.arange(h * 64 + 32, h * 64 + 64) for h in (4, 5)]
        )
        wq = w_qkv[:, 0:768][:, cs][:, perm]
        wk = w_qkv[:, 768:1536][:, cs][:, perm]
        wv = w_qkv[:, 1536:2304][:, cs]
        w_in = np.ascontiguousarray(
            np.concatenate([wq, wk, wv], axis=1).astype(bf16)
        )
        bqv = np.concatenate(
            [
                b_qkv[0:768][cs][perm],
                b_qkv[768:1536][cs][perm],
                b_qkv[1536:2304][cs],
            ]
        ).astype(np.float32)
        wp = np.ascontiguousarray(w_proj[cs, :].astype(bf16))
        in_maps.append(
            {
                "xT_in": xTb[b],
                "w_in": w_in,
                "bqkv_in": bqv,
                "wp_in": wp,
            }
        )
    return in_maps


def _run(x, w_qkv, b_qkv, w_proj, b_proj, trace=False):
    from concourse.bass_utils import run_bass_kernel_spmd

    nc = _get_nc()
    in_maps = _make_in_maps(x, w_qkv, b_qkv, w_proj)
    res = run_bass_kernel_spmd(
        nc, in_maps, core_ids=list(range(N_CORES)), trace=trace
    )
    out = np.zeros((B, S, D), np.float32)
    for core in range(N_CORES):
        out[core // 2] += np.asarray(res.results[core]["out"], np.float32)
    out += np.asarray(b_proj, np.float32)[None, None, :]
    return out, res


def kernel(**inputs):
    x = np.asarray(inputs["x"], np.float32)
    w_qkv = np.asarray(inputs["w_qkv"], np.float32)
    b_qkv = np.asarray(inputs["b_qkv"], np.float32)
    w_proj = np.asarray(inputs["w_proj"], np.float32)
    b_proj = np.asarray(inputs["b_proj"], np.float32)
    out, _ = _run(x, w_qkv, b_qkv, w_proj, b_proj, trace=False)
    return out


# revision 45
# speedup vs baseline: 1.0572x; 1.0114x over previous
"""Causal self-attention (B=4, S=2048, D=768, H=12) on 8 trn2 NeuronCores.

Sharding: core c -> (batch b = c//2, head-half hh = c%2). Each core handles
one batch and 6 of the 12 heads: it computes qkv for its 384 q/k/v columns,
full causal attention for its 6 heads, and a partial output projection over
its 384 rows of w_proj. Host sums the two half partials per batch + b_proj.

Device pipeline (bf16 matmul operands / f32 PSUM accumulation):
  x arrives PRE-TRANSPOSED from the host (xT [768, 2048] bf16) so no PE
  transposes are needed.  QT/KT pack 2 heads per 128 partitions (q
  pre-scaled by 1/8); VV v-tiles carry a ones column per head at column 0
  so A@V also yields the softmax rowsum on PSUM partition 0, with the 64
  v-dims at partitions 64-127 (legal partition bases for the custom-DVE
  reciprocal and the extract multiply).

  The Tile scheduler builds static in-order per-engine streams from
  emission order (dependency replay has no timing model), so the emission
  WEAVES the work: attention chunk c's score-strip pairs (PE) + exp (ACT)
  + causal mask (gpsimd) + AV accumulation are interleaved with "filler"
  closures carrying qkv chunk c+1 (half-groups of accumulation matmuls)
  and the projection of chunk c-1 (two 1-bank PSUM pieces per s-tile).
  This keeps the PE stream stocked with ready work at chunk boundaries so
  the exp stream never stalls and the PE clock-ramp never resets.

  Per (c,t) normalization: one DVE reciprocal straight off the PSUM rowsum
  rows, one fused gpsimd partition_broadcast, and two DVE multiplies that
  extract+normalize U^T from PSUM in one pass.  Output: per s-tile
  projection into PSUM, DVE drain to bf16, DMA out (host sums in f32).
"""

import numpy as np

B, S, D, H, HD = 4, 2048, 768, 12, 64
HPC = 6  # heads per core
N_CORES = 8

_built_nc = None


def _build():
    import concourse.bass as bass
    import concourse.mybir as mybir
    from concourse import bacc
    import concourse.tile as tile
    from concourse.masks import make_upper_triangular
    from contextlib import ExitStack

    f32 = mybir.dt.float32
    bf16 = mybir.dt.bfloat16
    fp8 = mybir.dt.float8e4
    DR = mybir.MatmulPerfMode.DoubleRow
    FT = mybir.ActivationFunctionType
    MUL = mybir.AluOpType.mult

    nc = bacc.Bacc("TRN2", target_bir_lowering=False, debug=False)
    # x arrives pre-transposed + pre-cast to bf16 from the host
    xT_d = nc.dram_tensor("xT_in", [D, S], bf16, kind="ExternalInput").ap()
    w_d = nc.dram_tensor("w_in", [D, 1152], bf16, kind="ExternalInput").ap()
    bqkv_d = nc.dram_tensor("bqkv_in", [1152], f32, kind="ExternalInput").ap()
    wp_d = nc.dram_tensor("wp_in", [384, D], bf16, kind="ExternalInput").ap()
    out_d = nc.dram_tensor("out", [S, D], bf16, kind="ExternalOutput").ap()

    with tile.TileContext(nc) as tc, ExitStack() as ctx:
        # ---------------- constants + persistent tiles ----------------
        pconst = ctx.enter_context(tc.tile_pool(name="const", bufs=1))
        utri = pconst.tile([128, 128], bf16)  # 1.0 where p <= c else 0.0
        make_upper_triangular(nc, utri[:], val=1.0, diag=True)
        bq = pconst.tile([128, 6], f32)  # per-chunk bias vecs: cols 0-2 q, 3-5 k
        ones64 = pconst.tile([1, 64], f32)
        nc.vector.memset(ones64[:], 1.0)
        bv_row = pconst.tile([1, 384], f32)
        bvb = pconst.tile([128, 384], f32)  # bias_v broadcast to 128 partitions

        pqkv = ctx.enter_context(tc.tile_pool(name="qkvout", bufs=1))
        # q/k in fp8 for DoubleRow score matmuls (0.5 cyc/row, effective
        # K=64 via the 2 pair slots).  Host orders the w_qkv columns so
        # slice ncI=0 holds heads 0-3 dims 0-31 (pair slot 0), ncI=1 holds
        # their dims 32-63 (slot 1), and ncI=2 holds heads 4/5 both halves.
        # Per s-chunk: f4 [128,2,512] = heads 0-3, f2 [64,2,512] = heads 4/5.
        # operand partition bases must be 0/32/64, so: f4 serves heads
        # 0/1/2 at bases 0/32/64 (head 3's data is parked at base 96 and
        # copied out), f2 serves heads 4@0, 5@32, 3@64
        # bf16 q/k for chunk 0 (permuted dim order; slot 3 = parked copies
        # of the base-96 rows): the j=0 strips of chunk 0 run in bf16 so the
        # short-softmax rows 0-127 (which set the output's max magnitude)
        # don't carry fp8 score noise
        QTb = pqkv.tile([128, 5, 512], bf16, name="qtb")
        KTb = pqkv.tile([128, 5, 512], bf16, name="ktb")
        Qf4 = [pqkv.tile([128, 2, 512], fp8, name=f"qf4_{sc}") for sc in range(4)]
        Qf2 = [pqkv.tile([96, 2, 512], fp8, name=f"qf2_{sc}") for sc in range(4)]
        Kf4 = [pqkv.tile([128, 2, 512], fp8, name=f"kf4_{sc}") for sc in range(4)]
        Kf2 = [pqkv.tile([96, 2, 512], fp8, name=f"kf2_{sc}") for sc in range(4)]
        # v tiles: per head 128 columns [ones | 63 unused | 64 v-dims], see
        # module docstring
        VV = [pqkv.tile([128, HPC * 128], bf16, name=f"vv{i}") for i in range(16)]
        UT = [pqkv.tile([128, S], bf16, name=f"ut{t}") for t in range(3)]
        wpt = pqkv.tile([128, 3, D], bf16)
        pes = ctx.enter_context(tc.tile_pool(name="espool", bufs=8))
        pnrm = ctx.enter_context(tc.tile_pool(name="nrm", bufs=6))
        prr = ctx.enter_context(tc.tile_pool(name="rrp", bufs=4))
        pout = ctx.enter_context(tc.tile_pool(name="outp", bufs=6))

        # attention PSUM: strips (4 banks) + AV (2 banks) + qkv/proj mm ring
        pst2 = ctx.enter_context(tc.tile_pool(name="stps", space="PSUM", bufs=2))
        pav = ctx.enter_context(tc.tile_pool(name="avps", space="PSUM", bufs=1))
        pmm = ctx.enter_context(tc.tile_pool(name="mmps", space="PSUM", bufs=2))

        p1 = ctx.enter_context(tc.tile_pool(name="ph1", bufs=1))
        wt = p1.tile([128, 6, 1152], bf16)
        xt = [
            [p1.tile([128, 3, 512], bf16, name=f"xt{sc}_{h}") for h in range(2)]
            for sc in range(4)
        ]

        def xts(sc, c):  # c-th 128-row input-dim slice of chunk sc
            return xt[sc][c // 3][:, c % 3, :]
        # The cost model serializes all transfers through one DMA lane, so
        # issue order ~= arrival order.  Gate-first: xt0 (SP queue) || wq, wk
        # (ACT queue), then everything else in need order.
        for h in range(2):
            nc.sync.dma_start(
                xt[0][h][:],
                xT_d[384 * h : 384 * (h + 1), 0:512].rearrange(
                    "(c p) s -> p c s", p=128
                ),
            )
        # q/k weights: bq first (it gates the QT/KT drains), then the
        # ncI=0 slices (gate the first strips), then the wide remainder
        nc.scalar.dma_start(bq[:], bqkv_d[0:768].rearrange("(c p) -> p c", p=128))
        for lo, hi in ((0, 128), (384, 512), (128, 384), (512, 768)):
            nc.scalar.dma_start(
                wt[:, :, lo:hi],
                w_d[:, lo:hi].rearrange("(c p) n -> p c n", p=128),
            )
        nc.scalar.dma_start(
            bv_row[:], bqkv_d[768:1152].rearrange("(o n) -> o n", o=1)
        )
        nc.gpsimd.partition_broadcast(bvb[:], bv_row[:])
        nc.scalar.dma_start(  # wv
            wt[:, :, 768:1152], w_d[:, 768:1152].rearrange("(c p) n -> p c n", p=128)
        )
        for sc in range(1, 4):
            for h in range(2):
                nc.scalar.dma_start(
                    xt[sc][h][:],
                    xT_d[384 * h : 384 * (h + 1), sc * 512 : (sc + 1) * 512]
                    .rearrange("(c p) s -> p c s", p=128),
                )
        nc.scalar.dma_start(wpt[:], wp_d.rearrange("(c p) n -> p c n", p=128))

        # ---------------- emission building blocks ----------------

        def qkv_fillers(sc):
            """qkv chunk sc as a list of ~0.5-0.7us PE closures (half
            accumulation groups). QT/KT slices first (they gate the next
            chunk's exp stream), V tiles after."""
            out = []
            state = {}

            def qk_half(ncI, which, dst, second):
                def run():
                    base = which * 384
                    if not second:
                        state[(ncI, which)] = pmm.tile([128, 512], f32, tag="mm", name=f"qk{sc}_{ncI}_{which}")
                    ps = state[(ncI, which)]
                    for c in range(3, 6) if second else range(3):
                        nc.tensor.matmul(
                            ps[:],
                            lhsT=wt[:, c, base + ncI * 128 : base + (ncI + 1) * 128],
                            rhs=xts(sc, c),
                            start=(c == 0),
                            stop=(c == 5),
                        )
                    if second:
                        cidx = which * 3 + ncI
                        f4, f2, fb = dst
                        if sc == 0:
                            nc.vector.tensor_scalar_add(
                                fb[:, ncI, :], ps[:], bq[:, cidx : cidx + 1]
                            )
                            # park base-96 rows so every head's two
                            # 32-partition pieces share a base (groups
                            # must keep a constant tile row position):
                            # h3 -> (3,64)+(4,64); h4 -> (2,0)+(3,0);
                            # h5 -> (2,32)+(3,32)
                            if ncI == 0:
                                nc.vector.tensor_copy(
                                    fb[64:96, 3, :], fb[96:128, 0, :]
                                )
                            elif ncI == 1:
                                nc.vector.tensor_copy(
                                    fb[64:96, 4, :], fb[96:128, 1, :]
                                )
                            elif ncI == 2:
                                nc.vector.tensor_copy(
                                    fb[0:64, 3, :], fb[64:128, 2, :]
                                )
                        if ncI < 2:
                            # drain on DVE: keeps the ACT stream pure-exp so
                            # drains never throttle the mm ring behind exps
                            nc.vector.tensor_scalar_add(
                                f4[sc][:, ncI, :],
                                ps[:],
                                bq[:, cidx : cidx + 1],
                            )
                            if ncI == 1:
                                # head 3 parked at f4 base 96 -> f2 base 64
                                nc.vector.tensor_copy(
                                    f2[sc][64:96, :, :], f4[sc][96:128, :, :]
                                )
                        else:
                            # heads 4/5 both halves: two partition-shifted
                            # half drains (DVE handles base shifts)
                            for off, slot in ((0, 0), (64, 1)):
                                nc.vector.tensor_scalar_add(
                                    f2[sc][0:64, slot, :],
                                    ps[off : off + 64, :],
                                    bq[off : off + 64, cidx : cidx + 1],
                                )
                return run

            def v_half(i, second):
                def run():
                    if not second:
                        state[("v", i)] = pmm.tile([128, 384], f32, tag="mm", name=f"psv{i}")
                    psv = state[("v", i)]
                    for c in range(3, 6) if second else range(3):
                        nc.tensor.matmul(
                            psv[:],
                            lhsT=xts(sc, c)[:, (i % 4) * 128 : (i % 4 + 1) * 128],
                            rhs=wt[:, c, 768:1152],
                            start=(c == 0),
                            stop=(c == 5),
                        )
                    if second:
                        vt = VV[i][:].rearrange("p (h m) -> p h m", m=128)
                        nc.vector.tensor_tensor(
                            vt[:, :, 64:128],
                            psv[:].rearrange("p (h m) -> p h m", m=64),
                            bvb[:].rearrange("p (h m) -> p h m", m=64),
                            mybir.AluOpType.add,
                        )
                        nc.vector.memset(vt[:, :, 0:1], 1.0)
                return run

            for ncI in range(3):
                for which, dst in ((0, (Qf4, Qf2, QTb)), (1, (Kf4, Kf2, KTb))):
                    out.append(qk_half(ncI, which, dst, False))
                    out.append(qk_half(ncI, which, dst, True))
            for i in range(sc * 4, sc * 4 + 4):
                out.append(v_half(i, False))
                out.append(v_half(i, True))
            return out

        def proj_fillers(c):
            """Projection of chunk c as 1-bank mm-ring pieces (2 per
            s-tile).  Accumulation leads with t=2 (the last-normalized
            pack) so a piece can't start and then block the PE stream."""
            out = []
            state = {}

            def piece(i, half):
                def run():
                    n0, n1 = (0, 512) if half == 0 else (512, 768)
                    po = pmm.tile([128, n1 - n0], f32, tag="mm", name=f"po{i}_{half}")
                    for t in (2, 0, 1):
                        nc.tensor.matmul(
                            po[:],
                            lhsT=UT[t][:, i * 128 : (i + 1) * 128],
                            rhs=wpt[:, t, n0:n1],
                            start=(t == 2),
                            stop=(t == 1),
                        )
                    if half == 0:
                        state[i] = pout.tile([128, D], bf16, tag="ob", name=f"ob{i}")
                    ob = state[i]
                    nc.vector.tensor_copy(ob[:, n0:n1], po[:])
                    if half == 1:
                        nc.sync.dma_start(
                            out_d[i * 128 : (i + 1) * 128, :], ob[:]
                        )
                return run

            for i in range(4 * c, 4 * c + 4):
                out.append(piece(i, 0))
                out.append(piece(i, 1))
            return out

        def attention_chunk(c, fillers, target_pairs=None):
            """Emit chunk c's attention, weaving filler closures between
            strip-pairs (never right before a pack boundary)."""
            g0 = c * 512
            npairs = 3 * (4 * c + 4)
            target = target_pairs if target_pairs else npairs - 2
            emitted = [0]

            def weave(allow=True):
                k = emitted[0] = emitted[0] + 1
                if not allow:
                    return
                total = len(fillers)
                want = min(total, (k * total) // target)
                while weave.done < want:
                    fillers[weave.done]()
                    weave.done += 1
            weave.done = 0

            for t in range(3):
                av = pav.tile([128, 2, 512], f32, tag="av")
                pend_av = []
                for j in range(4 * c + 4):
                    n0 = max(0, j * 128 - g0)
                    W = 512 - n0
                    jc, jr = j // 4, (j % 4) * 128
                    st = pst2.tile([128, 1024], f32, tag="st")
                    if c == 0 and j == 0:
                        # bf16 strips for the shortest-softmax rows; each
                        # head's 64 dims live as two 32-partition pieces
                        BFP = (
                            ((0, 0), (1, 0)),    # h0
                            ((0, 32), (1, 32)),  # h1
                            ((0, 64), (1, 64)),  # h2
                            ((3, 64), (4, 64)),  # h3
                            ((2, 0), (3, 0)),    # h4
                            ((2, 32), (3, 32)),  # h5
                        )
                        for hh in (0, 1):
                            for pi, (sl, rb) in enumerate(BFP[2 * t + hh]):
                                nc.tensor.matmul(
                                    st[:, 512 * hh : 512 * hh + 512],
                                    lhsT=KTb[rb : rb + 32, sl, 0:128],
                                    rhs=QTb[rb : rb + 32, sl, :],
                                    start=(pi == 0),
                                    stop=(pi == 1),
                                )
                    else:
                        # head -> (tile, base): t0: f4@0,f4@32; t1: f4@64,
                        # f2@64; t2: f2@0,f2@32
                        hmap = (
                            ((Qf4, Kf4, 0), (Qf4, Kf4, 32)),
                            ((Qf4, Kf4, 64), (Qf2, Kf2, 64)),
                            ((Qf2, Kf2, 0), (Qf2, Kf2, 32)),
                        )[t]
                        for hh, (qtl, ktl, hb) in enumerate(hmap):
                            nc.tensor.matmul(
                                st[:, 512 * hh : 512 * hh + W],
                                lhsT=ktl[jc][hb : hb + 32, :, jr : jr + 128],
                                rhs=qtl[c][hb : hb + 32, :, n0:512],
                                perf_mode=DR,
                                start=True,
                                stop=True,
                            )
                    es = pes.tile([128, 1024], bf16, tag="es")
                    # 1/sqrt(HD) folded into the exp's free affine scale
                    nc.scalar.activation(
                        es[:].rearrange("p (h w) -> p h w", h=2)[:, :, 0:W],
                        st[:].rearrange("p (h w) -> p h w", h=2)[:, :, 0:W],
                        FT.Exp,
                        scale=0.125,
                    )
                    if j * 128 >= g0:  # diagonal block at start of valid region
                        nc.gpsimd.tensor_tensor(
                            es[:, 0:128], es[:, 0:128], utri[:], MUL
                        )
                        nc.gpsimd.tensor_tensor(
                            es[:, 512:640], es[:, 512:640], utri[:], MUL
                        )
                    def av_mm(j, n0, W, es):
                        def run():
                            last = j == 4 * c + 3
                            nc.tensor.matmul(
                                av[:, 0, n0:512],
                                lhsT=VV[j][:, (2 * t) * 128 : (2 * t + 1) * 128],
                                rhs=es[:, 0:W],
                                start=(j == 0),
                                stop=last,
                            )
                            nc.tensor.matmul(
                                av[:, 1, n0:512],
                                lhsT=VV[j][:, (2 * t + 1) * 128 : (2 * t + 2) * 128],
                                rhs=es[:, 512 : 512 + W],
                                start=(j == 0),
                                stop=last,
                            )
                        return run

                    # delay AV by one j so the in-order PE stream never
                    # commits to an av-slot wait before the next strips
                    pend_av.append(av_mm(j, n0, W, es))
                    if len(pend_av) > 3:
                        pend_av.pop(0)()
                    # c=0: fillers carry this chunk's own V tiles, which the
                    # next AV emission needs -- never defer them
                    weave(allow=(c == 0 or j < 4 * c + 2))
                while pend_av:
                    pend_av.pop(0)()
                # normalize + extract U^T: per-head reciprocal straight off
                # PSUM partition 0, replicate across partitions, then one
                # multiply per head from PSUM.  Split per head to halve the
                # chain latency (av-slot release gates the next pack's AV).
                # The last pack replicates via an f32r PE matmul instead of
                # the gpsimd broadcast -- the PE is idle in the tail and the
                # matmul is 7x faster than the Pool broadcast.
                rsr = prr.tile([1, 2, 512], f32, tag="rr")
                for hh in (0, 1):
                    nc.vector.reciprocal_approx_fast(
                        rsr[0:1, hh, :], av[0:1, hh, :]
                    )
                    rec = pnrm.tile([64, 512], f32, tag="rec", name=f"rc{hh}")
                    nc.gpsimd.partition_broadcast(rec[:], rsr[0:1, hh, :])
                    nc.vector.tensor_tensor(
                        UT[t][64 * hh : 64 * hh + 64, g0 : g0 + 512],
                        av[64:128, hh, :],
                        rec[:],
                        MUL,
                    )
            # anything not woven (short chunks): emit now
            while weave.done < len(fillers):
                fillers[weave.done]()
                weave.done += 1

        def proj_tail():
            # final chunk's projection, 4-wide (the two strip slots + the mm
            # ring are idle; the AV slot is NOT used -- allocating it would
            # insert a ring-wait on the last norm into the PE stream).  All
            # t=0/t=1 accumulation matmuls run first: they only need the
            # already-normalized UT[0]/UT[1] and keep the PE busy (and the
            # clock-ramp warm) while the last pack's norm chain drains; the
            # 8 t=2 matmuls + drains follow.
            pos = {}
            for i in (12, 13):
                po = pst2.tile([128, 1024], f32, tag="st", name=f"pot{i}")
                pos[i] = [po[:, 0:512], po[:, 512:768]]
                for t in (0, 1):
                    for half, (n0, n1) in enumerate(((0, 512), (512, 768))):
                        nc.tensor.matmul(
                            pos[i][half][:],
                            lhsT=UT[t][:, i * 128 : (i + 1) * 128],
                            rhs=wpt[:, t, n0:n1],
                            start=(t == 0),
                            stop=False,
                        )
            for i in (14, 15):
                pos[i] = [
                    pmm.tile([128, 512], f32, tag="mm", name=f"pot{i}a"),
                    pmm.tile([128, 256], f32, tag="mm", name=f"pot{i}b"),
                ]
                for t in (0, 1):
                    for half in (0, 1):
                        nc.tensor.matmul(
                            pos[i][half][:],
                            lhsT=UT[t][:, i * 128 : (i + 1) * 128],
                            rhs=wpt[:, t, [0, 512][half] : [512, 768][half]],
                            start=(t == 0),
                            stop=False,
                        )
            for i in range(12, 16):
                ob = pout.tile([128, D], bf16, tag="ob", name=f"obt{i}")
                for half, (n0, n1) in enumerate(((0, 512), (512, 768))):
                    nc.tensor.matmul(
                        pos[i][half][:],
                        lhsT=UT[2][:, i * 128 : (i + 1) * 128],
                        rhs=wpt[:, 2, n0:n1],
                        start=False,
                        stop=True,
                    )
                    nc.vector.tensor_copy(ob[:, n0:n1], pos[i][half][:])
                nc.sync.dma_start(out_d[i * 128 : (i + 1) * 128, :], ob[:])

        # ---------------- the program ----------------
        f0 = qkv_fillers(0)
        # f0 order: Q0a,Q0b,K0a,K0b, Q1a,Q1b,K1a,K1b, Q2a,Q2b,K2a,K2b, V0..V3
        # ncI 0 AND 1 must complete before the first strip (the strips read
        # both pair slots of the f4 tiles)
        for f in f0[0:8]:
            f()
        # rest of qkv(0): V tiles early (the first AVs need them) woven with
        # the ncI=2 drains (pack t=1/t=2 strips read the f2 tiles)
        rest0 = [
            f0[12], f0[13], f0[8],   # V0 | Q2a
            f0[14], f0[15], f0[9],   # V1 | Q2b
            f0[16], f0[17], f0[10],  # V2 | K2a
            f0[18], f0[19], f0[11],  # V3 | K2b
        ]
        attention_chunk(0, rest0 + qkv_fillers(1), target_pairs=9)
        for c in range(1, 4):
            fillers = qkv_fillers(c + 1) if c < 3 else []
            fillers += proj_fillers(c - 1)
            attention_chunk(c, fillers)
        proj_tail()

    nc.compile()
    return nc


def _get_nc():
    global _built_nc
    if _built_nc is None:
        _built_nc = _build()
    return _built_nc


def _make_in_maps(x, w_qkv, b_qkv, w_proj):
    import ml_dtypes

    bf16 = ml_dtypes.bfloat16
    in_maps = []
    xTb = [np.ascontiguousarray(x[b].T.astype(bf16)) for b in range(B)]
    for core in range(N_CORES):
        b, hh = core // 2, core % 2
        cs = slice(hh * 384, (hh + 1) * 384)
        # head-interleaved column order for the fp8 DoubleRow layout
        # (1/sqrt(64) is applied in the exp's scale, not here)
        perm = np.concatenate(
            [np.arange(h * 64, h * 64 + 32) for h in range(4)]
            + [np.arange(h * 64 + 32, h * 64 + 64) for h in range(4)]
            + [np.arange(h * 64, h * 64 + 32) for h in (4, 5)]
            + [np.arange(h * 64 + 32, h * 64 + 64) for h in (4, 5)]
        )
        wq = w_qkv[:, 0:768][:, cs][:, perm]
        wk = w_qkv[:, 768:1536][:, cs][:, perm]
        wv = w_qkv[:, 1536:2304][:, cs]
        w_in = np.ascontiguousarray(
            np.concatenate([wq, wk, wv], axis=1).astype(bf16)
        )
        bqv = np.concatenate(
            [
                b_qkv[0:768][cs][perm],
                b_qkv[768:1536][cs][perm],
                b_qkv[1536:2304][cs],
            ]
        ).astype(np.float32)
        wp = np.ascontiguousarray(w_proj[cs, :].astype(bf16))
        in_maps.append(
            {
                "xT_in": xTb[b],
                "w_in": w_in,
                "bqkv_in": bqv,
                "wp_in": wp,
            }
        )
    return in_maps


def _run(x, w_qkv, b_qkv, w_proj, b_proj, trace=False):
    from concourse.bass_utils import run_bass_kernel_spmd

    nc = _get_nc()
    in_maps = _make_in_maps(x, w_qkv, b_qkv, w_proj)
    res = run_bass_kernel_spmd(
        nc, in_maps, core_ids=list(range(N_CORES)), trace=trace
    )
    out = np.zeros((B, S, D), np.float32)
    for core in range(N_CORES):
        out[core // 2] += np.asarray(res.results[core]["out"], np.float32)
    out += np.asarray(b_proj, np.float32)[None, None, :]
    return out, res


def kernel(**inputs):
    x = np.asarray(inputs["x"], np.float32)
    w_qkv = np.asarray(inputs["w_qkv"], np.float32)
    b_qkv = np.asarray(inputs["b_qkv"], np.float32)
    w_proj = np.asarray(inputs["w_proj"], np.float32)
    b_proj = np.asarray(inputs["b_proj"], np.float32)
    out, _ = _run(x, w_qkv, b_qkv, w_proj, b_proj, trace=False)
    return out


# revision 46
# speedup vs baseline: 1.0811x; 1.0225x over previous
"""Causal self-attention (B=4, S=2048, D=768, H=12) on 8 trn2 NeuronCores.

Sharding: core c -> (batch b = c//2, head-half hh = c%2). Each core handles
one batch and 6 of the 12 heads: it computes qkv for its 384 q/k/v columns,
full causal attention for its 6 heads, and a partial output projection over
its 384 rows of w_proj. Host sums the two half partials per batch + b_proj.

Device pipeline (bf16 matmul operands / f32 PSUM accumulation):
  x arrives PRE-TRANSPOSED from the host (xT [768, 2048] bf16) so no PE
  transposes are needed.  QT/KT pack 2 heads per 128 partitions (q
  pre-scaled by 1/8); VV v-tiles carry a ones column per head at column 0
  so A@V also yields the softmax rowsum on PSUM partition 0, with the 64
  v-dims at partitions 64-127 (legal partition bases for the custom-DVE
  reciprocal and the extract multiply).

  The Tile scheduler builds static in-order per-engine streams from
  emission order (dependency replay has no timing model), so the emission
  WEAVES the work: attention chunk c's score-strip pairs (PE) + exp (ACT)
  + causal mask (gpsimd) + AV accumulation are interleaved with "filler"
  closures carrying qkv chunk c+1 (half-groups of accumulation matmuls)
  and the projection of chunk c-1 (two 1-bank PSUM pieces per s-tile).
  This keeps the PE stream stocked with ready work at chunk boundaries so
  the exp stream never stalls and the PE clock-ramp never resets.

  Per (c,t) normalization: one DVE reciprocal straight off the PSUM rowsum
  rows, one fused gpsimd partition_broadcast, and two DVE multiplies that
  extract+normalize U^T from PSUM in one pass.  Output: per s-tile
  projection into PSUM, DVE drain to bf16, DMA out (host sums in f32).
"""

import numpy as np

B, S, D, H, HD = 4, 2048, 768, 12, 64
HPC = 6  # heads per core
N_CORES = 8

_built_nc = None


def _build():
    import concourse.bass as bass
    import concourse.mybir as mybir
    from concourse import bacc
    import concourse.tile as tile
    from concourse.masks import make_upper_triangular
    from contextlib import ExitStack

    f32 = mybir.dt.float32
    bf16 = mybir.dt.bfloat16
    fp8 = mybir.dt.float8e4
    DR = mybir.MatmulPerfMode.DoubleRow
    FT = mybir.ActivationFunctionType
    MUL = mybir.AluOpType.mult

    nc = bacc.Bacc("TRN2", target_bir_lowering=False, debug=False)
    # x arrives pre-transposed + pre-cast to bf16 from the host
    xT_d = nc.dram_tensor("xT_in", [D, S], bf16, kind="ExternalInput").ap()
    w_d = nc.dram_tensor("w_in", [D, 1152], bf16, kind="ExternalInput").ap()
    bqkv_d = nc.dram_tensor("bqkv_in", [1152], f32, kind="ExternalInput").ap()
    wp_d = nc.dram_tensor("wp_in", [384, D], bf16, kind="ExternalInput").ap()
    out_d = nc.dram_tensor("out", [S, D], bf16, kind="ExternalOutput").ap()

    with tile.TileContext(nc) as tc, ExitStack() as ctx:
        # ---------------- constants + persistent tiles ----------------
        pconst = ctx.enter_context(tc.tile_pool(name="const", bufs=1))
        utri = pconst.tile([128, 128], bf16)  # 1.0 where p <= c else 0.0
        make_upper_triangular(nc, utri[:], val=1.0, diag=True)
        bq = pconst.tile([128, 6], f32)  # per-chunk bias vecs: cols 0-2 q, 3-5 k
        ones64 = pconst.tile([1, 64], f32)
        nc.vector.memset(ones64[:], 1.0)
        bv_row = pconst.tile([1, 384], f32)
        bvb = pconst.tile([128, 384], f32)  # bias_v broadcast to 128 partitions

        pqkv = ctx.enter_context(tc.tile_pool(name="qkvout", bufs=1))
        # q/k in fp8 for DoubleRow score matmuls (0.5 cyc/row, effective
        # K=64 via the 2 pair slots).  Host orders the w_qkv columns so
        # slice ncI=0 holds heads 0-3 dims 0-31 (pair slot 0), ncI=1 holds
        # their dims 32-63 (slot 1), and ncI=2 holds heads 4/5 both halves.
        # Per s-chunk: f4 [128,2,512] = heads 0-3, f2 [64,2,512] = heads 4/5.
        # operand partition bases must be 0/32/64, so: f4 serves heads
        # 0/1/2 at bases 0/32/64 (head 3's data is parked at base 96 and
        # copied out), f2 serves heads 4@0, 5@32, 3@64
        # bf16 q/k for chunk 0 (permuted dim order; slot 3 = parked copies
        # of the base-96 rows): the j=0 strips of chunk 0 run in bf16 so the
        # short-softmax rows 0-127 (which set the output's max magnitude)
        # don't carry fp8 score noise
        QTb = pqkv.tile([128, 5, 512], bf16, name="qtb")
        KTb = pqkv.tile([128, 5, 512], bf16, name="ktb")
        Qf4 = [pqkv.tile([128, 2, 512], fp8, name=f"qf4_{sc}") for sc in range(4)]
        Qf2 = [pqkv.tile([96, 2, 512], fp8, name=f"qf2_{sc}") for sc in range(4)]
        Kf4 = [pqkv.tile([128, 2, 512], fp8, name=f"kf4_{sc}") for sc in range(4)]
        Kf2 = [pqkv.tile([96, 2, 512], fp8, name=f"kf2_{sc}") for sc in range(4)]
        # v tiles: per head 128 columns [ones | 63 unused | 64 v-dims], see
        # module docstring
        VV = [pqkv.tile([128, HPC * 128], bf16, name=f"vv{i}") for i in range(16)]
        UT = [pqkv.tile([128, S], bf16, name=f"ut{t}") for t in range(3)]
        wpt = pqkv.tile([128, 3, D], bf16)
        pes = ctx.enter_context(tc.tile_pool(name="espool", bufs=8))
        pnrm = ctx.enter_context(tc.tile_pool(name="nrm", bufs=6))
        prr = ctx.enter_context(tc.tile_pool(name="rrp", bufs=4))
        pout = ctx.enter_context(tc.tile_pool(name="outp", bufs=6))

        # attention PSUM: strips (4 banks) + AV (2 banks) + qkv/proj mm ring
        pst2 = ctx.enter_context(tc.tile_pool(name="stps", space="PSUM", bufs=2))
        pav = ctx.enter_context(tc.tile_pool(name="avps", space="PSUM", bufs=1))
        pmm = ctx.enter_context(tc.tile_pool(name="mmps", space="PSUM", bufs=2))

        p1 = ctx.enter_context(tc.tile_pool(name="ph1", bufs=1))
        wt = p1.tile([128, 6, 1152], bf16)
        xt = [
            [p1.tile([128, 3, 512], bf16, name=f"xt{sc}_{h}") for h in range(2)]
            for sc in range(4)
        ]

        def xts(sc, c):  # c-th 128-row input-dim slice of chunk sc
            return xt[sc][c // 3][:, c % 3, :]
        # The cost model serializes all transfers through one DMA lane, so
        # issue order ~= arrival order.  Gate-first: xt0 (SP queue) || wq, wk
        # (ACT queue), then everything else in need order.
        for h in range(2):
            nc.sync.dma_start(
                xt[0][h][:],
                xT_d[384 * h : 384 * (h + 1), 0:512].rearrange(
                    "(c p) s -> p c s", p=128
                ),
            )
        # q/k weights: bq first (it gates the QT/KT drains), then the
        # ncI=0 slices (gate the first strips), then the wide remainder
        nc.scalar.dma_start(bq[:], bqkv_d[0:768].rearrange("(c p) -> p c", p=128))
        for lo, hi in ((0, 128), (384, 512), (128, 384), (512, 768)):
            nc.scalar.dma_start(
                wt[:, :, lo:hi],
                w_d[:, lo:hi].rearrange("(c p) n -> p c n", p=128),
            )
        nc.scalar.dma_start(
            bv_row[:], bqkv_d[768:1152].rearrange("(o n) -> o n", o=1)
        )
        nc.gpsimd.partition_broadcast(bvb[:], bv_row[:])
        nc.scalar.dma_start(  # wv
            wt[:, :, 768:1152], w_d[:, 768:1152].rearrange("(c p) n -> p c n", p=128)
        )
        for sc in range(1, 4):
            for h in range(2):
                nc.scalar.dma_start(
                    xt[sc][h][:],
                    xT_d[384 * h : 384 * (h + 1), sc * 512 : (sc + 1) * 512]
                    .rearrange("(c p) s -> p c s", p=128),
                )
        nc.scalar.dma_start(wpt[:], wp_d.rearrange("(c p) n -> p c n", p=128))

        # ---------------- emission building blocks ----------------

        def qkv_fillers(sc):
            """qkv chunk sc as a list of ~0.5-0.7us PE closures (half
            accumulation groups). QT/KT slices first (they gate the next
            chunk's exp stream), V tiles after."""
            out = []
            state = {}

            def qk_half(ncI, which, dst, second):
                def run():
                    base = which * 384
                    if not second:
                        state[(ncI, which)] = pmm.tile([128, 512], f32, tag="mm", name=f"qk{sc}_{ncI}_{which}")
                    ps = state[(ncI, which)]
                    for c in range(3, 6) if second else range(3):
                        nc.tensor.matmul(
                            ps[:],
                            lhsT=wt[:, c, base + ncI * 128 : base + (ncI + 1) * 128],
                            rhs=xts(sc, c),
                            start=(c == 0),
                            stop=(c == 5),
                        )
                    if second:
                        cidx = which * 3 + ncI
                        f4, f2, fb = dst
                        if sc == 0:
                            nc.vector.tensor_scalar_add(
                                fb[:, ncI, :], ps[:], bq[:, cidx : cidx + 1]
                            )
                            # park base-96 rows so every head's two
                            # 32-partition pieces share a base (groups
                            # must keep a constant tile row position):
                            # h3 -> (3,64)+(4,64); h4 -> (2,0)+(3,0);
                            # h5 -> (2,32)+(3,32)
                            if ncI == 0:
                                nc.vector.tensor_copy(
                                    fb[64:96, 3, :], fb[96:128, 0, :]
                                )
                            elif ncI == 1:
                                nc.vector.tensor_copy(
                                    fb[64:96, 4, :], fb[96:128, 1, :]
                                )
                            elif ncI == 2:
                                nc.vector.tensor_copy(
                                    fb[0:64, 3, :], fb[64:128, 2, :]
                                )
                        if ncI < 2:
                            # drain on DVE: keeps the ACT stream pure-exp so
                            # drains never throttle the mm ring behind exps
                            nc.vector.tensor_scalar_add(
                                f4[sc][:, ncI, :],
                                ps[:],
                                bq[:, cidx : cidx + 1],
                            )
                            if ncI == 1:
                                # head 3 parked at f4 base 96 -> f2 base 64
                                nc.vector.tensor_copy(
                                    f2[sc][64:96, :, :], f4[sc][96:128, :, :]
                                )
                        else:
                            # heads 4/5 both halves: two partition-shifted
                            # half drains (DVE handles base shifts)
                            for off, slot in ((0, 0), (64, 1)):
                                nc.vector.tensor_scalar_add(
                                    f2[sc][0:64, slot, :],
                                    ps[off : off + 64, :],
                                    bq[off : off + 64, cidx : cidx + 1],
                                )
                return run

            def v_half(i, second):
                def run():
                    if not second:
                        state[("v", i)] = pmm.tile([128, 384], f32, tag="mm", name=f"psv{i}")
                    psv = state[("v", i)]
                    for c in range(3, 6) if second else range(3):
                        nc.tensor.matmul(
                            psv[:],
                            lhsT=xts(sc, c)[:, (i % 4) * 128 : (i % 4 + 1) * 128],
                            rhs=wt[:, c, 768:1152],
                            start=(c == 0),
                            stop=(c == 5),
                        )
                    if second:
                        vt = VV[i][:].rearrange("p (h m) -> p h m", m=128)
                        nc.vector.tensor_tensor(
                            vt[:, :, 64:128],
                            psv[:].rearrange("p (h m) -> p h m", m=64),
                            bvb[:].rearrange("p (h m) -> p h m", m=64),
                            mybir.AluOpType.add,
                        )
                        nc.vector.memset(vt[:, :, 0:1], 1.0)
                return run

            for ncI in range(3):
                for which, dst in ((0, (Qf4, Qf2, QTb)), (1, (Kf4, Kf2, KTb))):
                    out.append(qk_half(ncI, which, dst, False))
                    out.append(qk_half(ncI, which, dst, True))
            for i in range(sc * 4, sc * 4 + 4):
                out.append(v_half(i, False))
                out.append(v_half(i, True))
            return out

        def proj_fillers(c):
            """Projection of chunk c as 1-bank mm-ring pieces (2 per
            s-tile).  Accumulation leads with t=2 (the last-normalized
            pack) so a piece can't start and then block the PE stream."""
            out = []
            state = {}

            def piece(i, half):
                def run():
                    n0, n1 = (0, 512) if half == 0 else (512, 768)
                    po = pmm.tile([128, n1 - n0], f32, tag="mm", name=f"po{i}_{half}")
                    for t in (2, 0, 1):
                        nc.tensor.matmul(
                            po[:],
                            lhsT=UT[t][:, i * 128 : (i + 1) * 128],
                            rhs=wpt[:, t, n0:n1],
                            start=(t == 2),
                            stop=(t == 1),
                        )
                    if half == 0:
                        state[i] = pout.tile([128, D], bf16, tag="ob", name=f"ob{i}")
                    ob = state[i]
                    nc.vector.tensor_copy(ob[:, n0:n1], po[:])
                    if half == 1:
                        nc.sync.dma_start(
                            out_d[i * 128 : (i + 1) * 128, :], ob[:]
                        )
                return run

            for i in range(4 * c, 4 * c + 4):
                out.append(piece(i, 0))
                out.append(piece(i, 1))
            return out

        def attention_chunk(c, fillers, target_pairs=None):
            """Emit chunk c's attention, weaving filler closures between
            strip-pairs (never right before a pack boundary)."""
            g0 = c * 512
            npairs = 3 * (4 * c + 4)
            target = target_pairs if target_pairs else npairs - 2
            emitted = [0]

            def weave(allow=True):
                k = emitted[0] = emitted[0] + 1
                if not allow:
                    return
                total = len(fillers)
                want = min(total, (k * total) // target)
                while weave.done < want:
                    fillers[weave.done]()
                    weave.done += 1
            weave.done = 0

            for t in range(3):
                av = pav.tile([128, 2, 512], f32, tag="av")
                pend_av = []
                for j in range(4 * c + 4):
                    n0 = max(0, j * 128 - g0)
                    W = 512 - n0
                    jc, jr = j // 4, (j % 4) * 128
                    st = pst2.tile([128, 1024], f32, tag="st")
                    if c == 0 and j == 0:
                        # bf16 strips for the shortest-softmax rows; each
                        # head's 64 dims live as two 32-partition pieces
                        BFP = (
                            ((0, 0), (1, 0)),    # h0
                            ((0, 32), (1, 32)),  # h1
                            ((0, 64), (1, 64)),  # h2
                            ((3, 64), (4, 64)),  # h3
                            ((2, 0), (3, 0)),    # h4
                            ((2, 32), (3, 32)),  # h5
                        )
                        for hh in (0, 1):
                            for pi, (sl, rb) in enumerate(BFP[2 * t + hh]):
                                nc.tensor.matmul(
                                    st[:, 512 * hh : 512 * hh + 512],
                                    lhsT=KTb[rb : rb + 32, sl, 0:128],
                                    rhs=QTb[rb : rb + 32, sl, :],
                                    start=(pi == 0),
                                    stop=(pi == 1),
                                )
                    else:
                        # head -> (tile, base): t0: f4@0,f4@32; t1: f4@64,
                        # f2@64; t2: f2@0,f2@32
                        hmap = (
                            ((Qf4, Kf4, 0), (Qf4, Kf4, 32)),
                            ((Qf4, Kf4, 64), (Qf2, Kf2, 64)),
                            ((Qf2, Kf2, 0), (Qf2, Kf2, 32)),
                        )[t]
                        for hh, (qtl, ktl, hb) in enumerate(hmap):
                            nc.tensor.matmul(
                                st[:, 512 * hh : 512 * hh + W],
                                lhsT=ktl[jc][hb : hb + 32, :, jr : jr + 128],
                                rhs=qtl[c][hb : hb + 32, :, n0:512],
                                perf_mode=DR,
                                start=True,
                                stop=True,
                            )
                    es = pes.tile([128, 1024], bf16, tag="es")
                    # 1/sqrt(HD) folded into the exp's free affine scale
                    nc.scalar.activation(
                        es[:].rearrange("p (h w) -> p h w", h=2)[:, :, 0:W],
                        st[:].rearrange("p (h w) -> p h w", h=2)[:, :, 0:W],
                        FT.Exp,
                        scale=0.125,
                    )
                    if j * 128 >= g0:  # diagonal block at start of valid region
                        nc.gpsimd.tensor_tensor(
                            es[:, 0:128], es[:, 0:128], utri[:], MUL
                        )
                        nc.gpsimd.tensor_tensor(
                            es[:, 512:640], es[:, 512:640], utri[:], MUL
                        )
                    def av_mm(j, n0, W, es):
                        def run():
                            last = j == 4 * c + 3
                            nc.tensor.matmul(
                                av[:, 0, n0:512],
                                lhsT=VV[j][:, (2 * t) * 128 : (2 * t + 1) * 128],
                                rhs=es[:, 0:W],
                                start=(j == 0),
                                stop=last,
                            )
                            nc.tensor.matmul(
                                av[:, 1, n0:512],
                                lhsT=VV[j][:, (2 * t + 1) * 128 : (2 * t + 2) * 128],
                                rhs=es[:, 512 : 512 + W],
                                start=(j == 0),
                                stop=last,
                            )
                        return run

                    # delay AV by one j so the in-order PE stream never
                    # commits to an av-slot wait before the next strips
                    pend_av.append(av_mm(j, n0, W, es))
                    if len(pend_av) > 4:
                        pend_av.pop(0)()
                    # c=0: fillers carry this chunk's own V tiles, which the
                    # next AV emission needs -- never defer them
                    weave(allow=(c == 0 or j < 4 * c + 2))
                while pend_av:
                    pend_av.pop(0)()
                # normalize + extract U^T: per-head reciprocal straight off
                # PSUM partition 0, replicate across partitions, then one
                # multiply per head from PSUM.  Split per head to halve the
                # chain latency (av-slot release gates the next pack's AV).
                # The last pack replicates via an f32r PE matmul instead of
                # the gpsimd broadcast -- the PE is idle in the tail and the
                # matmul is 7x faster than the Pool broadcast.
                rsr = prr.tile([1, 2, 512], f32, tag="rr")
                for hh in (0, 1):
                    nc.vector.reciprocal_approx_fast(
                        rsr[0:1, hh, :], av[0:1, hh, :]
                    )
                    rec = pnrm.tile([64, 512], f32, tag="rec", name=f"rc{hh}")
                    nc.gpsimd.partition_broadcast(rec[:], rsr[0:1, hh, :])
                    nc.vector.tensor_tensor(
                        UT[t][64 * hh : 64 * hh + 64, g0 : g0 + 512],
                        av[64:128, hh, :],
                        rec[:],
                        MUL,
                    )
            # anything not woven (short chunks): emit now
            while weave.done < len(fillers):
                fillers[weave.done]()
                weave.done += 1

        def proj_tail():
            # final chunk's projection, 4-wide (the two strip slots + the mm
            # ring are idle; the AV slot is NOT used -- allocating it would
            # insert a ring-wait on the last norm into the PE stream).  All
            # t=0/t=1 accumulation matmuls run first: they only need the
            # already-normalized UT[0]/UT[1] and keep the PE busy (and the
            # clock-ramp warm) while the last pack's norm chain drains; the
            # 8 t=2 matmuls + drains follow.
            pos = {}
            for i in (12, 13):
                po = pst2.tile([128, 1024], f32, tag="st", name=f"pot{i}")
                pos[i] = [po[:, 0:512], po[:, 512:768]]
                for t in (0, 1):
                    for half, (n0, n1) in enumerate(((0, 512), (512, 768))):
                        nc.tensor.matmul(
                            pos[i][half][:],
                            lhsT=UT[t][:, i * 128 : (i + 1) * 128],
                            rhs=wpt[:, t, n0:n1],
                            start=(t == 0),
                            stop=False,
                        )
            for i in (14, 15):
                pos[i] = [
                    pmm.tile([128, 512], f32, tag="mm", name=f"pot{i}a"),
                    pmm.tile([128, 256], f32, tag="mm", name=f"pot{i}b"),
                ]
                for t in (0, 1):
                    for half in (0, 1):
                        nc.tensor.matmul(
                            pos[i][half][:],
                            lhsT=UT[t][:, i * 128 : (i + 1) * 128],
                            rhs=wpt[:, t, [0, 512][half] : [512, 768][half]],
                            start=(t == 0),
                            stop=False,
                        )
            for i in range(12, 16):
                ob = pout.tile([128, D], bf16, tag="ob", name=f"obt{i}")
                for half, (n0, n1) in enumerate(((0, 512), (512, 768))):
                    nc.tensor.matmul(
                        pos[i][half][:],
                        lhsT=UT[2][:, i * 128 : (i + 1) * 128],
                        rhs=wpt[:, 2, n0:n1],
                        start=False,
                        stop=True,
                    )
                    nc.vector.tensor_copy(ob[:, n0:n1], pos[i][half][:])
                nc.sync.dma_start(out_d[i * 128 : (i + 1) * 128, :], ob[:])

        # ---------------- the program ----------------
        f0 = qkv_fillers(0)
        # f0 order: Q0a,Q0b,K0a,K0b, Q1a,Q1b,K1a,K1b, Q2a,Q2b,K2a,K2b, V0..V3
        # ncI 0 AND 1 must complete before the first strip (the strips read
        # both pair slots of the f4 tiles)
        for f in f0[0:8]:
            f()
        # rest of qkv(0): V tiles early (the first AVs need them) woven with
        # the ncI=2 drains (pack t=1/t=2 strips read the f2 tiles)
        rest0 = [
            f0[12], f0[13], f0[8],   # V0 | Q2a
            f0[14], f0[15], f0[9],   # V1 | Q2b
            f0[16], f0[17], f0[10],  # V2 | K2a
            f0[18], f0[19], f0[11],  # V3 | K2b
        ]
        attention_chunk(0, rest0 + qkv_fillers(1), target_pairs=9)
        for c in range(1, 4):
            fillers = qkv_fillers(c + 1) if c < 3 else []
            fillers += proj_fillers(c - 1)
            attention_chunk(c, fillers)
        proj_tail()

    nc.compile()
    return nc


def _get_nc():
    global _built_nc
    if _built_nc is None:
        _built_nc = _build()
    return _built_nc


def _make_in_maps(x, w_qkv, b_qkv, w_proj):
    import ml_dtypes

    bf16 = ml_dtypes.bfloat16
    in_maps = []
    xTb = [np.ascontiguousarray(x[b].T.astype(bf16)) for b in range(B)]
    for core in range(N_CORES):
        b, hh = core // 2, core % 2
        cs = slice(hh * 384, (hh + 1) * 384)
        # head-interleaved column order for the fp8 DoubleRow layout
        # (1/sqrt(64) is applied in the exp's scale, not here)
        perm = np.concatenate(
            [np.arange(h * 64, h * 64 + 32) for h in range(4)]
            + [np.arange(h * 64 + 32, h * 64 + 64) for h in range(4)]
            + [np.arange(h * 64, h * 64 + 32) for h in (4, 5)]
            + [np.arange(h * 64 + 32, h * 64 + 64) for h in (4, 5)]
        )
        wq = w_qkv[:, 0:768][:, cs][:, perm]
        wk = w_qkv[:, 768:1536][:, cs][:, perm]
        wv = w_qkv[:, 1536:2304][:, cs]
        w_in = np.ascontiguousarray(
            np.concatenate([wq, wk, wv], axis=1).astype(bf16)
        )
        bqv = np.concatenate(
            [
                b_qkv[0:768][cs][perm],
                b_qkv[768:1536][cs][perm],
                b_qkv[1536:2304][cs],
            ]
        ).astype(np.float32)
        wp = np.ascontiguousarray(w_proj[cs, :].astype(bf16))
        in_maps.append(
            {
                "xT_in": xTb[b],
                "w_in": w_in,
                "bqkv_in": bqv,
                "wp_in": wp,
            }
        )
    return in_maps


def _run(x, w_qkv, b_qkv, w_proj, b_proj, trace=False):
    from concourse.bass_utils import run_bass_kernel_spmd

    nc = _get_nc()
    in_maps = _make_in_maps(x, w_qkv, b_qkv, w_proj)
    res = run_bass_kernel_spmd(
        nc, in_maps, core_ids=list(range(N_CORES)), trace=trace
    )
    out = np.zeros((B, S, D), np.float32)
    for core in range(N_CORES):
        out[core // 2] += np.asarray(res.results[core]["out"], np.float32)
    out += np.asarray(b_proj, np.float32)[None, None, :]
    return out, res


def kernel(**inputs):
    x = np.asarray(inputs["x"], np.float32)
    w_qkv = np.asarray(inputs["w_qkv"], np.float32)
    b_qkv = np.asarray(inputs["b_qkv"], np.float32)
    w_proj = np.asarray(inputs["w_proj"], np.float32)
    b_proj = np.asarray(inputs["b_proj"], np.float32)
    out, _ = _run(x, w_qkv, b_qkv, w_proj, b_proj, trace=False)
    return out


# revision 47
# speedup vs baseline: 1.0923x; 1.0104x over previous
"""Causal self-attention (B=4, S=2048, D=768, H=12) on 8 trn2 NeuronCores.

Sharding: core c -> (batch b = c//2, head-half hh = c%2). Each core handles
one batch and 6 of the 12 heads: it computes qkv for its 384 q/k/v columns,
full causal attention for its 6 heads, and a partial output projection over
its 384 rows of w_proj. Host sums the two half partials per batch + b_proj.

Device pipeline (bf16 matmul operands / f32 PSUM accumulation):
  x arrives PRE-TRANSPOSED from the host (xT [768, 2048] bf16) so no PE
  transposes are needed.  QT/KT pack 2 heads per 128 partitions (q
  pre-scaled by 1/8); VV v-tiles carry a ones column per head at column 0
  so A@V also yields the softmax rowsum on PSUM partition 0, with the 64
  v-dims at partitions 64-127 (legal partition bases for the custom-DVE
  reciprocal and the extract multiply).

  The Tile scheduler builds static in-order per-engine streams from
  emission order (dependency replay has no timing model), so the emission
  WEAVES the work: attention chunk c's score-strip pairs (PE) + exp (ACT)
  + causal mask (gpsimd) + AV accumulation are interleaved with "filler"
  closures carrying qkv chunk c+1 (half-groups of accumulation matmuls)
  and the projection of chunk c-1 (two 1-bank PSUM pieces per s-tile).
  This keeps the PE stream stocked with ready work at chunk boundaries so
  the exp stream never stalls and the PE clock-ramp never resets.

  Per (c,t) normalization: one DVE reciprocal straight off the PSUM rowsum
  rows, one fused gpsimd partition_broadcast, and two DVE multiplies that
  extract+normalize U^T from PSUM in one pass.  Output: per s-tile
  projection into PSUM, DVE drain to bf16, DMA out (host sums in f32).
"""

import numpy as np

B, S, D, H, HD = 4, 2048, 768, 12, 64
HPC = 6  # heads per core
N_CORES = 8

_built_nc = None


def _build():
    import concourse.bass as bass
    import concourse.mybir as mybir
    from concourse import bacc
    import concourse.tile as tile
    from concourse.masks import make_upper_triangular
    from contextlib import ExitStack

    f32 = mybir.dt.float32
    bf16 = mybir.dt.bfloat16
    fp8 = mybir.dt.float8e4
    DR = mybir.MatmulPerfMode.DoubleRow
    FT = mybir.ActivationFunctionType
    MUL = mybir.AluOpType.mult

    nc = bacc.Bacc("TRN2", target_bir_lowering=False, debug=False)
    # x arrives pre-transposed + pre-cast to bf16 from the host
    xT_d = nc.dram_tensor("xT_in", [D, S], bf16, kind="ExternalInput").ap()
    w_d = nc.dram_tensor("w_in", [D, 1152], bf16, kind="ExternalInput").ap()
    bqkv_d = nc.dram_tensor("bqkv_in", [1152], f32, kind="ExternalInput").ap()
    wp_d = nc.dram_tensor("wp_in", [384, D], bf16, kind="ExternalInput").ap()
    out_d = nc.dram_tensor("out", [S, D], bf16, kind="ExternalOutput").ap()

    with tile.TileContext(nc) as tc, ExitStack() as ctx:
        # ---------------- constants + persistent tiles ----------------
        pconst = ctx.enter_context(tc.tile_pool(name="const", bufs=1))
        utri = pconst.tile([128, 128], bf16)  # 1.0 where p <= c else 0.0
        make_upper_triangular(nc, utri[:], val=1.0, diag=True)
        bq = pconst.tile([128, 6], f32)  # per-chunk bias vecs: cols 0-2 q, 3-5 k
        ones64 = pconst.tile([1, 64], f32)
        nc.vector.memset(ones64[:], 1.0)
        bv_row = pconst.tile([1, 384], f32)
        bvb = pconst.tile([128, 384], f32)  # bias_v broadcast to 128 partitions

        pqkv = ctx.enter_context(tc.tile_pool(name="qkvout", bufs=1))
        # q/k in fp8 for DoubleRow score matmuls (0.5 cyc/row, effective
        # K=64 via the 2 pair slots).  Host orders the w_qkv columns so
        # slice ncI=0 holds heads 0-3 dims 0-31 (pair slot 0), ncI=1 holds
        # their dims 32-63 (slot 1), and ncI=2 holds heads 4/5 both halves.
        # Per s-chunk: f4 [128,2,512] = heads 0-3, f2 [64,2,512] = heads 4/5.
        # operand partition bases must be 0/32/64, so: f4 serves heads
        # 0/1/2 at bases 0/32/64 (head 3's data is parked at base 96 and
        # copied out), f2 serves heads 4@0, 5@32, 3@64
        # bf16 q/k for chunk 0 (permuted dim order; slot 3 = parked copies
        # of the base-96 rows): the j=0 strips of chunk 0 run in bf16 so the
        # short-softmax rows 0-127 (which set the output's max magnitude)
        # don't carry fp8 score noise
        QTb = pqkv.tile([128, 5, 512], bf16, name="qtb")
        KTb = pqkv.tile([128, 5, 512], bf16, name="ktb")
        Qf4 = [pqkv.tile([128, 2, 512], fp8, name=f"qf4_{sc}") for sc in range(4)]
        Qf2 = [pqkv.tile([96, 2, 512], fp8, name=f"qf2_{sc}") for sc in range(4)]
        Kf4 = [pqkv.tile([128, 2, 512], fp8, name=f"kf4_{sc}") for sc in range(4)]
        Kf2 = [pqkv.tile([96, 2, 512], fp8, name=f"kf2_{sc}") for sc in range(4)]
        # v tiles: per head 128 columns [ones | 63 unused | 64 v-dims], see
        # module docstring
        VV = [pqkv.tile([128, HPC * 128], bf16, name=f"vv{i}") for i in range(16)]
        UT = [pqkv.tile([128, S], bf16, name=f"ut{t}") for t in range(3)]
        wpt = pqkv.tile([128, 3, D], bf16)
        pes = ctx.enter_context(tc.tile_pool(name="espool", bufs=10))
        pnrm = ctx.enter_context(tc.tile_pool(name="nrm", bufs=6))
        prr = ctx.enter_context(tc.tile_pool(name="rrp", bufs=4))
        pout = ctx.enter_context(tc.tile_pool(name="outp", bufs=6))

        # attention PSUM: strips (4 banks) + AV (2 banks) + qkv/proj mm ring
        pst2 = ctx.enter_context(tc.tile_pool(name="stps", space="PSUM", bufs=2))
        pav = ctx.enter_context(tc.tile_pool(name="avps", space="PSUM", bufs=1))
        pmm = ctx.enter_context(tc.tile_pool(name="mmps", space="PSUM", bufs=2))

        p1 = ctx.enter_context(tc.tile_pool(name="ph1", bufs=1))
        wt = p1.tile([128, 6, 1152], bf16)
        xt = [
            [p1.tile([128, 3, 512], bf16, name=f"xt{sc}_{h}") for h in range(2)]
            for sc in range(4)
        ]

        def xts(sc, c):  # c-th 128-row input-dim slice of chunk sc
            return xt[sc][c // 3][:, c % 3, :]
        # The cost model serializes all transfers through one DMA lane, so
        # issue order ~= arrival order.  Gate-first: xt0 (SP queue) || wq, wk
        # (ACT queue), then everything else in need order.
        for h in range(2):
            nc.sync.dma_start(
                xt[0][h][:],
                xT_d[384 * h : 384 * (h + 1), 0:512].rearrange(
                    "(c p) s -> p c s", p=128
                ),
            )
        # q/k weights: bq first (it gates the QT/KT drains), then the
        # ncI=0 slices (gate the first strips), then the wide remainder
        nc.scalar.dma_start(bq[:], bqkv_d[0:768].rearrange("(c p) -> p c", p=128))
        for lo, hi in ((0, 128), (384, 512), (128, 384), (512, 768)):
            nc.scalar.dma_start(
                wt[:, :, lo:hi],
                w_d[:, lo:hi].rearrange("(c p) n -> p c n", p=128),
            )
        nc.scalar.dma_start(
            bv_row[:], bqkv_d[768:1152].rearrange("(o n) -> o n", o=1)
        )
        nc.gpsimd.partition_broadcast(bvb[:], bv_row[:])
        nc.scalar.dma_start(  # wv
            wt[:, :, 768:1152], w_d[:, 768:1152].rearrange("(c p) n -> p c n", p=128)
        )
        for sc in range(1, 4):
            for h in range(2):
                nc.scalar.dma_start(
                    xt[sc][h][:],
                    xT_d[384 * h : 384 * (h + 1), sc * 512 : (sc + 1) * 512]
                    .rearrange("(c p) s -> p c s", p=128),
                )
        nc.scalar.dma_start(wpt[:], wp_d.rearrange("(c p) n -> p c n", p=128))

        # ---------------- emission building blocks ----------------

        def qkv_fillers(sc):
            """qkv chunk sc as a list of ~0.5-0.7us PE closures (half
            accumulation groups). QT/KT slices first (they gate the next
            chunk's exp stream), V tiles after."""
            out = []
            state = {}

            def qk_half(ncI, which, dst, second):
                def run():
                    base = which * 384
                    if not second:
                        state[(ncI, which)] = pmm.tile([128, 512], f32, tag="mm", name=f"qk{sc}_{ncI}_{which}")
                    ps = state[(ncI, which)]
                    for c in range(3, 6) if second else range(3):
                        nc.tensor.matmul(
                            ps[:],
                            lhsT=wt[:, c, base + ncI * 128 : base + (ncI + 1) * 128],
                            rhs=xts(sc, c),
                            start=(c == 0),
                            stop=(c == 5),
                        )
                    if second:
                        cidx = which * 3 + ncI
                        f4, f2, fb = dst
                        if sc == 0:
                            nc.vector.tensor_scalar_add(
                                fb[:, ncI, :], ps[:], bq[:, cidx : cidx + 1]
                            )
                            # park base-96 rows so every head's two
                            # 32-partition pieces share a base (groups
                            # must keep a constant tile row position):
                            # h3 -> (3,64)+(4,64); h4 -> (2,0)+(3,0);
                            # h5 -> (2,32)+(3,32)
                            if ncI == 0:
                                nc.vector.tensor_copy(
                                    fb[64:96, 3, :], fb[96:128, 0, :]
                                )
                            elif ncI == 1:
                                nc.vector.tensor_copy(
                                    fb[64:96, 4, :], fb[96:128, 1, :]
                                )
                            elif ncI == 2:
                                nc.vector.tensor_copy(
                                    fb[0:64, 3, :], fb[64:128, 2, :]
                                )
                        if ncI < 2:
                            # drain on DVE: keeps the ACT stream pure-exp so
                            # drains never throttle the mm ring behind exps
                            nc.vector.tensor_scalar_add(
                                f4[sc][:, ncI, :],
                                ps[:],
                                bq[:, cidx : cidx + 1],
                            )
                            if ncI == 1:
                                # head 3 parked at f4 base 96 -> f2 base 64
                                nc.vector.tensor_copy(
                                    f2[sc][64:96, :, :], f4[sc][96:128, :, :]
                                )
                        else:
                            # heads 4/5 both halves: two partition-shifted
                            # half drains (DVE handles base shifts)
                            for off, slot in ((0, 0), (64, 1)):
                                nc.vector.tensor_scalar_add(
                                    f2[sc][0:64, slot, :],
                                    ps[off : off + 64, :],
                                    bq[off : off + 64, cidx : cidx + 1],
                                )
                return run

            def v_half(i, second):
                def run():
                    if not second:
                        state[("v", i)] = pmm.tile([128, 384], f32, tag="mm", name=f"psv{i}")
                    psv = state[("v", i)]
                    for c in range(3, 6) if second else range(3):
                        nc.tensor.matmul(
                            psv[:],
                            lhsT=xts(sc, c)[:, (i % 4) * 128 : (i % 4 + 1) * 128],
                            rhs=wt[:, c, 768:1152],
                            start=(c == 0),
                            stop=(c == 5),
                        )
                    if second:
                        vt = VV[i][:].rearrange("p (h m) -> p h m", m=128)
                        nc.vector.tensor_tensor(
                            vt[:, :, 64:128],
                            psv[:].rearrange("p (h m) -> p h m", m=64),
                            bvb[:].rearrange("p (h m) -> p h m", m=64),
                            mybir.AluOpType.add,
                        )
                        nc.vector.memset(vt[:, :, 0:1], 1.0)
                return run

            for ncI in range(3):
                for which, dst in ((0, (Qf4, Qf2, QTb)), (1, (Kf4, Kf2, KTb))):
                    out.append(qk_half(ncI, which, dst, False))
                    out.append(qk_half(ncI, which, dst, True))
            for i in range(sc * 4, sc * 4 + 4):
                out.append(v_half(i, False))
                out.append(v_half(i, True))
            return out

        def proj_fillers(c):
            """Projection of chunk c as 1-bank mm-ring pieces (2 per
            s-tile).  Accumulation leads with t=2 (the last-normalized
            pack) so a piece can't start and then block the PE stream."""
            out = []
            state = {}

            def piece(i, half):
                def run():
                    n0, n1 = (0, 512) if half == 0 else (512, 768)
                    po = pmm.tile([128, n1 - n0], f32, tag="mm", name=f"po{i}_{half}")
                    for t in (2, 0, 1):
                        nc.tensor.matmul(
                            po[:],
                            lhsT=UT[t][:, i * 128 : (i + 1) * 128],
                            rhs=wpt[:, t, n0:n1],
                            start=(t == 2),
                            stop=(t == 1),
                        )
                    if half == 0:
                        state[i] = pout.tile([128, D], bf16, tag="ob", name=f"ob{i}")
                    ob = state[i]
                    nc.vector.tensor_copy(ob[:, n0:n1], po[:])
                    if half == 1:
                        nc.sync.dma_start(
                            out_d[i * 128 : (i + 1) * 128, :], ob[:]
                        )
                return run

            for i in range(4 * c, 4 * c + 4):
                out.append(piece(i, 0))
                out.append(piece(i, 1))
            return out

        def attention_chunk(c, fillers, target_pairs=None):
            """Emit chunk c's attention, weaving filler closures between
            strip-pairs (never right before a pack boundary)."""
            g0 = c * 512
            npairs = 3 * (4 * c + 4)
            target = target_pairs if target_pairs else npairs - 2
            emitted = [0]

            def weave(allow=True):
                k = emitted[0] = emitted[0] + 1
                if not allow:
                    return
                total = len(fillers)
                want = min(total, (k * total) // target)
                while weave.done < want:
                    fillers[weave.done]()
                    weave.done += 1
            weave.done = 0

            for t in range(3):
                av = pav.tile([128, 2, 512], f32, tag="av")
                pend_av = []
                for j in range(4 * c + 4):
                    n0 = max(0, j * 128 - g0)
                    W = 512 - n0
                    jc, jr = j // 4, (j % 4) * 128
                    st = pst2.tile([128, 1024], f32, tag="st")
                    if c == 0 and j == 0:
                        # bf16 strips for the shortest-softmax rows; each
                        # head's 64 dims live as two 32-partition pieces
                        BFP = (
                            ((0, 0), (1, 0)),    # h0
                            ((0, 32), (1, 32)),  # h1
                            ((0, 64), (1, 64)),  # h2
                            ((3, 64), (4, 64)),  # h3
                            ((2, 0), (3, 0)),    # h4
                            ((2, 32), (3, 32)),  # h5
                        )
                        for hh in (0, 1):
                            for pi, (sl, rb) in enumerate(BFP[2 * t + hh]):
                                nc.tensor.matmul(
                                    st[:, 512 * hh : 512 * hh + 512],
                                    lhsT=KTb[rb : rb + 32, sl, 0:128],
                                    rhs=QTb[rb : rb + 32, sl, :],
                                    start=(pi == 0),
                                    stop=(pi == 1),
                                )
                    else:
                        # head -> (tile, base): t0: f4@0,f4@32; t1: f4@64,
                        # f2@64; t2: f2@0,f2@32
                        hmap = (
                            ((Qf4, Kf4, 0), (Qf4, Kf4, 32)),
                            ((Qf4, Kf4, 64), (Qf2, Kf2, 64)),
                            ((Qf2, Kf2, 0), (Qf2, Kf2, 32)),
                        )[t]
                        for hh, (qtl, ktl, hb) in enumerate(hmap):
                            nc.tensor.matmul(
                                st[:, 512 * hh : 512 * hh + W],
                                lhsT=ktl[jc][hb : hb + 32, :, jr : jr + 128],
                                rhs=qtl[c][hb : hb + 32, :, n0:512],
                                perf_mode=DR,
                                start=True,
                                stop=True,
                            )
                    es = pes.tile([128, 1024], bf16, tag="es")
                    # 1/sqrt(HD) folded into the exp's free affine scale
                    nc.scalar.activation(
                        es[:].rearrange("p (h w) -> p h w", h=2)[:, :, 0:W],
                        st[:].rearrange("p (h w) -> p h w", h=2)[:, :, 0:W],
                        FT.Exp,
                        scale=0.125,
                    )
                    if j * 128 >= g0:  # diagonal block at start of valid region
                        nc.gpsimd.tensor_tensor(
                            es[:, 0:128], es[:, 0:128], utri[:], MUL
                        )
                        nc.gpsimd.tensor_tensor(
                            es[:, 512:640], es[:, 512:640], utri[:], MUL
                        )
                    def av_mm(j, n0, W, es):
                        def run():
                            last = j == 4 * c + 3
                            nc.tensor.matmul(
                                av[:, 0, n0:512],
                                lhsT=VV[j][:, (2 * t) * 128 : (2 * t + 1) * 128],
                                rhs=es[:, 0:W],
                                start=(j == 0),
                                stop=last,
                            )
                            nc.tensor.matmul(
                                av[:, 1, n0:512],
                                lhsT=VV[j][:, (2 * t + 1) * 128 : (2 * t + 2) * 128],
                                rhs=es[:, 512 : 512 + W],
                                start=(j == 0),
                                stop=last,
                            )
                        return run

                    # delay AV by one j so the in-order PE stream never
                    # commits to an av-slot wait before the next strips
                    pend_av.append(av_mm(j, n0, W, es))
                    if len(pend_av) > 6:
                        pend_av.pop(0)()
                    # c=0: fillers carry this chunk's own V tiles, which the
                    # next AV emission needs -- never defer them
                    weave(allow=(c == 0 or j < 4 * c + 2))
                while pend_av:
                    pend_av.pop(0)()
                # normalize + extract U^T: per-head reciprocal straight off
                # PSUM partition 0, replicate across partitions, then one
                # multiply per head from PSUM.  Split per head to halve the
                # chain latency (av-slot release gates the next pack's AV).
                # The last pack replicates via an f32r PE matmul instead of
                # the gpsimd broadcast -- the PE is idle in the tail and the
                # matmul is 7x faster than the Pool broadcast.
                rsr = prr.tile([1, 2, 512], f32, tag="rr")
                for hh in (0, 1):
                    nc.vector.reciprocal_approx_fast(
                        rsr[0:1, hh, :], av[0:1, hh, :]
                    )
                    rec = pnrm.tile([64, 512], f32, tag="rec", name=f"rc{hh}")
                    nc.gpsimd.partition_broadcast(rec[:], rsr[0:1, hh, :])
                    nc.vector.tensor_tensor(
                        UT[t][64 * hh : 64 * hh + 64, g0 : g0 + 512],
                        av[64:128, hh, :],
                        rec[:],
                        MUL,
                    )
            # anything not woven (short chunks): emit now
            while weave.done < len(fillers):
                fillers[weave.done]()
                weave.done += 1

        def proj_tail():
            # final chunk's projection, 4-wide (the two strip slots + the mm
            # ring are idle; the AV slot is NOT used -- allocating it would
            # insert a ring-wait on the last norm into the PE stream).  All
            # t=0/t=1 accumulation matmuls run first: they only need the
            # already-normalized UT[0]/UT[1] and keep the PE busy (and the
            # clock-ramp warm) while the last pack's norm chain drains; the
            # 8 t=2 matmuls + drains follow.
            pos = {}
            for i in (12, 13):
                po = pst2.tile([128, 1024], f32, tag="st", name=f"pot{i}")
                pos[i] = [po[:, 0:512], po[:, 512:768]]
                for t in (0, 1):
                    for half, (n0, n1) in enumerate(((0, 512), (512, 768))):
                        nc.tensor.matmul(
                            pos[i][half][:],
                            lhsT=UT[t][:, i * 128 : (i + 1) * 128],
                            rhs=wpt[:, t, n0:n1],
                            start=(t == 0),
                            stop=False,
                        )
            for i in (14, 15):
                pos[i] = [
                    pmm.tile([128, 512], f32, tag="mm", name=f"pot{i}a"),
                    pmm.tile([128, 256], f32, tag="mm", name=f"pot{i}b"),
                ]
                for t in (0, 1):
                    for half in (0, 1):
                        nc.tensor.matmul(
                            pos[i][half][:],
                            lhsT=UT[t][:, i * 128 : (i + 1) * 128],
                            rhs=wpt[:, t, [0, 512][half] : [512, 768][half]],
                            start=(t == 0),
                            stop=False,
                        )
            for i in range(12, 16):
                ob = pout.tile([128, D], bf16, tag="ob", name=f"obt{i}")
                for half, (n0, n1) in enumerate(((0, 512), (512, 768))):
                    nc.tensor.matmul(
                        pos[i][half][:],
                        lhsT=UT[2][:, i * 128 : (i + 1) * 128],
                        rhs=wpt[:, 2, n0:n1],
                        start=False,
                        stop=True,
                    )
                    nc.vector.tensor_copy(ob[:, n0:n1], pos[i][half][:])
                nc.sync.dma_start(out_d[i * 128 : (i + 1) * 128, :], ob[:])

        # ---------------- the program ----------------
        f0 = qkv_fillers(0)
        # f0 order: Q0a,Q0b,K0a,K0b, Q1a,Q1b,K1a,K1b, Q2a,Q2b,K2a,K2b, V0..V3
        # ncI 0 AND 1 must complete before the first strip (the strips read
        # both pair slots of the f4 tiles)
        for f in f0[0:8]:
            f()
        # rest of qkv(0): V tiles early (the first AVs need them) woven with
        # the ncI=2 drains (pack t=1/t=2 strips read the f2 tiles)
        rest0 = [
            f0[12], f0[13], f0[8],   # V0 | Q2a
            f0[14], f0[15], f0[9],   # V1 | Q2b
            f0[16], f0[17], f0[10],  # V2 | K2a
            f0[18], f0[19], f0[11],  # V3 | K2b
        ]
        attention_chunk(0, rest0 + qkv_fillers(1), target_pairs=9)
        for c in range(1, 4):
            fillers = qkv_fillers(c + 1) if c < 3 else []
            fillers += proj_fillers(c - 1)
            attention_chunk(c, fillers)
        proj_tail()

    nc.compile()
    return nc


def _get_nc():
    global _built_nc
    if _built_nc is None:
        _built_nc = _build()
    return _built_nc


def _make_in_maps(x, w_qkv, b_qkv, w_proj):
    import ml_dtypes

    bf16 = ml_dtypes.bfloat16
    in_maps = []
    xTb = [np.ascontiguousarray(x[b].T.astype(bf16)) for b in range(B)]
    for core in range(N_CORES):
        b, hh = core // 2, core % 2
        cs = slice(hh * 384, (hh + 1) * 384)
        # head-interleaved column order for the fp8 DoubleRow layout
        # (1/sqrt(64) is applied in the exp's scale, not here)
        perm = np.concatenate(
            [np.arange(h * 64, h * 64 + 32) for h in range(4)]
            + [np.arange(h * 64 + 32, h * 64 + 64) for h in range(4)]
            + [np.arange(h * 64, h * 64 + 32) for h in (4, 5)]
            + [np.arange(h * 64 + 32, h * 64 + 64) for h in (4, 5)]
        )
        wq = w_qkv[:, 0:768][:, cs][:, perm]
        wk = w_qkv[:, 768:1536][:, cs][:, perm]
        wv = w_qkv[:, 1536:2304][:, cs]
        w_in = np.ascontiguousarray(
            np.concatenate([wq, wk, wv], axis=1).astype(bf16)
        )
        bqv = np.concatenate(
            [
                b_qkv[0:768][cs][perm],
                b_qkv[768:1536][cs][perm],
                b_qkv[1536:2304][cs],
            ]
        ).astype(np.float32)
        wp = np.ascontiguousarray(w_proj[cs, :].astype(bf16))
        in_maps.append(
            {
                "xT_in": xTb[b],
                "w_in": w_in,
                "bqkv_in": bqv,
                "wp_in": wp,
            }
        )
    return in_maps


def _run(x, w_qkv, b_qkv, w_proj, b_proj, trace=False):
    from concourse.bass_utils import run_bass_kernel_spmd

    nc = _get_nc()
    in_maps = _make_in_maps(x, w_qkv, b_qkv, w_proj)
    res = run_bass_kernel_spmd(
        nc, in_maps, core_ids=list(range(N_CORES)), trace=trace
    )
    out = np.zeros((B, S, D), np.float32)
    for core in range(N_CORES):
        out[core // 2] += np.asarray(res.results[core]["out"], np.float32)
    out += np.asarray(b_proj, np.float32)[None, None, :]
    return out, res


def kernel(**inputs):
    x = np.asarray(inputs["x"], np.float32)
    w_qkv = np.asarray(inputs["w_qkv"], np.float32)
    b_qkv = np.asarray(inputs["b_qkv"], np.float32)
    w_proj = np.asarray(inputs["w_proj"], np.float32)
    b_proj = np.asarray(inputs["b_proj"], np.float32)
    out, _ = _run(x, w_qkv, b_qkv, w_proj, b_proj, trace=False)
    return out


# revision 48
# speedup vs baseline: 1.0952x; 1.0026x over previous
"""Causal self-attention (B=4, S=2048, D=768, H=12) on 8 trn2 NeuronCores.

Sharding: core c -> (batch b = c//2, head-half hh = c%2). Each core handles
one batch and 6 of the 12 heads: it computes qkv for its 384 q/k/v columns,
full causal attention for its 6 heads, and a partial output projection over
its 384 rows of w_proj. Host sums the two half partials per batch + b_proj.

Device pipeline (bf16 matmul operands / f32 PSUM accumulation):
  x arrives PRE-TRANSPOSED from the host (xT [768, 2048] bf16) so no PE
  transposes are needed.  QT/KT pack 2 heads per 128 partitions (q
  pre-scaled by 1/8); VV v-tiles carry a ones column per head at column 0
  so A@V also yields the softmax rowsum on PSUM partition 0, with the 64
  v-dims at partitions 64-127 (legal partition bases for the custom-DVE
  reciprocal and the extract multiply).

  The Tile scheduler builds static in-order per-engine streams from
  emission order (dependency replay has no timing model), so the emission
  WEAVES the work: attention chunk c's score-strip pairs (PE) + exp (ACT)
  + causal mask (gpsimd) + AV accumulation are interleaved with "filler"
  closures carrying qkv chunk c+1 (half-groups of accumulation matmuls)
  and the projection of chunk c-1 (two 1-bank PSUM pieces per s-tile).
  This keeps the PE stream stocked with ready work at chunk boundaries so
  the exp stream never stalls and the PE clock-ramp never resets.

  Per (c,t) normalization: one DVE reciprocal straight off the PSUM rowsum
  rows, one fused gpsimd partition_broadcast, and two DVE multiplies that
  extract+normalize U^T from PSUM in one pass.  Output: per s-tile
  projection into PSUM, DVE drain to bf16, DMA out (host sums in f32).
"""

import numpy as np

B, S, D, H, HD = 4, 2048, 768, 12, 64
HPC = 6  # heads per core
N_CORES = 8

_built_nc = None


def _build():
    import concourse.bass as bass
    import concourse.mybir as mybir
    from concourse import bacc
    import concourse.tile as tile
    from concourse.masks import make_upper_triangular
    from contextlib import ExitStack

    f32 = mybir.dt.float32
    bf16 = mybir.dt.bfloat16
    fp8 = mybir.dt.float8e4
    DR = mybir.MatmulPerfMode.DoubleRow
    FT = mybir.ActivationFunctionType
    MUL = mybir.AluOpType.mult

    nc = bacc.Bacc("TRN2", target_bir_lowering=False, debug=False)
    # x arrives pre-transposed + pre-cast to bf16 from the host
    xT_d = nc.dram_tensor("xT_in", [D, S], bf16, kind="ExternalInput").ap()
    w_d = nc.dram_tensor("w_in", [D, 1152], bf16, kind="ExternalInput").ap()
    bqkv_d = nc.dram_tensor("bqkv_in", [1152], f32, kind="ExternalInput").ap()
    wp_d = nc.dram_tensor("wp_in", [384, D], bf16, kind="ExternalInput").ap()
    out_d = nc.dram_tensor("out", [S, D], bf16, kind="ExternalOutput").ap()

    with tile.TileContext(nc) as tc, ExitStack() as ctx:
        # ---------------- constants + persistent tiles ----------------
        pconst = ctx.enter_context(tc.tile_pool(name="const", bufs=1))
        utri = pconst.tile([128, 128], bf16)  # 1.0 where p <= c else 0.0
        make_upper_triangular(nc, utri[:], val=1.0, diag=True)
        bq = pconst.tile([128, 6], f32)  # per-chunk bias vecs: cols 0-2 q, 3-5 k
        ones64 = pconst.tile([1, 64], f32)
        nc.vector.memset(ones64[:], 1.0)
        bv_row = pconst.tile([1, 384], f32)
        bvb = pconst.tile([128, 384], f32)  # bias_v broadcast to 128 partitions

        pqkv = ctx.enter_context(tc.tile_pool(name="qkvout", bufs=1))
        # q/k in fp8 for DoubleRow score matmuls (0.5 cyc/row, effective
        # K=64 via the 2 pair slots).  Host orders the w_qkv columns so
        # slice ncI=0 holds heads 0-3 dims 0-31 (pair slot 0), ncI=1 holds
        # their dims 32-63 (slot 1), and ncI=2 holds heads 4/5 both halves.
        # Per s-chunk: f4 [128,2,512] = heads 0-3, f2 [64,2,512] = heads 4/5.
        # operand partition bases must be 0/32/64, so: f4 serves heads
        # 0/1/2 at bases 0/32/64 (head 3's data is parked at base 96 and
        # copied out), f2 serves heads 4@0, 5@32, 3@64
        # bf16 q/k for chunk 0 (permuted dim order; slot 3 = parked copies
        # of the base-96 rows): the j=0 strips of chunk 0 run in bf16 so the
        # short-softmax rows 0-127 (which set the output's max magnitude)
        # don't carry fp8 score noise
        QTb = pqkv.tile([128, 5, 512], bf16, name="qtb")
        KTb = pqkv.tile([128, 5, 512], bf16, name="ktb")
        Qf4 = [pqkv.tile([128, 2, 512], fp8, name=f"qf4_{sc}") for sc in range(4)]
        Qf2 = [pqkv.tile([96, 2, 512], fp8, name=f"qf2_{sc}") for sc in range(4)]
        Kf4 = [pqkv.tile([128, 2, 512], fp8, name=f"kf4_{sc}") for sc in range(4)]
        Kf2 = [pqkv.tile([96, 2, 512], fp8, name=f"kf2_{sc}") for sc in range(4)]
        # v tiles: per head 128 columns [ones | 63 unused | 64 v-dims], see
        # module docstring
        VV = [pqkv.tile([128, HPC * 128], bf16, name=f"vv{i}") for i in range(16)]
        UT = [pqkv.tile([128, S], bf16, name=f"ut{t}") for t in range(3)]
        wpt = pqkv.tile([128, 3, D], bf16)
        pes = ctx.enter_context(tc.tile_pool(name="espool", bufs=14))
        pnrm = ctx.enter_context(tc.tile_pool(name="nrm", bufs=6))
        prr = ctx.enter_context(tc.tile_pool(name="rrp", bufs=4))
        pout = ctx.enter_context(tc.tile_pool(name="outp", bufs=6))

        # attention PSUM: strips (4 banks) + AV (2 banks) + qkv/proj mm ring
        pst2 = ctx.enter_context(tc.tile_pool(name="stps", space="PSUM", bufs=2))
        pav = ctx.enter_context(tc.tile_pool(name="avps", space="PSUM", bufs=1))
        pmm = ctx.enter_context(tc.tile_pool(name="mmps", space="PSUM", bufs=2))

        p1 = ctx.enter_context(tc.tile_pool(name="ph1", bufs=1))
        wt = p1.tile([128, 6, 1152], bf16)
        xt = [
            [p1.tile([128, 3, 512], bf16, name=f"xt{sc}_{h}") for h in range(2)]
            for sc in range(4)
        ]

        def xts(sc, c):  # c-th 128-row input-dim slice of chunk sc
            return xt[sc][c // 3][:, c % 3, :]
        # The cost model serializes all transfers through one DMA lane, so
        # issue order ~= arrival order.  Gate-first: xt0 (SP queue) || wq, wk
        # (ACT queue), then everything else in need order.
        for h in range(2):
            nc.sync.dma_start(
                xt[0][h][:],
                xT_d[384 * h : 384 * (h + 1), 0:512].rearrange(
                    "(c p) s -> p c s", p=128
                ),
            )
        # q/k weights: bq first (it gates the QT/KT drains), then the
        # ncI=0 slices (gate the first strips), then the wide remainder
        nc.scalar.dma_start(bq[:], bqkv_d[0:768].rearrange("(c p) -> p c", p=128))
        for lo, hi in ((0, 128), (384, 512), (128, 384), (512, 768)):
            nc.scalar.dma_start(
                wt[:, :, lo:hi],
                w_d[:, lo:hi].rearrange("(c p) n -> p c n", p=128),
            )
        nc.scalar.dma_start(
            bv_row[:], bqkv_d[768:1152].rearrange("(o n) -> o n", o=1)
        )
        nc.gpsimd.partition_broadcast(bvb[:], bv_row[:])
        nc.scalar.dma_start(  # wv
            wt[:, :, 768:1152], w_d[:, 768:1152].rearrange("(c p) n -> p c n", p=128)
        )
        for sc in range(1, 4):
            for h in range(2):
                nc.scalar.dma_start(
                    xt[sc][h][:],
                    xT_d[384 * h : 384 * (h + 1), sc * 512 : (sc + 1) * 512]
                    .rearrange("(c p) s -> p c s", p=128),
                )
        nc.scalar.dma_start(wpt[:], wp_d.rearrange("(c p) n -> p c n", p=128))

        # ---------------- emission building blocks ----------------

        def qkv_fillers(sc):
            """qkv chunk sc as a list of ~0.5-0.7us PE closures (half
            accumulation groups). QT/KT slices first (they gate the next
            chunk's exp stream), V tiles after."""
            out = []
            state = {}

            def qk_half(ncI, which, dst, second):
                def run():
                    base = which * 384
                    if not second:
                        state[(ncI, which)] = pmm.tile([128, 512], f32, tag="mm", name=f"qk{sc}_{ncI}_{which}")
                    ps = state[(ncI, which)]
                    for c in range(3, 6) if second else range(3):
                        nc.tensor.matmul(
                            ps[:],
                            lhsT=wt[:, c, base + ncI * 128 : base + (ncI + 1) * 128],
                            rhs=xts(sc, c),
                            start=(c == 0),
                            stop=(c == 5),
                        )
                    if second:
                        cidx = which * 3 + ncI
                        f4, f2, fb = dst
                        if sc == 0:
                            nc.vector.tensor_scalar_add(
                                fb[:, ncI, :], ps[:], bq[:, cidx : cidx + 1]
                            )
                            # park base-96 rows so every head's two
                            # 32-partition pieces share a base (groups
                            # must keep a constant tile row position):
                            # h3 -> (3,64)+(4,64); h4 -> (2,0)+(3,0);
                            # h5 -> (2,32)+(3,32)
                            if ncI == 0:
                                nc.vector.tensor_copy(
                                    fb[64:96, 3, :], fb[96:128, 0, :]
                                )
                            elif ncI == 1:
                                nc.vector.tensor_copy(
                                    fb[64:96, 4, :], fb[96:128, 1, :]
                                )
                            elif ncI == 2:
                                nc.vector.tensor_copy(
                                    fb[0:64, 3, :], fb[64:128, 2, :]
                                )
                        if ncI < 2:
                            # drain on DVE: keeps the ACT stream pure-exp so
                            # drains never throttle the mm ring behind exps
                            nc.vector.tensor_scalar_add(
                                f4[sc][:, ncI, :],
                                ps[:],
                                bq[:, cidx : cidx + 1],
                            )
                            if ncI == 1:
                                # head 3 parked at f4 base 96 -> f2 base 64
                                nc.vector.tensor_copy(
                                    f2[sc][64:96, :, :], f4[sc][96:128, :, :]
                                )
                        else:
                            # heads 4/5 both halves: two partition-shifted
                            # half drains (DVE handles base shifts)
                            for off, slot in ((0, 0), (64, 1)):
                                nc.vector.tensor_scalar_add(
                                    f2[sc][0:64, slot, :],
                                    ps[off : off + 64, :],
                                    bq[off : off + 64, cidx : cidx + 1],
                                )
                return run

            def v_half(i, second):
                def run():
                    if not second:
                        state[("v", i)] = pmm.tile([128, 384], f32, tag="mm", name=f"psv{i}")
                    psv = state[("v", i)]
                    for c in range(3, 6) if second else range(3):
                        nc.tensor.matmul(
                            psv[:],
                            lhsT=xts(sc, c)[:, (i % 4) * 128 : (i % 4 + 1) * 128],
                            rhs=wt[:, c, 768:1152],
                            start=(c == 0),
                            stop=(c == 5),
                        )
                    if second:
                        vt = VV[i][:].rearrange("p (h m) -> p h m", m=128)
                        nc.vector.tensor_tensor(
                            vt[:, :, 64:128],
                            psv[:].rearrange("p (h m) -> p h m", m=64),
                            bvb[:].rearrange("p (h m) -> p h m", m=64),
                            mybir.AluOpType.add,
                        )
                        nc.vector.memset(vt[:, :, 0:1], 1.0)
                return run

            for ncI in range(3):
                for which, dst in ((0, (Qf4, Qf2, QTb)), (1, (Kf4, Kf2, KTb))):
                    out.append(qk_half(ncI, which, dst, False))
                    out.append(qk_half(ncI, which, dst, True))
            for i in range(sc * 4, sc * 4 + 4):
                out.append(v_half(i, False))
                out.append(v_half(i, True))
            return out

        def proj_fillers(c):
            """Projection of chunk c as 1-bank mm-ring pieces (2 per
            s-tile).  Accumulation leads with t=2 (the last-normalized
            pack) so a piece can't start and then block the PE stream."""
            out = []
            state = {}

            def piece(i, half):
                def run():
                    n0, n1 = (0, 512) if half == 0 else (512, 768)
                    po = pmm.tile([128, n1 - n0], f32, tag="mm", name=f"po{i}_{half}")
                    for t in (2, 0, 1):
                        nc.tensor.matmul(
                            po[:],
                            lhsT=UT[t][:, i * 128 : (i + 1) * 128],
                            rhs=wpt[:, t, n0:n1],
                            start=(t == 2),
                            stop=(t == 1),
                        )
                    if half == 0:
                        state[i] = pout.tile([128, D], bf16, tag="ob", name=f"ob{i}")
                    ob = state[i]
                    nc.vector.tensor_copy(ob[:, n0:n1], po[:])
                    if half == 1:
                        nc.sync.dma_start(
                            out_d[i * 128 : (i + 1) * 128, :], ob[:]
                        )
                return run

            for i in range(4 * c, 4 * c + 4):
                out.append(piece(i, 0))
                out.append(piece(i, 1))
            return out

        def attention_chunk(c, fillers, target_pairs=None):
            """Emit chunk c's attention, weaving filler closures between
            strip-pairs (never right before a pack boundary)."""
            g0 = c * 512
            npairs = 3 * (4 * c + 4)
            target = target_pairs if target_pairs else npairs - 2
            emitted = [0]

            def weave(allow=True):
                k = emitted[0] = emitted[0] + 1
                if not allow:
                    return
                total = len(fillers)
                want = min(total, (k * total) // target)
                while weave.done < want:
                    fillers[weave.done]()
                    weave.done += 1
            weave.done = 0

            for t in range(3):
                av = pav.tile([128, 2, 512], f32, tag="av")
                pend_av = []
                for j in range(4 * c + 4):
                    n0 = max(0, j * 128 - g0)
                    W = 512 - n0
                    jc, jr = j // 4, (j % 4) * 128
                    st = pst2.tile([128, 1024], f32, tag="st")
                    if c == 0 and j == 0:
                        # bf16 strips for the shortest-softmax rows; each
                        # head's 64 dims live as two 32-partition pieces
                        BFP = (
                            ((0, 0), (1, 0)),    # h0
                            ((0, 32), (1, 32)),  # h1
                            ((0, 64), (1, 64)),  # h2
                            ((3, 64), (4, 64)),  # h3
                            ((2, 0), (3, 0)),    # h4
                            ((2, 32), (3, 32)),  # h5
                        )
                        for hh in (0, 1):
                            for pi, (sl, rb) in enumerate(BFP[2 * t + hh]):
                                nc.tensor.matmul(
                                    st[:, 512 * hh : 512 * hh + 512],
                                    lhsT=KTb[rb : rb + 32, sl, 0:128],
                                    rhs=QTb[rb : rb + 32, sl, :],
                                    start=(pi == 0),
                                    stop=(pi == 1),
                                )
                    else:
                        # head -> (tile, base): t0: f4@0,f4@32; t1: f4@64,
                        # f2@64; t2: f2@0,f2@32
                        hmap = (
                            ((Qf4, Kf4, 0), (Qf4, Kf4, 32)),
                            ((Qf4, Kf4, 64), (Qf2, Kf2, 64)),
                            ((Qf2, Kf2, 0), (Qf2, Kf2, 32)),
                        )[t]
                        for hh, (qtl, ktl, hb) in enumerate(hmap):
                            nc.tensor.matmul(
                                st[:, 512 * hh : 512 * hh + W],
                                lhsT=ktl[jc][hb : hb + 32, :, jr : jr + 128],
                                rhs=qtl[c][hb : hb + 32, :, n0:512],
                                perf_mode=DR,
                                start=True,
                                stop=True,
                            )
                    es = pes.tile([128, 1024], bf16, tag="es")
                    # 1/sqrt(HD) folded into the exp's free affine scale
                    nc.scalar.activation(
                        es[:].rearrange("p (h w) -> p h w", h=2)[:, :, 0:W],
                        st[:].rearrange("p (h w) -> p h w", h=2)[:, :, 0:W],
                        FT.Exp,
                        scale=0.125,
                    )
                    if j * 128 >= g0:  # diagonal block at start of valid region
                        nc.gpsimd.tensor_tensor(
                            es[:, 0:128], es[:, 0:128], utri[:], MUL
                        )
                        nc.gpsimd.tensor_tensor(
                            es[:, 512:640], es[:, 512:640], utri[:], MUL
                        )
                    def av_mm(j, n0, W, es):
                        def run():
                            last = j == 4 * c + 3
                            nc.tensor.matmul(
                                av[:, 0, n0:512],
                                lhsT=VV[j][:, (2 * t) * 128 : (2 * t + 1) * 128],
                                rhs=es[:, 0:W],
                                start=(j == 0),
                                stop=last,
                            )
                            nc.tensor.matmul(
                                av[:, 1, n0:512],
                                lhsT=VV[j][:, (2 * t + 1) * 128 : (2 * t + 2) * 128],
                                rhs=es[:, 512 : 512 + W],
                                start=(j == 0),
                                stop=last,
                            )
                        return run

                    # delay AV by one j so the in-order PE stream never
                    # commits to an av-slot wait before the next strips
                    pend_av.append(av_mm(j, n0, W, es))
                    if len(pend_av) > 10:
                        pend_av.pop(0)()
                    # c=0: fillers carry this chunk's own V tiles, which the
                    # next AV emission needs -- never defer them
                    weave(allow=(c == 0 or j < 4 * c + 2))
                while pend_av:
                    pend_av.pop(0)()
                # normalize + extract U^T: per-head reciprocal straight off
                # PSUM partition 0, replicate across partitions, then one
                # multiply per head from PSUM.  Split per head to halve the
                # chain latency (av-slot release gates the next pack's AV).
                # The last pack replicates via an f32r PE matmul instead of
                # the gpsimd broadcast -- the PE is idle in the tail and the
                # matmul is 7x faster than the Pool broadcast.
                rsr = prr.tile([1, 2, 512], f32, tag="rr")
                for hh in (0, 1):
                    nc.vector.reciprocal_approx_fast(
                        rsr[0:1, hh, :], av[0:1, hh, :]
                    )
                    rec = pnrm.tile([64, 512], f32, tag="rec", name=f"rc{hh}")
                    nc.gpsimd.partition_broadcast(rec[:], rsr[0:1, hh, :])
                    nc.vector.tensor_tensor(
                        UT[t][64 * hh : 64 * hh + 64, g0 : g0 + 512],
                        av[64:128, hh, :],
                        rec[:],
                        MUL,
                    )
            # anything not woven (short chunks): emit now
            while weave.done < len(fillers):
                fillers[weave.done]()
                weave.done += 1

        def proj_tail():
            # final chunk's projection, 4-wide (the two strip slots + the mm
            # ring are idle; the AV slot is NOT used -- allocating it would
            # insert a ring-wait on the last norm into the PE stream).  All
            # t=0/t=1 accumulation matmuls run first: they only need the
            # already-normalized UT[0]/UT[1] and keep the PE busy (and the
            # clock-ramp warm) while the last pack's norm chain drains; the
            # 8 t=2 matmuls + drains follow.
            pos = {}
            for i in (12, 13):
                po = pst2.tile([128, 1024], f32, tag="st", name=f"pot{i}")
                pos[i] = [po[:, 0:512], po[:, 512:768]]
                for t in (0, 1):
                    for half, (n0, n1) in enumerate(((0, 512), (512, 768))):
                        nc.tensor.matmul(
                            pos[i][half][:],
                            lhsT=UT[t][:, i * 128 : (i + 1) * 128],
                            rhs=wpt[:, t, n0:n1],
                            start=(t == 0),
                            stop=False,
                        )
            for i in (14, 15):
                pos[i] = [
                    pmm.tile([128, 512], f32, tag="mm", name=f"pot{i}a"),
                    pmm.tile([128, 256], f32, tag="mm", name=f"pot{i}b"),
                ]
                for t in (0, 1):
                    for half in (0, 1):
                        nc.tensor.matmul(
                            pos[i][half][:],
                            lhsT=UT[t][:, i * 128 : (i + 1) * 128],
                            rhs=wpt[:, t, [0, 512][half] : [512, 768][half]],
                            start=(t == 0),
                            stop=False,
                        )
            for i in range(12, 16):
                ob = pout.tile([128, D], bf16, tag="ob", name=f"obt{i}")
                for half, (n0, n1) in enumerate(((0, 512), (512, 768))):
                    nc.tensor.matmul(
                        pos[i][half][:],
                        lhsT=UT[2][:, i * 128 : (i + 1) * 128],
                        rhs=wpt[:, 2, n0:n1],
                        start=False,
                        stop=True,
                    )
                    nc.vector.tensor_copy(ob[:, n0:n1], pos[i][half][:])
                nc.sync.dma_start(out_d[i * 128 : (i + 1) * 128, :], ob[:])

        # ---------------- the program ----------------
        f0 = qkv_fillers(0)
        # f0 order: Q0a,Q0b,K0a,K0b, Q1a,Q1b,K1a,K1b, Q2a,Q2b,K2a,K2b, V0..V3
        # ncI 0 AND 1 must complete before the first strip (the strips read
        # both pair slots of the f4 tiles)
        for f in f0[0:8]:
            f()
        # rest of qkv(0): V tiles early (the first AVs need them) woven with
        # the ncI=2 drains (pack t=1/t=2 strips read the f2 tiles)
        rest0 = [
            f0[12], f0[13], f0[8],   # V0 | Q2a
            f0[14], f0[15], f0[9],   # V1 | Q2b
            f0[16], f0[17], f0[10],  # V2 | K2a
            f0[18], f0[19], f0[11],  # V3 | K2b
        ]
        attention_chunk(0, rest0 + qkv_fillers(1), target_pairs=9)
        for c in range(1, 4):
            fillers = qkv_fillers(c + 1) if c < 3 else []
            fillers += proj_fillers(c - 1)
            attention_chunk(c, fillers)
        proj_tail()

    nc.compile()
    return nc


def _get_nc():
    global _built_nc
    if _built_nc is None:
        _built_nc = _build()
    return _built_nc


def _make_in_maps(x, w_qkv, b_qkv, w_proj):
    import ml_dtypes

    bf16 = ml_dtypes.bfloat16
    in_maps = []
    xTb = [np.ascontiguousarray(x[b].T.astype(bf16)) for b in range(B)]
    for core in range(N_CORES):
        b, hh = core // 2, core % 2
        cs = slice(hh * 384, (hh + 1) * 384)
        # head-interleaved column order for the fp8 DoubleRow layout
        # (1/sqrt(64) is applied in the exp's scale, not here)
        perm = np.concatenate(
            [np.arange(h * 64, h * 64 + 32) for h in range(4)]
            + [np.arange(h * 64 + 32, h * 64 + 64) for h in range(4)]
            + [np.arange(h * 64, h * 64 + 32) for h in (4, 5)]
            + [np.arange(h * 64 + 32, h * 64 + 64) for h in (4, 5)]
        )
        wq = w_qkv[:, 0:768][:, cs][:, perm]
        wk = w_qkv[:, 768:1536][:, cs][:, perm]
        wv = w_qkv[:, 1536:2304][:, cs]
        w_in = np.ascontiguousarray(
            np.concatenate([wq, wk, wv], axis=1).astype(bf16)
        )
        bqv = np.concatenate(
            [
                b_qkv[0:768][cs][perm],
                b_qkv[768:1536][cs][perm],
                b_qkv[1536:2304][cs],
            ]
        ).astype(np.float32)
        wp = np.ascontiguousarray(w_proj[cs, :].astype(bf16))
        in_maps.append(
            {
                "xT_in": xTb[b],
                "w_in": w_in,
                "bqkv_in": bqv,
                "wp_in": wp,
            }
        )
    return in_maps


def _run(x, w_qkv, b_qkv, w_proj, b_proj, trace=False):
    from concourse.bass_utils import run_bass_kernel_spmd

    nc = _get_nc()
    in_maps = _make_in_maps(x, w_qkv, b_qkv, w_proj)
    res = run_bass_kernel_spmd(
        nc, in_maps, core_ids=list(range(N_CORES)), trace=trace
    )
    out = np.zeros((B, S, D), np.float32)
    for core in range(N_CORES):
        out[core // 2] += np.asarray(res.results[core]["out"], np.float32)
    out += np.asarray(b_proj, np.float32)[None, None, :]
    return out, res


def kernel(**inputs):
    x = np.asarray(inputs["x"], np.float32)
    w_qkv = np.asarray(inputs["w_qkv"], np.float32)
    b_qkv = np.asarray(inputs["b_qkv"], np.float32)
    w_proj = np.asarray(inputs["w_proj"], np.float32)
    b_proj = np.asarray(inputs["b_proj"], np.float32)
    out, _ = _run(x, w_qkv, b_qkv, w_proj, b_proj, trace=False)
    return out
